# revision 1
# baseline (speedup 1.0000x reference)
"""BEiT-style windowed attention (B=128, N=197, C=768, H=12) on 8 TRN2 NeuronCores.

Data-parallel over batch: 16 batches per core, 2-batch half-blocks inside
4-batch superblocks. Host pre-processing casts x and the qkv/v/proj weights to
bf16, folds the attention scale into the q weights/bias, folds v_bias into the
projection bias (softmax rows sum to 1), and pre-gathers exp(rel_pos_bias).

Device pipeline per core, per 2-batch half-block:
  qkT  [1536, 394] = qk_wT.T @ xT      (bf16 matmuls, moving dim 394)
  v    [394, 768]  = xT.T @ v_wT       (bf16) with interleaved ones columns
  S.T  [197, 197]  = kT.T @ qT         (bf16 per head; both batches land in one
                                        [128,394] psum as two closed groups)
  E    = exp(S.T) * exp_rb             (one ACT exp per psum — ACT ops have
                                        ~530ns fixed overhead, so fewer+wider
                                        wins; exp(rb) multiply split DVE/Pool;
                                        no max-subtraction: |scores| < ~3)
  outT [128, 197]  = v.T @ E           (heads of a pair stacked at partitions
                                        0:64/64:128 via output col-groups; the
                                        softmax sums land in cols 197:394 of
                                        the same psum via ones-matmuls)
  attnoutT = outT * recip(colsums)     (one ACT reciprocal + one DVE multiply
                                        per pair — DVE recip is 3.2us/op on HW)
  out  = attnoutT.T @ proj_wT + bias   (bf16, projected once per 4-batch
                                        superblock: 7 M-tiles instead of 8;
                                        bias added via a pre-broadcast tensor)
"""
import sys
sys.path.insert(0, '/opt/trn_rl_repo')

import numpy as np
import ml_dtypes
from contextlib import ExitStack

import concourse.bass as bass
import concourse.tile as tile
from concourse.tile import add_dep_helper
from concourse import mybir
from concourse.bass_utils import run_bass_kernel_spmd
from concourse.vector_clock import ScopedClock, VectorClock

f32 = mybir.dt.float32
f32r = mybir.dt.float32r
bf16 = mybir.dt.bfloat16

N_CORES = 8
RB_MODE = "ident_pe"
B, N, C, H, HD = 128, 197, 768, 12, 64
BC = B // N_CORES          # batches per core
BLK = 2                    # batches per block
NB = BC // BLK             # blocks per core
NP = BLK * N               # block column width (394)
SCALE = HD ** -0.5


class TileContextFixed(tile.TileContext):
    """The walrus in this container accepts at most ONE sync wait per
    instruction. Stock Tile attaches several (both on ordinary instructions
    during wait assignment and on the tail drain). Split the extras onto
    same-engine InstNoOps, and emit the tail drain one proc at a time."""

    def _lower_ordered_insts(self, ordered):
        for bb_name, insts in ordered.items():
            i = 0
            while i < len(insts):
                inst = insts[i]
                si = inst.sync_info
                if si is not None and si.on_wait and len(si.on_wait) > 1:
                    waits = list(si.on_wait)
                    inst.sync_info = mybir.SyncInfo(
                        on_wait=[waits[-1]], on_update=list(si.on_update)
                    )
                    nops = [
                        mybir.InstNoOp(
                            name=f"{inst.name}__wsplit{k}",
                            engine=inst.engine,
                            bass_nofuse=True,
                            sync_info=mybir.SyncInfo(on_wait=[w], on_update=[]),
                        )
                        for k, w in enumerate(waits[:-1])
                    ]
                    insts[i:i] = nops
                    i += len(nops)
                i += 1
        return super()._lower_ordered_insts(ordered)

    def _drain_and_barrier(self, tick_clock, wait_clock):
        gc = tick_clock.global_clock
        n = len(gc)
        for i in range(n):
            if gc[i] > 0:
                vc = VectorClock([0] * n)
                vc.require_at_least(i, gc[i])
                d = self.nc.sync.drain()
                wait_clock.add_sem_waits(d.ins, ScopedClock({None: vc}))
        self.nc.all_engine_barrier()
        assert self.sems is not None
        popped = self.nc._tile_sem_poison_stack.pop()
        assert popped is self._sem_poison
        self.nc.clear_and_free_semaphores(list(self.sems.allocated().values()))
        self.nc.all_engine_barrier()


def _act_recip(eng, out, in_):
    imm = lambda v: mybir.ImmediateValue(dtype=f32, value=v)
    return eng.add_instruction(mybir.InstActivation(
        name=eng.bass.get_next_instruction_name(),
        func=mybir.ActivationFunctionType.Reciprocal,
        ins=[eng.lower_ap(in_), imm(0.0), imm(1.0), imm(0.0)],
        outs=[eng.lower_ap(out)],
    ))


def build_nc(rb_mode=RB_MODE, patt_bufs=3, pmm_bufs=3, ppv_bufs=2, e_bufs=10):
    # rb_mode: how exp(S+rb) is formed:
    #   "mul_pool"  E = exp(S) * erb on gpsimd
    #   "mul_dve"   E = exp(S) * erb on DVE
    #   "mul_split" alternate gpsimd/DVE by head parity
    #   "ident_pe"  S += rb via identity matmul on PE, E = exp(S)
    nc = bass.Bass("TRN2", target_bir_lowering=False, debug=False)
    Exp = mybir.ActivationFunctionType.Exp

    xT_d = nc.dram_tensor("xT", [BC, C, N], bf16, kind="ExternalInput").ap()
    qkw_d = nc.dram_tensor("qkw", [C, 2 * C], bf16, kind="ExternalInput").ap()
    vw_d = nc.dram_tensor("vw", [C, C], bf16, kind="ExternalInput").ap()
    pw_d = nc.dram_tensor("pw", [C, C], bf16, kind="ExternalInput").ap()
    pb_d = nc.dram_tensor("pb", [1, C], f32, kind="ExternalInput").ap()
    qb_d = nc.dram_tensor("qb", [128, 6], f32, kind="ExternalInput").ap()

    erb_d = nc.dram_tensor("erb", [H, N, NP], bf16, kind="ExternalInput").ap()
    out_d = nc.dram_tensor("out", [BC * N, C], f32, kind="ExternalOutput").ap()

    MT = ((0, 128), (128, 69))  # (row offset, rows) m-tiles of 197

    with TileContextFixed(nc) as tc, ExitStack() as ctx:
        consts = ctx.enter_context(tc.tile_pool(name="consts", bufs=1))
        xt_p = ctx.enter_context(tc.tile_pool(name="xt", bufs=2))
        qkt_p = ctx.enter_context(tc.tile_pool(name="qkt", bufs=3))
        v_p = ctx.enter_context(tc.tile_pool(name="v", bufs=2))
        at_p = ctx.enter_context(tc.tile_pool(name="at", bufs=3))
        e_p = ctx.enter_context(tc.tile_pool(name="e", bufs=e_bufs))
        rcp_p = ctx.enter_context(tc.tile_pool(name="rcp", bufs=4))
        stage_p = ctx.enter_context(tc.tile_pool(name="stage", bufs=3))
        pmm = ctx.enter_context(tc.tile_pool(name="pmm", bufs=pmm_bufs, space="PSUM"))
        patt = ctx.enter_context(tc.tile_pool(name="patt", bufs=patt_bufs, space="PSUM"))
        ppv = ctx.enter_context(tc.tile_pool(name="ppv", bufs=ppv_bufs, space="PSUM"))

        xt_pre = xt_p.tile([128, 6, NP], bf16)
        for j in range(BLK):
            nc.sync.dma_start(
                out=xt_pre[:, :, j * N:(j + 1) * N],
                in_=xT_d[j].rearrange("(k p) n -> p k n", p=128),
            )
        qkw_s = consts.tile([128, 6, 2 * C], bf16)
        qkw_r = qkw_d.rearrange("(k p) c -> p k c", p=128)
        for k in range(6):
            nc.sync.dma_start(out=qkw_s[:, k, :], in_=qkw_r[:, k, :])
        qb_s = consts.tile([128, 6], f32)
        nc.sync.dma_start(out=qb_s[:], in_=qb_d[:])
        vw_s = consts.tile([128, 6, C], bf16)
        pw_s = consts.tile([128, 6, C], bf16)
        erb0_s = consts.tile([128, H, NP], bf16)
        erb1_s = consts.tile([69, H, NP], bf16)
        pbb_s = consts.tile([128, C], f32)
        ones64 = consts.tile([128, 64], bf16)
        nc.gpsimd.memset(ones64[:], 1.0)

        SB = NB // 2                      # superblocks of 4 batches
        MT7 = [(g, min(128, 2 * NP - g)) for g in range(0, 2 * NP, 128)]

        for sb in range(SB):
            at_s = at_p.tile([128, 6, 2 * NP], bf16)
            for bh in range(2):
                blk = sb * 2 + bh
                b0 = blk * BLK
                off = bh * NP

                if blk == 0:
                    xt_s = xt_pre
                else:
                    xt_s = xt_p.tile([128, 6, NP], bf16)
                    for j in range(BLK):
                        nc.sync.dma_start(
                            out=xt_s[:, :, j * N:(j + 1) * N],
                            in_=xT_d[b0 + j].rearrange("(k p) n -> p k n", p=128),
                        )

                # ---- qkT [12 x 128, NP] bf16 (q part gets scaled bias) ----
                qkt_s = qkt_p.tile([128, H, NP], bf16)
                anchors = {}
                for mi in range(12):
                    ps = pmm.tile([128, NP], f32, tag="pmm")
                    for k in range(6):
                        mm = nc.tensor.matmul(
                            ps[:],
                            lhsT=qkw_s[:, k, mi * 128:(mi + 1) * 128],
                            rhs=xt_s[:, k, :],
                            start=(k == 0), stop=(k == 5),
                        )
                        if blk == 0 and mi in (0, 6) and k == 0:
                            anchors[mi] = mm.ins
                    if mi < 6:
                        nc.vector.tensor_scalar_add(qkt_s[:, mi, :], ps[:], qb_s[:, mi:mi + 1])
                    else:
                        nc.vector.tensor_copy(out=qkt_s[:, mi, :], in_=ps[:])

                if blk == 0:
                    d1 = nc.gpsimd.dma_start(out=vw_s[:], in_=vw_d.rearrange("(k p) c -> p k c", p=128))
                    d2 = nc.gpsimd.dma_start(out=erb0_s[:], in_=erb_d[:, 0:128, :].rearrange("h p n -> p h n"))
                    d3 = nc.gpsimd.dma_start(out=erb1_s[:], in_=erb_d[:, 128:197, :].rearrange("h p n -> p h n"))
                    d4 = nc.gpsimd.dma_start(out=pw_s[:], in_=pw_d.rearrange("(k p) c -> p k c", p=128))
                    d5 = nc.gpsimd.dma_start(out=pbb_s[:], in_=bass.AP(tensor=pb_d.tensor, offset=0,
                                                                       ap=[[0, 128], [1, C]]))
                    for d in (d1, d2, d3):
                        add_dep_helper(d.ins, anchors[0], reason="defer const load past startup")
                    for d in (d4, d5):
                        add_dep_helper(d.ins, anchors[6], reason="defer const load past startup")

                # ---- v natural [NP, 12 heads x 64] bf16 ----
                v_s = v_p.tile([128, BLK, 2, H, 64], bf16)
                for j in range(BLK):
                    for t, (r0, msz) in enumerate(MT):
                        for nt in range(2):
                            ps = pmm.tile([128, 384], f32, tag="pmm")
                            for k in range(6):
                                nc.tensor.matmul(
                                    ps[0:msz, :],
                                    lhsT=xt_s[:, k, j * N + r0: j * N + r0 + msz],
                                    rhs=vw_s[:, k, nt * 384:(nt + 1) * 384],
                                    start=(k == 0), stop=(k == 5),
                                )
                            nc.vector.tensor_copy(
                                out=v_s[0:msz, j, t, nt * 6:(nt + 1) * 6, :],
                                in_=ps[0:msz, :].rearrange("p (h d) -> p h d", h=6),
                            )

                # ---- attention: scores psum holds both batches of the half-block
                # as two CLOSED groups; one exp + one exp(rb)-multiply ----
                for hp in range(6):
                    es = {}
                    for t, (r0, msz) in enumerate(MT):
                        erb_t = erb0_s if t == 0 else erb1_s
                        # Interleave the two heads' K=64 scores matmuls so each
                        # adjacent PE instruction targets a disjoint row group
                        # (0:64 vs 64:128) and the sub-arrays overlap them.
                        ps_a = patt.tile([128, NP], f32, tag="patt")
                        ps_b = patt.tile([128, NP], f32, tag="patt")
                        pss = {0: ps_a, 1: ps_b}
                        for j in range(BLK):
                            for hi in range(2):
                                nc.tensor.matmul(
                                    pss[hi][0:msz, j * N:(j + 1) * N],
                                    lhsT=qkt_s[64 * hi:64 * (hi + 1), 6 + hp,
                                               j * N + r0: j * N + r0 + msz],
                                    rhs=qkt_s[64 * hi:64 * (hi + 1), hp, j * N:(j + 1) * N],
                                    start=True, stop=True, skip_group_check=True,
                                )
                        for hi in range(2):
                            h = 2 * hp + hi
                            e = e_p.tile([128, NP], bf16, tag="e")
                            nc.scalar.activation(out=e[0:msz, :], in_=pss[hi][0:msz, :], func=Exp)
                            eng = nc.gpsimd if hi == 0 else nc.vector
                            eng.tensor_mul(e[0:msz, :], e[0:msz, :], erb_t[0:msz, h, :])
                            es[(t, hi)] = e
                    for j in range(BLK):
                        ps_o = ppv.tile([128, 2 * N], f32, tag="ppv")
                        for hi in range(2):
                            h = 2 * hp + hi
                            for t, (r0, msz) in enumerate(MT):
                                nc.tensor.matmul(
                                    ps_o[hi * 64:(hi + 1) * 64, 0:N],
                                    lhsT=v_s[0:msz, j, t, h, :],
                                    rhs=es[(t, hi)][0:msz, j * N:(j + 1) * N],
                                    start=(t == 0), stop=(t == 1),
                                    skip_group_check=True,
                                )
                        for hi in range(2):
                            for t, (r0, msz) in enumerate(MT):
                                nc.tensor.matmul(
                                    ps_o[hi * 64:(hi + 1) * 64, N:2 * N],
                                    lhsT=ones64[0:msz, :],
                                    rhs=es[(t, hi)][0:msz, j * N:(j + 1) * N],
                                    start=(t == 0), stop=(t == 1),
                                    skip_group_check=True,
                                )
                        rcp = rcp_p.tile([128, N], f32, tag="rcp")
                        _act_recip(nc.scalar, rcp[:], ps_o[:, N:2 * N])
                        nc.vector.tensor_mul(
                            at_s[:, hp, off + j * N:off + (j + 1) * N],
                            ps_o[:, 0:N], rcp[:],
                        )

            # ---- projection over the whole superblock (flat rows), + bias ----
            for g0, msz in MT7:
                stage = stage_p.tile([128, C], f32)
                for nt in range(2):
                    ps = pmm.tile([128, 384], f32, tag="pmm")
                    for k in range(6):
                        nc.tensor.matmul(
                            ps[0:msz, :],
                            lhsT=at_s[:, k, g0:g0 + msz],
                            rhs=pw_s[:, k, nt * 384:(nt + 1) * 384],
                            start=(k == 0), stop=(k == 5),
                        )
                    nc.vector.scalar_tensor_tensor(
                        out=stage[0:msz, nt * 384:(nt + 1) * 384],
                        in0=ps[0:msz, :], scalar=1.0,
                        in1=pbb_s[0:msz, nt * 384:(nt + 1) * 384],
                        op0=mybir.AluOpType.mult, op1=mybir.AluOpType.add,
                    )
                nc.sync.dma_start(
                    out=out_d[sb * 2 * NP + g0: sb * 2 * NP + g0 + msz, :],
                    in_=stage[0:msz, :],
                )
    return nc


_NC = None


def _get_nc():
    global _NC
    if _NC is None:
        _NC = build_nc()
    return _NC


_EXEC = None


def _get_exec():
    """Build the sharded PJRT executable once and reuse it across calls
    (run_bass_via_pjrt re-traces jax.jit on every invocation)."""
    global _EXEC
    if _EXEC is not None:
        return _EXEC
    import jax
    import numpy as _np
    from jax.sharding import Mesh, PartitionSpec
    from jax.experimental.shard_map import shard_map
    import concourse.mybir as mybir_
    from concourse import bass2jax

    nc = _get_nc()
    bass2jax.install_neuronx_cc_hook()
    partition_name = nc.partition_id_tensor.name if nc.partition_id_tensor else None
    in_names, out_names, out_avals = [], [], []
    for alloc in nc.m.functions[0].allocations:
        if not isinstance(alloc, mybir_.MemoryLocationSet):
            continue
        name = alloc.memorylocations[0].name
        if alloc.kind == "ExternalInput":
            if name != partition_name:
                in_names.append(name)
        elif alloc.kind == "ExternalOutput":
            out_names.append(name)
            out_avals.append(jax.core.ShapedArray(
                tuple(alloc.tensor_shape), mybir_.dt.np(alloc.dtype)))
    all_names = list(in_names)
    if partition_name is not None:
        all_names = all_names + [partition_name]

    def _body(*args):
        operands = list(args)
        if partition_name is not None:
            operands.append(bass2jax.partition_id_tensor())
        outs = bass2jax._bass_exec_p.bind(
            *operands,
            out_avals=tuple(out_avals),
            in_names=tuple(all_names),
            out_names=tuple(out_names),
            lowering_input_output_aliases=(),
            sim_require_finite=True,
            sim_require_nnan=True,
            nc=nc,
        )
        return tuple(outs)

    devices = jax.devices()[:N_CORES]
    mesh = Mesh(_np.asarray(devices), ("core",))
    # xT is data-parallel (split on axis 0); every other input is replicated,
    # so it uploads once instead of 8x.
    in_specs = tuple(
        PartitionSpec("core") if name == "xT" else PartitionSpec()
        for name in in_names
    )
    out_specs = (PartitionSpec("core"),) * len(out_avals)
    sharded = jax.jit(
        shard_map(_body, mesh=mesh, in_specs=in_specs, out_specs=out_specs,
                  check_rep=False),
        keep_unused=True,
    )
    _EXEC = (sharded, in_names, out_names, out_avals)
    return _EXEC


def _prep_host(x, qkv_w, q_bias, v_bias, rel_pos_table, proj_w, proj_b, rel_index,
               rb_mode="mul_pool"):
    x = np.asarray(x, np.float32)
    qkv_w = np.asarray(qkv_w, np.float32)
    xT = np.ascontiguousarray(x.transpose(0, 2, 1)).astype(ml_dtypes.bfloat16)
    qk_wT = np.ascontiguousarray(qkv_w[:2 * C].T)              # [C, 2C]
    qk_wT[:, :C] *= SCALE
    qk_wT = qk_wT.astype(ml_dtypes.bfloat16)
    qb = (np.asarray(q_bias, np.float32) * SCALE).reshape(6, 128).T.copy()  # [128, 6]
    v_wT = np.ascontiguousarray(qkv_w[2 * C:].T).astype(ml_dtypes.bfloat16)
    proj_wT = np.ascontiguousarray(np.asarray(proj_w, np.float32).T).astype(ml_dtypes.bfloat16)
    pb_eff = (np.asarray(proj_b, np.float32)
              + np.asarray(proj_w, np.float32) @ np.asarray(v_bias, np.float32))
    rb = np.asarray(rel_pos_table, np.float32)[
        np.asarray(rel_index).reshape(-1)].reshape(N, N, H)    # [n, m, h]
    rbT = np.exp(rb.transpose(2, 1, 0))
    rbT = np.concatenate([rbT] * BLK, axis=2)
    erbT = rbT.astype(ml_dtypes.bfloat16)
    return xT, qk_wT, qb, v_wT, proj_wT, pb_eff.reshape(1, C), erbT


def kernel(x, qkv_w, q_bias, v_bias, rel_pos_table, proj_w, proj_b, rel_index):
    xT, qk_wT, qb, v_wT, proj_wT, pb_eff, erbT = _prep_host(
        x, qkv_w, q_bias, v_bias, rel_pos_table, proj_w, proj_b, rel_index,
        rb_mode=RB_MODE)
    per_core = {
        "xT": xT,                                   # [B, C, N] -> split on axis 0
        "qkw": qk_wT, "vw": v_wT, "pw": proj_wT,
        "pb": pb_eff, "qb": qb, "erb": erbT,
    }
    try:
        sharded, in_names, out_names, out_avals = _get_exec()
        concat_in = [np.ascontiguousarray(per_core[name]) for name in in_names]
        out_arrs = sharded(*concat_in)
        out = np.asarray(out_arrs[out_names.index("out")]).reshape(B, N, C)
    except Exception:
        # Robust fallback: the stock SPMD runner (slower per call, same NEFF).
        in_maps = []
        for c in range(N_CORES):
            m = {k: v for k, v in per_core.items() if k != "xT"}
            m["xT"] = np.ascontiguousarray(xT[c * BC:(c + 1) * BC])
            in_maps.append(m)
        res = run_bass_kernel_spmd(_get_nc(), in_maps, core_ids=list(range(N_CORES)))
        out = np.concatenate(
            [res.results[c]["out"].reshape(BC, N, C) for c in range(N_CORES)], axis=0)
    return out.astype(np.float32)



# revision 14
# speedup vs baseline: 1.0642x; 1.0642x over previous
"""BEiT-style windowed attention (B=128, N=197, C=768, H=12) on 8 TRN2 NeuronCores.

Data-parallel over batch: 16 batches per core, 2-batch half-blocks inside
4-batch superblocks. Host pre-processing casts x and the qkv/v/proj weights to
bf16, folds the attention scale into the q weights/bias, folds v_bias into the
projection bias (softmax rows sum to 1), and pre-gathers exp(rel_pos_bias).

Device pipeline per core, per 2-batch half-block:
  qkT  [1536, 394] = qk_wT.T @ xT      (bf16 matmuls, moving dim 394)
  v    [394, 768]  = xT.T @ v_wT       (bf16) with interleaved ones columns
  S.T  [197, 197]  = kT.T @ qT         (bf16 per head; both batches land in one
                                        [128,394] psum as two closed groups)
  E    = exp(S.T) * exp_rb             (one ACT exp per psum — ACT ops have
                                        ~530ns fixed overhead, so fewer+wider
                                        wins; exp(rb) multiply split DVE/Pool;
                                        no max-subtraction: |scores| < ~3)
  outT [128, 197]  = v.T @ E           (heads of a pair stacked at partitions
                                        0:64/64:128 via output col-groups; the
                                        softmax sums land in cols 197:394 of
                                        the same psum via ones-matmuls)
  attnoutT = outT * recip(colsums)     (one ACT reciprocal + one DVE multiply
                                        per pair — DVE recip is 3.2us/op on HW)
  out  = attnoutT.T @ proj_wT + bias   (bf16, projected once per 4-batch
                                        superblock: 7 M-tiles instead of 8;
                                        bias added via a pre-broadcast tensor)
"""
import sys
sys.path.insert(0, '/opt/trn_rl_repo')

import numpy as np
import ml_dtypes
from contextlib import ExitStack

import concourse.bass as bass
import concourse.tile as tile
from concourse.tile import add_dep_helper
from concourse import mybir
from concourse.bass_utils import run_bass_kernel_spmd
from concourse.vector_clock import ScopedClock, VectorClock

f32 = mybir.dt.float32
f32r = mybir.dt.float32r
bf16 = mybir.dt.bfloat16
f8 = mybir.dt.float8e4
DR = mybir.MatmulPerfMode.DoubleRow

N_CORES = 8
RB_MODE = "ident_pe"
B, N, C, H, HD = 128, 197, 768, 12, 64
BC = B // N_CORES          # batches per core
BLK = 2                    # batches per block
NB = BC // BLK             # blocks per core
NP = BLK * N               # block column width (394)
SCALE = HD ** -0.5
QS = 64.0                  # fp8 weight pre-scale for the qk gemm
EXP_SCALE = SCALE / (QS * QS)


class TileContextFixed(tile.TileContext):
    """The walrus in this container accepts at most ONE sync wait per
    instruction. Stock Tile attaches several (both on ordinary instructions
    during wait assignment and on the tail drain). Split the extras onto
    same-engine InstNoOps, and emit the tail drain one proc at a time."""

    def _lower_ordered_insts(self, ordered):
        for bb_name, insts in ordered.items():
            i = 0
            while i < len(insts):
                inst = insts[i]
                si = inst.sync_info
                if si is not None and si.on_wait and len(si.on_wait) > 1:
                    waits = list(si.on_wait)
                    inst.sync_info = mybir.SyncInfo(
                        on_wait=[waits[-1]], on_update=list(si.on_update)
                    )
                    nops = [
                        mybir.InstNoOp(
                            name=f"{inst.name}__wsplit{k}",
                            engine=inst.engine,
                            bass_nofuse=True,
                            sync_info=mybir.SyncInfo(on_wait=[w], on_update=[]),
                        )
                        for k, w in enumerate(waits[:-1])
                    ]
                    insts[i:i] = nops
                    i += len(nops)
                i += 1
        return super()._lower_ordered_insts(ordered)

    def _drain_and_barrier(self, tick_clock, wait_clock):
        gc = tick_clock.global_clock
        n = len(gc)
        for i in range(n):
            if gc[i] > 0:
                vc = VectorClock([0] * n)
                vc.require_at_least(i, gc[i])
                d = self.nc.sync.drain()
                wait_clock.add_sem_waits(d.ins, ScopedClock({None: vc}))
        self.nc.all_engine_barrier()
        assert self.sems is not None
        popped = self.nc._tile_sem_poison_stack.pop()
        assert popped is self._sem_poison
        self.nc.clear_and_free_semaphores(list(self.sems.allocated().values()))
        self.nc.all_engine_barrier()


def _act_recip(eng, out, in_):
    imm = lambda v: mybir.ImmediateValue(dtype=f32, value=v)
    return eng.add_instruction(mybir.InstActivation(
        name=eng.bass.get_next_instruction_name(),
        func=mybir.ActivationFunctionType.Reciprocal,
        ins=[eng.lower_ap(in_), imm(0.0), imm(1.0), imm(0.0)],
        outs=[eng.lower_ap(out)],
    ))


def build_nc(rb_mode=RB_MODE, patt_bufs=3, pmm_bufs=3, ppv_bufs=2, e_bufs=10):
    # rb_mode: how exp(S+rb) is formed:
    #   "mul_pool"  E = exp(S) * erb on gpsimd
    #   "mul_dve"   E = exp(S) * erb on DVE
    #   "mul_split" alternate gpsimd/DVE by head parity
    #   "ident_pe"  S += rb via identity matmul on PE, E = exp(S)
    nc = bass.Bass("TRN2", target_bir_lowering=False, debug=False)
    Exp = mybir.ActivationFunctionType.Exp

    xT_d = nc.dram_tensor("xT", [BC, C, N], bf16, kind="ExternalInput").ap()
    xT8_d = nc.dram_tensor("xT8", [BC, 2, C, N], f8, kind="ExternalInput").ap()
    qkw_d = nc.dram_tensor("qkw", [C, 2 * C], f8, kind="ExternalInput").ap()
    vw_d = nc.dram_tensor("vw", [C, C], bf16, kind="ExternalInput").ap()
    pw_d = nc.dram_tensor("pw", [C, C], bf16, kind="ExternalInput").ap()
    pb_d = nc.dram_tensor("pb", [1, C], f32, kind="ExternalInput").ap()
    qb_d = nc.dram_tensor("qb", [128, 6], f32, kind="ExternalInput").ap()

    erb_d = nc.dram_tensor("erb", [H, N, NP], bf16, kind="ExternalInput").ap()
    out_d = nc.dram_tensor("out", [BC * N, C], f32, kind="ExternalOutput").ap()

    MT = ((0, 128), (128, 69))  # (row offset, rows) m-tiles of 197

    with TileContextFixed(nc) as tc, ExitStack() as ctx:
        consts = ctx.enter_context(tc.tile_pool(name="consts", bufs=1))
        xt_p = ctx.enter_context(tc.tile_pool(name="xt", bufs=2))
        xt8_p = ctx.enter_context(tc.tile_pool(name="xt8", bufs=2))
        qkt_p = ctx.enter_context(tc.tile_pool(name="qkt", bufs=3))
        v_p = ctx.enter_context(tc.tile_pool(name="v", bufs=2))
        at_p = ctx.enter_context(tc.tile_pool(name="at", bufs=3))
        e_p = ctx.enter_context(tc.tile_pool(name="e", bufs=e_bufs))
        rcp_p = ctx.enter_context(tc.tile_pool(name="rcp", bufs=4))
        stage_p = ctx.enter_context(tc.tile_pool(name="stage", bufs=3))
        pmm = ctx.enter_context(tc.tile_pool(name="pmm", bufs=pmm_bufs, space="PSUM"))
        patt = ctx.enter_context(tc.tile_pool(name="patt", bufs=patt_bufs, space="PSUM"))
        ppv = ctx.enter_context(tc.tile_pool(name="ppv", bufs=ppv_bufs, space="PSUM"))

        qkw_s = consts.tile([128, 6, 2 * C], f8)
        qkw_r = qkw_d.rearrange("(k p) c -> p k c", p=128)
        # mi0's weight chunk first so the first matmul can start ASAP
        nc.sync.dma_start(out=qkw_s[:, :, 0:128], in_=qkw_r[:, :, 0:128])
        xt8_pre = xt8_p.tile([128, 2, 6, NP], f8)
        for s in range(2):
            for j in range(BLK):
                nc.sync.dma_start(
                    out=xt8_pre[:, s, :, j * N:(j + 1) * N],
                    in_=xT8_d[j][s].rearrange("(k p) n -> p k n", p=128),
                )
        for mi in range(1, 12):
            nc.sync.dma_start(out=qkw_s[:, :, mi * 128:(mi + 1) * 128],
                              in_=qkw_r[:, :, mi * 128:(mi + 1) * 128])
        xt_pre = xt_p.tile([128, 6, NP], bf16)
        for j in range(BLK):
            nc.scalar.dma_start(
                out=xt_pre[:, :, j * N:(j + 1) * N],
                in_=xT_d[j].rearrange("(k p) n -> p k n", p=128),
            )
        qb_s = consts.tile([128, 6], f32)
        nc.sync.dma_start(out=qb_s[:], in_=qb_d[:])
        vw_s = consts.tile([128, 6, C], bf16)
        pw_s = consts.tile([128, 6, C], bf16)
        erb0_s = consts.tile([128, H, NP], bf16)
        erb1_s = consts.tile([69, H, NP], bf16)
        pbb_s = consts.tile([128, C], f32)
        ones64 = consts.tile([128, 64], bf16)
        nc.gpsimd.memset(ones64[:], 1.0)

        SB = NB // 2                      # superblocks of 4 batches
        MT7 = [(g, min(128, 2 * NP - g)) for g in range(0, 2 * NP, 128)]

        for sb in range(SB):
            at_s = at_p.tile([128, 6, 2 * NP], bf16)
            for bh in range(2):
                blk = sb * 2 + bh
                b0 = blk * BLK
                off = bh * NP

                if blk == 0:
                    xt_s = xt_pre
                    xt8_s = xt8_pre
                else:
                    xt_s = xt_p.tile([128, 6, NP], bf16)
                    xt8_s = xt8_p.tile([128, 2, 6, NP], f8)
                    for s in range(2):
                        for j in range(BLK):
                            nc.sync.dma_start(
                                out=xt8_s[:, s, :, j * N:(j + 1) * N],
                                in_=xT8_d[b0 + j][s].rearrange("(k p) n -> p k n", p=128),
                            )
                    for j in range(BLK):
                        nc.sync.dma_start(
                            out=xt_s[:, :, j * N:(j + 1) * N],
                            in_=xT_d[b0 + j].rearrange("(k p) n -> p k n", p=128),
                        )

                # ---- qkT [12 x 128, NP] bf16 via split-fp8 DoubleRow gemm;
                # q/k scaled by QS=64, rescale folded into the exp ----
                qkt_s = qkt_p.tile([128, H, NP], bf16)
                anchors = {}
                for mi in range(12):
                    ps = pmm.tile([128, NP], f32, tag="pmm")
                    for s in range(2):
                        for t in range(3):
                            mm = nc.tensor.matmul(
                                ps[:],
                                lhsT=qkw_s[:, 2 * t:2 * t + 2, mi * 128:(mi + 1) * 128],
                                rhs=xt8_s[:, s, 2 * t:2 * t + 2, :],
                                start=(s == 0 and t == 0), stop=(s == 1 and t == 2),
                                perf_mode=DR,
                            )
                            if blk == 0 and mi in (0, 6) and s == 0 and t == 0:
                                anchors[mi] = mm.ins
                    if mi < 6:
                        nc.vector.tensor_scalar_add(qkt_s[:, mi, :], ps[:], qb_s[:, mi:mi + 1])
                    else:
                        nc.vector.tensor_copy(out=qkt_s[:, mi, :], in_=ps[:])

                if blk == 0:
                    d1 = nc.gpsimd.dma_start(out=vw_s[:], in_=vw_d.rearrange("(k p) c -> p k c", p=128))
                    d2 = nc.gpsimd.dma_start(out=erb0_s[:], in_=erb_d[:, 0:128, :].rearrange("h p n -> p h n"))
                    d3 = nc.gpsimd.dma_start(out=erb1_s[:], in_=erb_d[:, 128:197, :].rearrange("h p n -> p h n"))
                    d4 = nc.gpsimd.dma_start(out=pw_s[:], in_=pw_d.rearrange("(k p) c -> p k c", p=128))
                    d5 = nc.gpsimd.dma_start(out=pbb_s[:], in_=bass.AP(tensor=pb_d.tensor, offset=0,
                                                                       ap=[[0, 128], [1, C]]))
                    for d in (d1, d2, d3):
                        add_dep_helper(d.ins, anchors[0], reason="defer const load past startup")
                    for d in (d4, d5):
                        add_dep_helper(d.ins, anchors[6], reason="defer const load past startup")

                # ---- v natural [NP, 12 heads x 64] bf16 ----
                v_s = v_p.tile([128, BLK, 2, H, 64], bf16)
                for j in range(BLK):
                    for t, (r0, msz) in enumerate(MT):
                        for nt in range(2):
                            ps = pmm.tile([128, 384], f32, tag="pmm")
                            for k in range(6):
                                nc.tensor.matmul(
                                    ps[0:msz, :],
                                    lhsT=xt_s[:, k, j * N + r0: j * N + r0 + msz],
                                    rhs=vw_s[:, k, nt * 384:(nt + 1) * 384],
                                    start=(k == 0), stop=(k == 5),
                                )
                            nc.vector.tensor_copy(
                                out=v_s[0:msz, j, t, nt * 6:(nt + 1) * 6, :],
                                in_=ps[0:msz, :].rearrange("p (h d) -> p h d", h=6),
                            )

                # ---- attention: scores psum holds both batches of the half-block
                # as two CLOSED groups; one exp + one exp(rb)-multiply ----
                for hp in range(6):
                    es = {}
                    for t, (r0, msz) in enumerate(MT):
                        erb_t = erb0_s if t == 0 else erb1_s
                        # Interleave the two heads' K=64 scores matmuls so each
                        # adjacent PE instruction targets a disjoint row group
                        # (0:64 vs 64:128) and the sub-arrays overlap them.
                        ps_a = patt.tile([128, NP], f32, tag="patt")
                        ps_b = patt.tile([128, NP], f32, tag="patt")
                        pss = {0: ps_a, 1: ps_b}
                        for j in range(BLK):
                            for hi in range(2):
                                nc.tensor.matmul(
                                    pss[hi][0:msz, j * N:(j + 1) * N],
                                    lhsT=qkt_s[64 * hi:64 * (hi + 1), 6 + hp,
                                               j * N + r0: j * N + r0 + msz],
                                    rhs=qkt_s[64 * hi:64 * (hi + 1), hp, j * N:(j + 1) * N],
                                    start=True, stop=True, skip_group_check=True,
                                )
                        for hi in range(2):
                            h = 2 * hp + hi
                            e = e_p.tile([128, NP], bf16, tag="e")
                            nc.scalar.activation(out=e[0:msz, :], in_=pss[hi][0:msz, :],
                                                 func=Exp, scale=EXP_SCALE)
                            eng = nc.gpsimd if hi == 0 else nc.vector
                            eng.tensor_mul(e[0:msz, :], e[0:msz, :], erb_t[0:msz, h, :])
                            es[(t, hi)] = e
                    for j in range(BLK):
                        ps_o = ppv.tile([128, 2 * N], f32, tag="ppv")
                        for hi in range(2):
                            h = 2 * hp + hi
                            for t, (r0, msz) in enumerate(MT):
                                nc.tensor.matmul(
                                    ps_o[hi * 64:(hi + 1) * 64, 0:N],
                                    lhsT=v_s[0:msz, j, t, h, :],
                                    rhs=es[(t, hi)][0:msz, j * N:(j + 1) * N],
                                    start=(t == 0), stop=(t == 1),
                                    skip_group_check=True,
                                )
                        for hi in range(2):
                            for t, (r0, msz) in enumerate(MT):
                                nc.tensor.matmul(
                                    ps_o[hi * 64:(hi + 1) * 64, N:2 * N],
                                    lhsT=ones64[0:msz, :],
                                    rhs=es[(t, hi)][0:msz, j * N:(j + 1) * N],
                                    start=(t == 0), stop=(t == 1),
                                    skip_group_check=True,
                                )
                        rcp = rcp_p.tile([128, N], f32, tag="rcp")
                        _act_recip(nc.scalar, rcp[:], ps_o[:, N:2 * N])
                        nc.vector.tensor_mul(
                            at_s[:, hp, off + j * N:off + (j + 1) * N],
                            ps_o[:, 0:N], rcp[:],
                        )

            # ---- projection over the whole superblock (flat rows), + bias ----
            for g0, msz in MT7:
                stage = stage_p.tile([128, C], f32)
                for nt in range(2):
                    ps = pmm.tile([128, 384], f32, tag="pmm")
                    for k in range(6):
                        nc.tensor.matmul(
                            ps[0:msz, :],
                            lhsT=at_s[:, k, g0:g0 + msz],
                            rhs=pw_s[:, k, nt * 384:(nt + 1) * 384],
                            start=(k == 0), stop=(k == 5),
                        )
                    nc.vector.scalar_tensor_tensor(
                        out=stage[0:msz, nt * 384:(nt + 1) * 384],
                        in0=ps[0:msz, :], scalar=1.0,
                        in1=pbb_s[0:msz, nt * 384:(nt + 1) * 384],
                        op0=mybir.AluOpType.mult, op1=mybir.AluOpType.add,
                    )
                nc.sync.dma_start(
                    out=out_d[sb * 2 * NP + g0: sb * 2 * NP + g0 + msz, :],
                    in_=stage[0:msz, :],
                )
    return nc


_NC = None


def _get_nc():
    global _NC
    if _NC is None:
        _NC = build_nc()
    return _NC


_EXEC = None


def _get_exec():
    """Build the sharded PJRT executable once and reuse it across calls
    (run_bass_via_pjrt re-traces jax.jit on every invocation)."""
    global _EXEC
    if _EXEC is not None:
        return _EXEC
    import jax
    import numpy as _np
    from jax.sharding import Mesh, PartitionSpec
    from jax.experimental.shard_map import shard_map
    import concourse.mybir as mybir_
    from concourse import bass2jax

    nc = _get_nc()
    bass2jax.install_neuronx_cc_hook()
    partition_name = nc.partition_id_tensor.name if nc.partition_id_tensor else None
    in_names, out_names, out_avals = [], [], []
    for alloc in nc.m.functions[0].allocations:
        if not isinstance(alloc, mybir_.MemoryLocationSet):
            continue
        name = alloc.memorylocations[0].name
        if alloc.kind == "ExternalInput":
            if name != partition_name:
                in_names.append(name)
        elif alloc.kind == "ExternalOutput":
            out_names.append(name)
            out_avals.append(jax.core.ShapedArray(
                tuple(alloc.tensor_shape), mybir_.dt.np(alloc.dtype)))
    all_names = list(in_names)
    if partition_name is not None:
        all_names = all_names + [partition_name]

    def _body(*args):
        operands = list(args)
        if partition_name is not None:
            operands.append(bass2jax.partition_id_tensor())
        outs = bass2jax._bass_exec_p.bind(
            *operands,
            out_avals=tuple(out_avals),
            in_names=tuple(all_names),
            out_names=tuple(out_names),
            lowering_input_output_aliases=(),
            sim_require_finite=True,
            sim_require_nnan=True,
            nc=nc,
        )
        return tuple(outs)

    devices = jax.devices()[:N_CORES]
    mesh = Mesh(_np.asarray(devices), ("core",))
    # xT is data-parallel (split on axis 0); every other input is replicated,
    # so it uploads once instead of 8x.
    in_specs = tuple(
        PartitionSpec("core") if name in ("xT", "xT8") else PartitionSpec()
        for name in in_names
    )
    out_specs = (PartitionSpec("core"),) * len(out_avals)
    sharded = jax.jit(
        shard_map(_body, mesh=mesh, in_specs=in_specs, out_specs=out_specs,
                  check_rep=False),
        keep_unused=True,
    )
    _EXEC = (sharded, in_names, out_names, out_avals)
    return _EXEC


def _prep_host(x, qkv_w, q_bias, v_bias, rel_pos_table, proj_w, proj_b, rel_index,
               rb_mode="mul_pool"):
    x = np.asarray(x, np.float32)
    qkv_w = np.asarray(qkv_w, np.float32)
    xT32 = np.ascontiguousarray(x.transpose(0, 2, 1))
    xT = xT32.astype(ml_dtypes.bfloat16)
    x8h = xT32.astype(ml_dtypes.float8_e4m3)
    x8l = (xT32 - x8h.astype(np.float32)).astype(ml_dtypes.float8_e4m3)
    xT8 = np.ascontiguousarray(np.stack([x8h, x8l], axis=1))   # [B, 2, C, N]
    qk_wT = np.ascontiguousarray(qkv_w[:2 * C].T) * QS         # [C, 2C]
    qk_wT = qk_wT.astype(ml_dtypes.float8_e4m3)
    qb = (np.asarray(q_bias, np.float32) * QS).reshape(6, 128).T.copy()  # [128, 6]
    v_wT = np.ascontiguousarray(qkv_w[2 * C:].T).astype(ml_dtypes.bfloat16)
    proj_wT = np.ascontiguousarray(np.asarray(proj_w, np.float32).T).astype(ml_dtypes.bfloat16)
    pb_eff = (np.asarray(proj_b, np.float32)
              + np.asarray(proj_w, np.float32) @ np.asarray(v_bias, np.float32))
    rb = np.asarray(rel_pos_table, np.float32)[
        np.asarray(rel_index).reshape(-1)].reshape(N, N, H)    # [n, m, h]
    rbT = np.exp(rb.transpose(2, 1, 0))
    rbT = np.concatenate([rbT] * BLK, axis=2)
    erbT = rbT.astype(ml_dtypes.bfloat16)
    return xT, xT8, qk_wT, qb, v_wT, proj_wT, pb_eff.reshape(1, C), erbT


def kernel(x, qkv_w, q_bias, v_bias, rel_pos_table, proj_w, proj_b, rel_index):
    xT, xT8, qk_wT, qb, v_wT, proj_wT, pb_eff, erbT = _prep_host(
        x, qkv_w, q_bias, v_bias, rel_pos_table, proj_w, proj_b, rel_index,
        rb_mode=RB_MODE)
    per_core = {
        "xT": xT, "xT8": xT8,                       # split on axis 0
        "qkw": qk_wT, "vw": v_wT, "pw": proj_wT,
        "pb": pb_eff, "qb": qb, "erb": erbT,
    }
    try:
        sharded, in_names, out_names, out_avals = _get_exec()
        concat_in = [np.ascontiguousarray(per_core[name]) for name in in_names]
        out_arrs = sharded(*concat_in)
        out = np.asarray(out_arrs[out_names.index("out")]).reshape(B, N, C)
    except Exception:
        # Robust fallback: the stock SPMD runner (slower per call, same NEFF).
        in_maps = []
        for c in range(N_CORES):
            m = {k: v for k, v in per_core.items() if k not in ("xT", "xT8")}
            m["xT"] = np.ascontiguousarray(xT[c * BC:(c + 1) * BC])
            m["xT8"] = np.ascontiguousarray(xT8[c * BC:(c + 1) * BC])
            in_maps.append(m)
        res = run_bass_kernel_spmd(_get_nc(), in_maps, core_ids=list(range(N_CORES)))
        out = np.concatenate(
            [res.results[c]["out"].reshape(BC, N, C) for c in range(N_CORES)], axis=0)
    return out.astype(np.float32)



# revision 19
# speedup vs baseline: 1.0856x; 1.0202x over previous
"""BEiT-style windowed attention (B=128, N=197, C=768, H=12) on 8 TRN2 NeuronCores.

Data-parallel over batch: 16 batches per core, 2-batch half-blocks inside
4-batch superblocks. Host pre-processing casts x and the qkv/v/proj weights to
bf16, folds the attention scale into the q weights/bias, folds v_bias into the
projection bias (softmax rows sum to 1), and pre-gathers exp(rel_pos_bias).

Device pipeline per core, per 2-batch half-block:
  qkT  [1536, 394] = qk_wT.T @ xT      (bf16 matmuls, moving dim 394)
  v    [394, 768]  = xT.T @ v_wT       (bf16) with interleaved ones columns
  S.T  [197, 197]  = kT.T @ qT         (bf16 per head; both batches land in one
                                        [128,394] psum as two closed groups)
  E    = exp(S.T) * exp_rb             (one ACT exp per psum — ACT ops have
                                        ~530ns fixed overhead, so fewer+wider
                                        wins; exp(rb) multiply split DVE/Pool;
                                        no max-subtraction: |scores| < ~3)
  outT [128, 197]  = v.T @ E           (heads of a pair stacked at partitions
                                        0:64/64:128 via output col-groups; the
                                        softmax sums land in cols 197:394 of
                                        the same psum via ones-matmuls)
  attnoutT = outT * recip(colsums)     (one ACT reciprocal + one DVE multiply
                                        per pair — DVE recip is 3.2us/op on HW)
  out  = attnoutT.T @ proj_wT + bias   (bf16, projected once per 4-batch
                                        superblock: 7 M-tiles instead of 8;
                                        bias added via a pre-broadcast tensor)
"""
import sys
sys.path.insert(0, '/opt/trn_rl_repo')

import numpy as np
import ml_dtypes
from contextlib import ExitStack

import concourse.bass as bass
import concourse.tile as tile
from concourse.tile import add_dep_helper
from concourse import mybir
from concourse.bass_utils import run_bass_kernel_spmd
from concourse.vector_clock import ScopedClock, VectorClock

f32 = mybir.dt.float32
f32r = mybir.dt.float32r
bf16 = mybir.dt.bfloat16
f8 = mybir.dt.float8e4
DR = mybir.MatmulPerfMode.DoubleRow

N_CORES = 8
RB_MODE = "ident_pe"
B, N, C, H, HD = 128, 197, 768, 12, 64
BC = B // N_CORES          # batches per core
BLK = 2                    # batches per block
NB = BC // BLK             # blocks per core
NP = BLK * N               # block column width (394)
SCALE = HD ** -0.5
QS = 64.0                  # fp8 weight pre-scale for the qk gemm
EXP_SCALE = SCALE / (QS * QS)


class TileContextFixed(tile.TileContext):
    """The walrus in this container accepts at most ONE sync wait per
    instruction. Stock Tile attaches several (both on ordinary instructions
    during wait assignment and on the tail drain). Split the extras onto
    same-engine InstNoOps, and emit the tail drain one proc at a time."""

    def _lower_ordered_insts(self, ordered):
        for bb_name, insts in ordered.items():
            i = 0
            while i < len(insts):
                inst = insts[i]
                si = inst.sync_info
                if si is not None and si.on_wait and len(si.on_wait) > 1:
                    waits = list(si.on_wait)
                    inst.sync_info = mybir.SyncInfo(
                        on_wait=[waits[-1]], on_update=list(si.on_update)
                    )
                    nops = [
                        mybir.InstNoOp(
                            name=f"{inst.name}__wsplit{k}",
                            engine=inst.engine,
                            bass_nofuse=True,
                            sync_info=mybir.SyncInfo(on_wait=[w], on_update=[]),
                        )
                        for k, w in enumerate(waits[:-1])
                    ]
                    insts[i:i] = nops
                    i += len(nops)
                i += 1
        return super()._lower_ordered_insts(ordered)

    def _drain_and_barrier(self, tick_clock, wait_clock):
        gc = tick_clock.global_clock
        n = len(gc)
        for i in range(n):
            if gc[i] > 0:
                vc = VectorClock([0] * n)
                vc.require_at_least(i, gc[i])
                d = self.nc.sync.drain()
                wait_clock.add_sem_waits(d.ins, ScopedClock({None: vc}))
        self.nc.all_engine_barrier()
        assert self.sems is not None
        popped = self.nc._tile_sem_poison_stack.pop()
        assert popped is self._sem_poison
        self.nc.clear_and_free_semaphores(list(self.sems.allocated().values()))
        self.nc.all_engine_barrier()


def _act_recip(eng, out, in_):
    imm = lambda v: mybir.ImmediateValue(dtype=f32, value=v)
    return eng.add_instruction(mybir.InstActivation(
        name=eng.bass.get_next_instruction_name(),
        func=mybir.ActivationFunctionType.Reciprocal,
        ins=[eng.lower_ap(in_), imm(0.0), imm(1.0), imm(0.0)],
        outs=[eng.lower_ap(out)],
    ))


def build_nc(rb_mode=RB_MODE, patt_bufs=3, pmm_bufs=3, ppv_bufs=2, e_bufs=10):
    # rb_mode: how exp(S+rb) is formed:
    #   "mul_pool"  E = exp(S) * erb on gpsimd
    #   "mul_dve"   E = exp(S) * erb on DVE
    #   "mul_split" alternate gpsimd/DVE by head parity
    #   "ident_pe"  S += rb via identity matmul on PE, E = exp(S)
    nc = bass.Bass("TRN2", target_bir_lowering=False, debug=False)
    Exp = mybir.ActivationFunctionType.Exp

    xT_d = nc.dram_tensor("xT", [NB, 128, 6, NP], bf16, kind="ExternalInput").ap()
    xT8_d = nc.dram_tensor("xT8", [NB, 128, 2, 6, NP], f8, kind="ExternalInput").ap()
    qkw_d = nc.dram_tensor("qkw", [C, 2 * C], f8, kind="ExternalInput").ap()
    vw_d = nc.dram_tensor("vw", [C, C], bf16, kind="ExternalInput").ap()
    pw_d = nc.dram_tensor("pw", [C, C], bf16, kind="ExternalInput").ap()
    pb_d = nc.dram_tensor("pb", [1, C], f32, kind="ExternalInput").ap()
    qb_d = nc.dram_tensor("qb", [128, 6], f32, kind="ExternalInput").ap()

    erb_d = nc.dram_tensor("erb", [H, N, NP], bf16, kind="ExternalInput").ap()
    out_d = nc.dram_tensor("out", [BC * N, C], f32, kind="ExternalOutput").ap()

    MT = ((0, 128), (128, 69))  # (row offset, rows) m-tiles of 197

    with TileContextFixed(nc) as tc, ExitStack() as ctx:
        consts = ctx.enter_context(tc.tile_pool(name="consts", bufs=1))
        xt_p = ctx.enter_context(tc.tile_pool(name="xt", bufs=2))
        xt8_p = ctx.enter_context(tc.tile_pool(name="xt8", bufs=2))
        qkt_p = ctx.enter_context(tc.tile_pool(name="qkt", bufs=3))
        v_p = ctx.enter_context(tc.tile_pool(name="v", bufs=2))
        at_p = ctx.enter_context(tc.tile_pool(name="at", bufs=3))
        e_p = ctx.enter_context(tc.tile_pool(name="e", bufs=e_bufs))
        rcp_p = ctx.enter_context(tc.tile_pool(name="rcp", bufs=4))
        stage_p = ctx.enter_context(tc.tile_pool(name="stage", bufs=3))
        pmm = ctx.enter_context(tc.tile_pool(name="pmm", bufs=pmm_bufs, space="PSUM"))
        patt = ctx.enter_context(tc.tile_pool(name="patt", bufs=patt_bufs, space="PSUM"))
        ppv = ctx.enter_context(tc.tile_pool(name="ppv", bufs=ppv_bufs, space="PSUM"))

        qkw_s = consts.tile([128, 6, 2 * C], f8)
        qkw_r = qkw_d.rearrange("(k p) c -> p k c", p=128)
        # weight chunks for mi0-3 first so the first matmuls can start ASAP,
        # interleaved with the split-fp8 x tile halves
        nc.sync.dma_start(out=qkw_s[:, :, 0:512], in_=qkw_r[:, :, 0:512])
        xt8_pre = xt8_p.tile([128, 2, 6, NP], f8)
        nc.sync.dma_start(out=xt8_pre[:, 0], in_=xT8_d[0][:, 0])
        nc.sync.dma_start(out=xt8_pre[:, 1], in_=xT8_d[0][:, 1])
        nc.sync.dma_start(out=qkw_s[:, :, 512:1024], in_=qkw_r[:, :, 512:1024])
        nc.sync.dma_start(out=qkw_s[:, :, 1024:1536], in_=qkw_r[:, :, 1024:1536])
        xt_pre = xt_p.tile([128, 6, NP], bf16)
        nc.scalar.dma_start(out=xt_pre[:], in_=xT_d[0])
        qb_s = consts.tile([128, 6], f32)
        nc.sync.dma_start(out=qb_s[:], in_=qb_d[:])
        vw_s = consts.tile([128, 6, C], bf16)
        pw_s = consts.tile([128, 6, C], bf16)
        erb0_s = consts.tile([128, H, NP], bf16)
        erb1_s = consts.tile([69, H, NP], bf16)
        pbb_s = consts.tile([128, C], f32)
        ones64 = consts.tile([128, 64], bf16)
        nc.gpsimd.memset(ones64[:], 1.0)

        SB = NB // 2                      # superblocks of 4 batches
        MT7 = [(g, min(128, 2 * NP - g)) for g in range(0, 2 * NP, 128)]

        for sb in range(SB):
            at_s = at_p.tile([128, 6, 2 * NP], bf16)
            for bh in range(2):
                blk = sb * 2 + bh
                b0 = blk * BLK
                off = bh * NP

                if blk == 0:
                    xt_s = xt_pre
                    xt8_s = xt8_pre
                else:
                    xt_s = xt_p.tile([128, 6, NP], bf16)
                    xt8_s = xt8_p.tile([128, 2, 6, NP], f8)
                    nc.sync.dma_start(out=xt8_s[:], in_=xT8_d[blk])
                    nc.sync.dma_start(out=xt_s[:], in_=xT_d[blk])

                # ---- qkT [12 x 128, NP] bf16 via split-fp8 DoubleRow gemm;
                # q/k scaled by QS=64, rescale folded into the exp ----
                qkt_s = qkt_p.tile([128, H, NP], bf16)
                anchors = {}
                for mi in range(12):
                    ps = pmm.tile([128, NP], f32, tag="pmm")
                    for s in range(2):
                        for t in range(3):
                            mm = nc.tensor.matmul(
                                ps[:],
                                lhsT=qkw_s[:, 2 * t:2 * t + 2, mi * 128:(mi + 1) * 128],
                                rhs=xt8_s[:, s, 2 * t:2 * t + 2, :],
                                start=(s == 0 and t == 0), stop=(s == 1 and t == 2),
                                perf_mode=DR,
                            )
                            if blk == 0 and mi in (0, 6) and s == 0 and t == 0:
                                anchors[mi] = mm.ins
                    if mi < 6:
                        nc.vector.tensor_scalar_add(qkt_s[:, mi, :], ps[:], qb_s[:, mi:mi + 1])
                    else:
                        nc.vector.tensor_copy(out=qkt_s[:, mi, :], in_=ps[:])

                if blk == 0:
                    d1 = nc.gpsimd.dma_start(out=vw_s[:], in_=vw_d.rearrange("(k p) c -> p k c", p=128))
                    d2 = nc.gpsimd.dma_start(out=erb0_s[:], in_=erb_d[:, 0:128, :].rearrange("h p n -> p h n"))
                    d3 = nc.gpsimd.dma_start(out=erb1_s[:], in_=erb_d[:, 128:197, :].rearrange("h p n -> p h n"))
                    d4 = nc.gpsimd.dma_start(out=pw_s[:], in_=pw_d.rearrange("(k p) c -> p k c", p=128))
                    d5 = nc.gpsimd.dma_start(out=pbb_s[:], in_=bass.AP(tensor=pb_d.tensor, offset=0,
                                                                       ap=[[0, 128], [1, C]]))
                    for d in (d1, d2, d3):
                        add_dep_helper(d.ins, anchors[0], reason="defer const load past startup")
                    for d in (d4, d5):
                        add_dep_helper(d.ins, anchors[6], reason="defer const load past startup")

                # ---- v natural [NP, 12 heads x 64] bf16 ----
                v_s = v_p.tile([128, BLK, 2, H, 64], bf16)
                for j in range(BLK):
                    for t, (r0, msz) in enumerate(MT):
                        for nt in range(2):
                            ps = pmm.tile([128, 384], f32, tag="pmm")
                            for k in range(6):
                                nc.tensor.matmul(
                                    ps[0:msz, :],
                                    lhsT=xt_s[:, k, j * N + r0: j * N + r0 + msz],
                                    rhs=vw_s[:, k, nt * 384:(nt + 1) * 384],
                                    start=(k == 0), stop=(k == 5),
                                )
                            nc.vector.tensor_copy(
                                out=v_s[0:msz, j, t, nt * 6:(nt + 1) * 6, :],
                                in_=ps[0:msz, :].rearrange("p (h d) -> p h d", h=6),
                            )

                # ---- attention: scores psum holds both batches of the half-block
                # as two CLOSED groups; one exp + one exp(rb)-multiply ----
                for hp in range(6):
                    es = {}
                    for t, (r0, msz) in enumerate(MT):
                        erb_t = erb0_s if t == 0 else erb1_s
                        # Interleave the two heads' K=64 scores matmuls so each
                        # adjacent PE instruction targets a disjoint row group
                        # (0:64 vs 64:128) and the sub-arrays overlap them.
                        ps_a = patt.tile([128, NP], f32, tag="patt")
                        ps_b = patt.tile([128, NP], f32, tag="patt")
                        pss = {0: ps_a, 1: ps_b}
                        for j in range(BLK):
                            for hi in range(2):
                                nc.tensor.matmul(
                                    pss[hi][0:msz, j * N:(j + 1) * N],
                                    lhsT=qkt_s[64 * hi:64 * (hi + 1), 6 + hp,
                                               j * N + r0: j * N + r0 + msz],
                                    rhs=qkt_s[64 * hi:64 * (hi + 1), hp, j * N:(j + 1) * N],
                                    start=True, stop=True, skip_group_check=True,
                                )
                        for hi in range(2):
                            h = 2 * hp + hi
                            e = e_p.tile([128, NP], bf16, tag="e")
                            nc.scalar.activation(out=e[0:msz, :], in_=pss[hi][0:msz, :],
                                                 func=Exp, scale=EXP_SCALE)
                            eng = nc.gpsimd if hi == 0 else nc.vector
                            eng.tensor_mul(e[0:msz, :], e[0:msz, :], erb_t[0:msz, h, :])
                            es[(t, hi)] = e
                    for j in range(BLK):
                        ps_o = ppv.tile([128, 2 * N], f32, tag="ppv")
                        for hi in range(2):
                            h = 2 * hp + hi
                            for t, (r0, msz) in enumerate(MT):
                                nc.tensor.matmul(
                                    ps_o[hi * 64:(hi + 1) * 64, 0:N],
                                    lhsT=v_s[0:msz, j, t, h, :],
                                    rhs=es[(t, hi)][0:msz, j * N:(j + 1) * N],
                                    start=(t == 0), stop=(t == 1),
                                    skip_group_check=True,
                                )
                        for hi in range(2):
                            for t, (r0, msz) in enumerate(MT):
                                nc.tensor.matmul(
                                    ps_o[hi * 64:(hi + 1) * 64, N:2 * N],
                                    lhsT=ones64[0:msz, :],
                                    rhs=es[(t, hi)][0:msz, j * N:(j + 1) * N],
                                    start=(t == 0), stop=(t == 1),
                                    skip_group_check=True,
                                )
                        rcp = rcp_p.tile([128, N], f32, tag="rcp")
                        _act_recip(nc.scalar, rcp[:], ps_o[:, N:2 * N])
                        nc.vector.tensor_mul(
                            at_s[:, hp, off + j * N:off + (j + 1) * N],
                            ps_o[:, 0:N], rcp[:],
                        )

            # ---- projection over the whole superblock (flat rows), + bias ----
            for g0, msz in MT7:
                stage = stage_p.tile([128, C], f32)
                for nt in range(2):
                    ps = pmm.tile([128, 384], f32, tag="pmm")
                    for k in range(6):
                        nc.tensor.matmul(
                            ps[0:msz, :],
                            lhsT=at_s[:, k, g0:g0 + msz],
                            rhs=pw_s[:, k, nt * 384:(nt + 1) * 384],
                            start=(k == 0), stop=(k == 5),
                        )
                    nc.vector.scalar_tensor_tensor(
                        out=stage[0:msz, nt * 384:(nt + 1) * 384],
                        in0=ps[0:msz, :], scalar=1.0,
                        in1=pbb_s[0:msz, nt * 384:(nt + 1) * 384],
                        op0=mybir.AluOpType.mult, op1=mybir.AluOpType.add,
                    )
                nc.sync.dma_start(
                    out=out_d[sb * 2 * NP + g0: sb * 2 * NP + g0 + msz, :],
                    in_=stage[0:msz, :],
                )
    return nc


_NC = None


def _get_nc():
    global _NC
    if _NC is None:
        _NC = build_nc()
    return _NC


_EXEC = None


def _get_exec():
    """Build the sharded PJRT executable once and reuse it across calls
    (run_bass_via_pjrt re-traces jax.jit on every invocation)."""
    global _EXEC
    if _EXEC is not None:
        return _EXEC
    import jax
    import numpy as _np
    from jax.sharding import Mesh, PartitionSpec
    from jax.experimental.shard_map import shard_map
    import concourse.mybir as mybir_
    from concourse import bass2jax

    nc = _get_nc()
    bass2jax.install_neuronx_cc_hook()
    partition_name = nc.partition_id_tensor.name if nc.partition_id_tensor else None
    in_names, out_names, out_avals = [], [], []
    for alloc in nc.m.functions[0].allocations:
        if not isinstance(alloc, mybir_.MemoryLocationSet):
            continue
        name = alloc.memorylocations[0].name
        if alloc.kind == "ExternalInput":
            if name != partition_name:
                in_names.append(name)
        elif alloc.kind == "ExternalOutput":
            out_names.append(name)
            out_avals.append(jax.core.ShapedArray(
                tuple(alloc.tensor_shape), mybir_.dt.np(alloc.dtype)))
    all_names = list(in_names)
    if partition_name is not None:
        all_names = all_names + [partition_name]

    def _body(*args):
        operands = list(args)
        if partition_name is not None:
            operands.append(bass2jax.partition_id_tensor())
        outs = bass2jax._bass_exec_p.bind(
            *operands,
            out_avals=tuple(out_avals),
            in_names=tuple(all_names),
            out_names=tuple(out_names),
            lowering_input_output_aliases=(),
            sim_require_finite=True,
            sim_require_nnan=True,
            nc=nc,
        )
        return tuple(outs)

    devices = jax.devices()[:N_CORES]
    mesh = Mesh(_np.asarray(devices), ("core",))
    # xT is data-parallel (split on axis 0); every other input is replicated,
    # so it uploads once instead of 8x.
    in_specs = tuple(
        PartitionSpec("core") if name in ("xT", "xT8") else PartitionSpec()
        for name in in_names
    )
    out_specs = (PartitionSpec("core"),) * len(out_avals)
    sharded = jax.jit(
        shard_map(_body, mesh=mesh, in_specs=in_specs, out_specs=out_specs,
                  check_rep=False),
        keep_unused=True,
    )
    _EXEC = (sharded, in_names, out_names, out_avals)
    return _EXEC


def _prep_host(x, qkv_w, q_bias, v_bias, rel_pos_table, proj_w, proj_b, rel_index,
               rb_mode="mul_pool"):
    x = np.asarray(x, np.float32)
    qkv_w = np.asarray(qkv_w, np.float32)
    xT32 = np.ascontiguousarray(x.transpose(0, 2, 1))          # [B, C, N]
    # bf16 x packed to the SBUF tile layout: [blk, p, k, j, n] so each
    # 2-batch block is ONE contiguous DMA
    xtp = xT32.reshape(B // 2, 2, 6, 128, N).transpose(0, 3, 2, 1, 4)
    xT = np.ascontiguousarray(xtp.reshape(B // 2, 128, 6, 2 * N)).astype(
        ml_dtypes.bfloat16)
    # split-fp8 x packed likewise with a leading hi/lo dim per partition
    x8h = xT32.astype(ml_dtypes.float8_e4m3)
    x8l = (xT32 - x8h.astype(np.float32)).astype(ml_dtypes.float8_e4m3)
    x8 = np.stack([x8h, x8l], axis=1)                          # [B, 2, C, N]
    x8p = x8.reshape(B // 2, 2, 2, 6, 128, N).transpose(0, 4, 2, 3, 1, 5)
    xT8 = np.ascontiguousarray(x8p.reshape(B // 2, 128, 2, 6, 2 * N))
    qk_wT = np.ascontiguousarray(qkv_w[:2 * C].T) * QS         # [C, 2C]
    qk_wT = qk_wT.astype(ml_dtypes.float8_e4m3)
    qb = (np.asarray(q_bias, np.float32) * QS).reshape(6, 128).T.copy()  # [128, 6]
    v_wT = np.ascontiguousarray(qkv_w[2 * C:].T).astype(ml_dtypes.bfloat16)
    proj_wT = np.ascontiguousarray(np.asarray(proj_w, np.float32).T).astype(ml_dtypes.bfloat16)
    pb_eff = (np.asarray(proj_b, np.float32)
              + np.asarray(proj_w, np.float32) @ np.asarray(v_bias, np.float32))
    rb = np.asarray(rel_pos_table, np.float32)[
        np.asarray(rel_index).reshape(-1)].reshape(N, N, H)    # [n, m, h]
    rbT = np.exp(rb.transpose(2, 1, 0))
    rbT = np.concatenate([rbT] * BLK, axis=2)
    erbT = rbT.astype(ml_dtypes.bfloat16)
    return xT, xT8, qk_wT, qb, v_wT, proj_wT, pb_eff.reshape(1, C), erbT


def kernel(x, qkv_w, q_bias, v_bias, rel_pos_table, proj_w, proj_b, rel_index):
    xT, xT8, qk_wT, qb, v_wT, proj_wT, pb_eff, erbT = _prep_host(
        x, qkv_w, q_bias, v_bias, rel_pos_table, proj_w, proj_b, rel_index,
        rb_mode=RB_MODE)
    per_core = {
        "xT": xT, "xT8": xT8,                       # split on axis 0
        "qkw": qk_wT, "vw": v_wT, "pw": proj_wT,
        "pb": pb_eff, "qb": qb, "erb": erbT,
    }
    try:
        sharded, in_names, out_names, out_avals = _get_exec()
        concat_in = [np.ascontiguousarray(per_core[name]) for name in in_names]
        out_arrs = sharded(*concat_in)
        out = np.asarray(out_arrs[out_names.index("out")]).reshape(B, N, C)
    except Exception:
        # Robust fallback: the stock SPMD runner (slower per call, same NEFF).
        in_maps = []
        for c in range(N_CORES):
            m = {k: v for k, v in per_core.items() if k not in ("xT", "xT8")}
            m["xT"] = np.ascontiguousarray(xT[c * NB:(c + 1) * NB])
            m["xT8"] = np.ascontiguousarray(xT8[c * NB:(c + 1) * NB])
            in_maps.append(m)
        res = run_bass_kernel_spmd(_get_nc(), in_maps, core_ids=list(range(N_CORES)))
        out = np.concatenate(
            [res.results[c]["out"].reshape(BC, N, C) for c in range(N_CORES)], axis=0)
    return out.astype(np.float32)



# revision 33
# speedup vs baseline: 1.1669x; 1.0748x over previous
"""BEiT-style windowed attention (B=128, N=197, C=768, H=12) on 8 TRN2 NeuronCores.

Data-parallel over batch: 16 batches per core, 2-batch half-blocks inside
4-batch superblocks. Host pre-processing casts x and the qkv/v/proj weights to
bf16, folds the attention scale into the q weights/bias, folds v_bias into the
projection bias (softmax rows sum to 1), and pre-gathers exp(rel_pos_bias).

Device pipeline per core, per 2-batch half-block:
  qkT  [1536, 394] = qk_wT.T @ xT      (bf16 matmuls, moving dim 394)
  v    [394, 768]  = xT.T @ v_wT       (bf16) with interleaved ones columns
  S.T  [197, 197]  = kT.T @ qT         (bf16 per head; both batches land in one
                                        [128,394] psum as two closed groups)
  E    = exp(S.T) * exp_rb             (one ACT exp per psum — ACT ops have
                                        ~530ns fixed overhead, so fewer+wider
                                        wins; exp(rb) multiply split DVE/Pool;
                                        no max-subtraction: |scores| < ~3)
  outT [128, 197]  = v.T @ E           (heads of a pair stacked at partitions
                                        0:64/64:128 via output col-groups; the
                                        softmax sums land in cols 197:394 of
                                        the same psum via ones-matmuls)
  attnoutT = outT * recip(colsums)     (one ACT reciprocal + one DVE multiply
                                        per pair — DVE recip is 3.2us/op on HW)
  out  = attnoutT.T @ proj_wT + bias   (bf16, projected once per 4-batch
                                        superblock: 7 M-tiles instead of 8;
                                        bias added via a pre-broadcast tensor)
"""
import sys
sys.path.insert(0, '/opt/trn_rl_repo')

import numpy as np
import ml_dtypes
from contextlib import ExitStack

import concourse.bass as bass
import concourse.tile as tile
from concourse.tile import add_dep_helper
from concourse import mybir
from concourse.bass_utils import run_bass_kernel_spmd
from concourse.vector_clock import ScopedClock, VectorClock

f32 = mybir.dt.float32
f32r = mybir.dt.float32r
bf16 = mybir.dt.bfloat16
f8 = mybir.dt.float8e4
DR = mybir.MatmulPerfMode.DoubleRow

N_CORES = 8
RB_MODE = "ident_pe"
B, N, C, H, HD = 128, 197, 768, 12, 64
BC = B // N_CORES          # batches per core
BLK = 2                    # batches per block
NB = BC // BLK             # blocks per core
NP = BLK * N               # block column width (394)
NPP = 400                  # xt8 tile pitch (DR ldweights needs step%16==0)
SCALE = HD ** -0.5
QS = 64.0                  # fp8 weight pre-scale for the qk gemm
EXP_SCALE = SCALE / (QS * QS)


class TileContextFixed(tile.TileContext):
    """The walrus in this container accepts at most ONE sync wait per
    instruction. Stock Tile attaches several (both on ordinary instructions
    during wait assignment and on the tail drain). Split the extras onto
    same-engine InstNoOps, and emit the tail drain one proc at a time."""

    def _lower_ordered_insts(self, ordered):
        for bb_name, insts in ordered.items():
            i = 0
            while i < len(insts):
                inst = insts[i]
                si = inst.sync_info
                if si is not None and si.on_wait and len(si.on_wait) > 1:
                    waits = list(si.on_wait)
                    inst.sync_info = mybir.SyncInfo(
                        on_wait=[waits[-1]], on_update=list(si.on_update)
                    )
                    nops = [
                        mybir.InstNoOp(
                            name=f"{inst.name}__wsplit{k}",
                            engine=inst.engine,
                            bass_nofuse=True,
                            sync_info=mybir.SyncInfo(on_wait=[w], on_update=[]),
                        )
                        for k, w in enumerate(waits[:-1])
                    ]
                    insts[i:i] = nops
                    i += len(nops)
                i += 1
        return super()._lower_ordered_insts(ordered)

    def _drain_and_barrier(self, tick_clock, wait_clock):
        gc = tick_clock.global_clock
        n = len(gc)
        for i in range(n):
            if gc[i] > 0:
                vc = VectorClock([0] * n)
                vc.require_at_least(i, gc[i])
                d = self.nc.sync.drain()
                wait_clock.add_sem_waits(d.ins, ScopedClock({None: vc}))
        self.nc.all_engine_barrier()
        assert self.sems is not None
        popped = self.nc._tile_sem_poison_stack.pop()
        assert popped is self._sem_poison
        self.nc.clear_and_free_semaphores(list(self.sems.allocated().values()))
        self.nc.all_engine_barrier()


def _act_recip(eng, out, in_):
    imm = lambda v: mybir.ImmediateValue(dtype=f32, value=v)
    return eng.add_instruction(mybir.InstActivation(
        name=eng.bass.get_next_instruction_name(),
        func=mybir.ActivationFunctionType.Reciprocal,
        ins=[eng.lower_ap(in_), imm(0.0), imm(1.0), imm(0.0)],
        outs=[eng.lower_ap(out)],
    ))


def build_nc(rb_mode=RB_MODE, patt_bufs=3, pmm_bufs=3, ppv_bufs=2, e_bufs=10):
    # rb_mode: how exp(S+rb) is formed:
    #   "mul_pool"  E = exp(S) * erb on gpsimd
    #   "mul_dve"   E = exp(S) * erb on DVE
    #   "mul_split" alternate gpsimd/DVE by head parity
    #   "ident_pe"  S += rb via identity matmul on PE, E = exp(S)
    nc = bass.Bass("TRN2", target_bir_lowering=False, debug=False)
    Exp = mybir.ActivationFunctionType.Exp

    xT8_d = nc.dram_tensor("xT8", [NB, 128, 2, 6, NP], f8, kind="ExternalInput").ap()
    qkw_d = nc.dram_tensor("qkw", [C, 2 * C], f8, kind="ExternalInput").ap()
    vw_d = nc.dram_tensor("vw", [2, C, C], f8, kind="ExternalInput").ap()
    pw_d = nc.dram_tensor("pw", [C, C], bf16, kind="ExternalInput").ap()
    pb_d = nc.dram_tensor("pb", [1, C], f32, kind="ExternalInput").ap()
    qb_d = nc.dram_tensor("qb", [128, 6], f32, kind="ExternalInput").ap()

    erb_d = nc.dram_tensor("erb", [H, N, NP], bf16, kind="ExternalInput").ap()
    out_d = nc.dram_tensor("out", [BC * N, C], f32, kind="ExternalOutput").ap()

    MT = ((0, 128), (128, 69))  # (row offset, rows) m-tiles of 197

    with TileContextFixed(nc) as tc, ExitStack() as ctx:
        consts = ctx.enter_context(tc.tile_pool(name="consts", bufs=1))
        xt8_p = ctx.enter_context(tc.tile_pool(name="xt8", bufs=2))
        qkt_p = ctx.enter_context(tc.tile_pool(name="qkt", bufs=3))
        v_p = ctx.enter_context(tc.tile_pool(name="v", bufs=2))
        at_p = ctx.enter_context(tc.tile_pool(name="at", bufs=3))
        e_p = ctx.enter_context(tc.tile_pool(name="e", bufs=e_bufs))
        rcp_p = ctx.enter_context(tc.tile_pool(name="rcp", bufs=4))
        stage_p = ctx.enter_context(tc.tile_pool(name="stage", bufs=3))
        pmm = ctx.enter_context(tc.tile_pool(name="pmm", bufs=pmm_bufs, space="PSUM"))
        patt = ctx.enter_context(tc.tile_pool(name="patt", bufs=patt_bufs, space="PSUM"))
        ppv = ctx.enter_context(tc.tile_pool(name="ppv", bufs=ppv_bufs, space="PSUM"))

        # One serial DMA stream (sync queue), ordered by first consumption:
        # qk weights (chunked by mi group), split-fp8 x for block 0, q bias,
        # split-fp8 v weights, exp(rel-bias), then later blocks / proj consts.
        qkw_s = consts.tile([128, 6, 2 * C], f8)
        qkw_r = qkw_d.rearrange("(k p) c -> p k c", p=128)
        nc.sync.dma_start(out=qkw_s[:, :, 0:512], in_=qkw_r[:, :, 0:512])
        xt8_pre = xt8_p.tile([128, 2, 6, NPP], f8)
        nc.sync.dma_start(out=xt8_pre[:, 0, :, 0:NP], in_=xT8_d[0][:, 0])
        qb_s = consts.tile([128, 6], f32)
        nc.sync.dma_start(out=qb_s[:], in_=qb_d[:])
        nc.sync.dma_start(out=xt8_pre[:, 1, :, 0:NP], in_=xT8_d[0][:, 1])
        nc.sync.dma_start(out=qkw_s[:, :, 512:1024], in_=qkw_r[:, :, 512:1024])
        nc.sync.dma_start(out=qkw_s[:, :, 1024:1536], in_=qkw_r[:, :, 1024:1536])
        vw_s = consts.tile([128, 2, 6, C], f8)
        for s in range(2):
            nc.sync.dma_start(out=vw_s[:, s],
                              in_=vw_d[s].rearrange("(k p) c -> p k c", p=128))
        erb0_s = consts.tile([128, H, NP], bf16)
        erb1_s = consts.tile([69, H, NP], bf16)
        nc.sync.dma_start(out=erb0_s[:], in_=erb_d[:, 0:128, :].rearrange("h p n -> p h n"))
        nc.sync.dma_start(out=erb1_s[:], in_=erb_d[:, 128:197, :].rearrange("h p n -> p h n"))
        pw_s = consts.tile([128, 6, C], bf16)
        pbb_s = consts.tile([128, C], f32)
        ones64 = consts.tile([128, 64], bf16)
        nc.gpsimd.memset(ones64[:], 1.0)

        SB = NB // 2                      # superblocks of 4 batches
        MT7 = [(g, min(128, 2 * NP - g)) for g in range(0, 2 * NP, 128)]

        for sb in range(SB):
            at_s = at_p.tile([128, 6, 2 * NP], bf16)
            for bh in range(2):
                blk = sb * 2 + bh
                b0 = blk * BLK
                off = bh * NP

                if blk == 0:
                    xt8_s = xt8_pre
                else:
                    xt8_s = xt8_p.tile([128, 2, 6, NPP], f8)
                    nc.sync.dma_start(out=xt8_s[:, :, :, 0:NP], in_=xT8_d[blk])
                    if blk == 1:
                        nc.sync.dma_start(
                            out=pw_s[:], in_=pw_d.rearrange("(k p) c -> p k c", p=128))
                        nc.sync.dma_start(
                            out=pbb_s[:], in_=bass.AP(tensor=pb_d.tensor, offset=0,
                                                      ap=[[0, 128], [1, C]]))

                # ---- qkT [12 x 128, NP] bf16 via split-fp8 DoubleRow gemm;
                # q/k scaled by QS=64, rescale folded into the exp ----
                qkt_s = qkt_p.tile([128, H, NP], bf16)
                for mi in range(12):
                    ps = pmm.tile([128, NP], f32, tag="pmm")
                    for s in range(2):
                        for t in range(3):
                            nc.tensor.matmul(
                                ps[:],
                                lhsT=qkw_s[:, 2 * t:2 * t + 2, mi * 128:(mi + 1) * 128],
                                rhs=xt8_s[:, s, 2 * t:2 * t + 2, 0:NP],
                                start=(s == 0 and t == 0), stop=(s == 1 and t == 2),
                                perf_mode=DR,
                            )
                    if mi < 6:
                        nc.vector.tensor_scalar_add(qkt_s[:, mi, :], ps[:], qb_s[:, mi:mi + 1])
                    else:
                        nc.vector.tensor_copy(out=qkt_s[:, mi, :], in_=ps[:])

                # ---- v natural [NP, 12 heads x 64] bf16, split-fp8 gemm
                # (hi*hi + hi*lo + lo*hi; the lo*lo term is negligible) ----
                v_s = v_p.tile([128, BLK, 2, H, 64], bf16)
                for j in range(BLK):
                    for t, (r0, msz) in enumerate(MT):
                        for nt in range(2):
                            ps = pmm.tile([128, 384], f32, tag="pmm")
                            for pi, (sx, sv) in enumerate(((0, 0), (1, 0), (0, 1))):
                                for kt in range(3):
                                    nc.tensor.matmul(
                                        ps[0:msz, :],
                                        lhsT=xt8_s[:, sx, 2 * kt:2 * kt + 2,
                                                   j * N + r0: j * N + r0 + msz],
                                        rhs=vw_s[:, sv, 2 * kt:2 * kt + 2,
                                                 nt * 384:(nt + 1) * 384],
                                        start=(pi == 0 and kt == 0),
                                        stop=(pi == 2 and kt == 2),
                                        perf_mode=DR,
                                    )
                            nc.vector.tensor_copy(
                                out=v_s[0:msz, j, t, nt * 6:(nt + 1) * 6, :],
                                in_=ps[0:msz, :].rearrange("p (h d) -> p h d", h=6),
                            )

                # ---- attention: scores psum holds both batches of the half-block
                # as two CLOSED groups; one exp + one exp(rb)-multiply ----
                for hp in range(6):
                    es = {}
                    for t, (r0, msz) in enumerate(MT):
                        erb_t = erb0_s if t == 0 else erb1_s
                        # Interleave the two heads' K=64 scores matmuls so each
                        # adjacent PE instruction targets a disjoint row group
                        # (0:64 vs 64:128) and the sub-arrays overlap them.
                        ps_a = patt.tile([128, NP], f32, tag="patt")
                        ps_b = patt.tile([128, NP], f32, tag="patt")
                        pss = {0: ps_a, 1: ps_b}
                        for j in range(BLK):
                            for hi in range(2):
                                nc.tensor.matmul(
                                    pss[hi][0:msz, j * N:(j + 1) * N],
                                    lhsT=qkt_s[64 * hi:64 * (hi + 1), 6 + hp,
                                               j * N + r0: j * N + r0 + msz],
                                    rhs=qkt_s[64 * hi:64 * (hi + 1), hp, j * N:(j + 1) * N],
                                    start=True, stop=True, skip_group_check=True,
                                )
                        for hi in range(2):
                            h = 2 * hp + hi
                            e = e_p.tile([128, NP], bf16, tag="e")
                            nc.scalar.activation(out=e[0:msz, :], in_=pss[hi][0:msz, :],
                                                 func=Exp, scale=EXP_SCALE)
                            eng = nc.gpsimd if hi == 0 else nc.vector
                            eng.tensor_mul(e[0:msz, :], e[0:msz, :], erb_t[0:msz, h, :])
                            es[(t, hi)] = e
                    for j in range(BLK):
                        ps_o = ppv.tile([128, 2 * N], f32, tag="ppv")
                        for hi in range(2):
                            h = 2 * hp + hi
                            for t, (r0, msz) in enumerate(MT):
                                nc.tensor.matmul(
                                    ps_o[hi * 64:(hi + 1) * 64, 0:N],
                                    lhsT=v_s[0:msz, j, t, h, :],
                                    rhs=es[(t, hi)][0:msz, j * N:(j + 1) * N],
                                    start=(t == 0), stop=(t == 1),
                                    skip_group_check=True,
                                )
                        for hi in range(2):
                            for t, (r0, msz) in enumerate(MT):
                                nc.tensor.matmul(
                                    ps_o[hi * 64:(hi + 1) * 64, N:2 * N],
                                    lhsT=ones64[0:msz, :],
                                    rhs=es[(t, hi)][0:msz, j * N:(j + 1) * N],
                                    start=(t == 0), stop=(t == 1),
                                    skip_group_check=True,
                                )
                        rcp = rcp_p.tile([128, N], f32, tag="rcp")
                        _act_recip(nc.scalar, rcp[:], ps_o[:, N:2 * N])
                        nc.vector.tensor_mul(
                            at_s[:, hp, off + j * N:off + (j + 1) * N],
                            ps_o[:, 0:N], rcp[:],
                        )

            # ---- projection over the whole superblock (flat rows), + bias ----
            for g0, msz in MT7:
                stage = stage_p.tile([128, C], f32)
                for nt in range(2):
                    ps = pmm.tile([128, 384], f32, tag="pmm")
                    for k in range(6):
                        nc.tensor.matmul(
                            ps[0:msz, :],
                            lhsT=at_s[:, k, g0:g0 + msz],
                            rhs=pw_s[:, k, nt * 384:(nt + 1) * 384],
                            start=(k == 0), stop=(k == 5),
                        )
                    nc.vector.scalar_tensor_tensor(
                        out=stage[0:msz, nt * 384:(nt + 1) * 384],
                        in0=ps[0:msz, :], scalar=1.0 / QS,
                        in1=pbb_s[0:msz, nt * 384:(nt + 1) * 384],
                        op0=mybir.AluOpType.mult, op1=mybir.AluOpType.add,
                    )
                nc.sync.dma_start(
                    out=out_d[sb * 2 * NP + g0: sb * 2 * NP + g0 + msz, :],
                    in_=stage[0:msz, :],
                )
    return nc


_NC = None


def _get_nc():
    global _NC
    if _NC is None:
        _NC = build_nc()
    return _NC


_EXEC = None


def _get_exec():
    """Build the sharded PJRT executable once and reuse it across calls
    (run_bass_via_pjrt re-traces jax.jit on every invocation)."""
    global _EXEC
    if _EXEC is not None:
        return _EXEC
    import jax
    import numpy as _np
    from jax.sharding import Mesh, PartitionSpec
    from jax.experimental.shard_map import shard_map
    import concourse.mybir as mybir_
    from concourse import bass2jax

    nc = _get_nc()
    bass2jax.install_neuronx_cc_hook()
    partition_name = nc.partition_id_tensor.name if nc.partition_id_tensor else None
    in_names, out_names, out_avals = [], [], []
    for alloc in nc.m.functions[0].allocations:
        if not isinstance(alloc, mybir_.MemoryLocationSet):
            continue
        name = alloc.memorylocations[0].name
        if alloc.kind == "ExternalInput":
            if name != partition_name:
                in_names.append(name)
        elif alloc.kind == "ExternalOutput":
            out_names.append(name)
            out_avals.append(jax.core.ShapedArray(
                tuple(alloc.tensor_shape), mybir_.dt.np(alloc.dtype)))
    all_names = list(in_names)
    if partition_name is not None:
        all_names = all_names + [partition_name]

    def _body(*args):
        operands = list(args)
        if partition_name is not None:
            operands.append(bass2jax.partition_id_tensor())
        outs = bass2jax._bass_exec_p.bind(
            *operands,
            out_avals=tuple(out_avals),
            in_names=tuple(all_names),
            out_names=tuple(out_names),
            lowering_input_output_aliases=(),
            sim_require_finite=True,
            sim_require_nnan=True,
            nc=nc,
        )
        return tuple(outs)

    devices = jax.devices()[:N_CORES]
    mesh = Mesh(_np.asarray(devices), ("core",))
    # xT is data-parallel (split on axis 0); every other input is replicated,
    # so it uploads once instead of 8x.
    in_specs = tuple(
        PartitionSpec("core") if name == "xT8" else PartitionSpec()
        for name in in_names
    )
    out_specs = (PartitionSpec("core"),) * len(out_avals)
    sharded = jax.jit(
        shard_map(_body, mesh=mesh, in_specs=in_specs, out_specs=out_specs,
                  check_rep=False),
        keep_unused=True,
    )
    _EXEC = (sharded, in_names, out_names, out_avals)
    return _EXEC


def _prep_host(x, qkv_w, q_bias, v_bias, rel_pos_table, proj_w, proj_b, rel_index,
               rb_mode="mul_pool"):
    x = np.asarray(x, np.float32)
    qkv_w = np.asarray(qkv_w, np.float32)
    xT32 = np.ascontiguousarray(x.transpose(0, 2, 1))          # [B, C, N]
    # split-fp8 x packed to the SBUF tile layout [blk, p, s, k, j, n] so each
    # 2-batch block is ONE contiguous DMA
    x8h = xT32.astype(ml_dtypes.float8_e4m3)
    x8l = (xT32 - x8h.astype(np.float32)).astype(ml_dtypes.float8_e4m3)
    x8 = np.stack([x8h, x8l], axis=1)                          # [B, 2, C, N]
    x8p = x8.reshape(B // 2, 2, 2, 6, 128, N).transpose(0, 4, 2, 3, 1, 5)
    xT8 = np.ascontiguousarray(x8p.reshape(B // 2, 128, 2, 6, 2 * N))
    qk_wT = np.ascontiguousarray(qkv_w[:2 * C].T) * QS         # [C, 2C]
    qk_wT = qk_wT.astype(ml_dtypes.float8_e4m3)
    qb = (np.asarray(q_bias, np.float32) * QS).reshape(6, 128).T.copy()  # [128, 6]
    vw64 = np.ascontiguousarray(qkv_w[2 * C:].T) * QS          # [C, C]
    vwh = vw64.astype(ml_dtypes.float8_e4m3)
    vwl = (vw64 - vwh.astype(np.float32)).astype(ml_dtypes.float8_e4m3)
    v_wT = np.ascontiguousarray(np.stack([vwh, vwl], axis=0))  # [2, C, C]
    proj_wT = np.ascontiguousarray(np.asarray(proj_w, np.float32).T).astype(ml_dtypes.bfloat16)
    pb_eff = (np.asarray(proj_b, np.float32)
              + np.asarray(proj_w, np.float32) @ np.asarray(v_bias, np.float32))
    rb = np.asarray(rel_pos_table, np.float32)[
        np.asarray(rel_index).reshape(-1)].reshape(N, N, H)    # [n, m, h]
    rbT = np.exp(rb.transpose(2, 1, 0))
    rbT = np.concatenate([rbT] * BLK, axis=2)
    erbT = rbT.astype(ml_dtypes.bfloat16)
    return xT8, qk_wT, qb, v_wT, proj_wT, pb_eff.reshape(1, C), erbT


def kernel(x, qkv_w, q_bias, v_bias, rel_pos_table, proj_w, proj_b, rel_index):
    xT8, qk_wT, qb, v_wT, proj_wT, pb_eff, erbT = _prep_host(
        x, qkv_w, q_bias, v_bias, rel_pos_table, proj_w, proj_b, rel_index,
        rb_mode=RB_MODE)
    per_core = {
        "xT8": xT8,                                 # split on axis 0
        "qkw": qk_wT, "vw": v_wT, "pw": proj_wT,
        "pb": pb_eff, "qb": qb, "erb": erbT,
    }
    try:
        sharded, in_names, out_names, out_avals = _get_exec()
        concat_in = [np.ascontiguousarray(per_core[name]) for name in in_names]
        out_arrs = sharded(*concat_in)
        out = np.asarray(out_arrs[out_names.index("out")]).reshape(B, N, C)
    except Exception:
        # Robust fallback: the stock SPMD runner (slower per call, same NEFF).
        in_maps = []
        for c in range(N_CORES):
            m = {k: v for k, v in per_core.items() if k != "xT8"}
            m["xT8"] = np.ascontiguousarray(xT8[c * NB:(c + 1) * NB])
            in_maps.append(m)
        res = run_bass_kernel_spmd(_get_nc(), in_maps, core_ids=list(range(N_CORES)))
        out = np.concatenate(
            [res.results[c]["out"].reshape(BC, N, C) for c in range(N_CORES)], axis=0)
    return out.astype(np.float32)



# revision 36
# speedup vs baseline: 1.2356x; 1.0589x over previous
"""BEiT-style windowed attention (B=128, N=197, C=768, H=12) on 8 TRN2 NeuronCores.

Data-parallel over batch: 16 batches per core, 2-batch half-blocks inside
4-batch superblocks. Host pre-processing casts x and the qkv/v/proj weights to
bf16, folds the attention scale into the q weights/bias, folds v_bias into the
projection bias (softmax rows sum to 1), and pre-gathers exp(rel_pos_bias).

Device pipeline per core, per 2-batch half-block:
  qkT  [1536, 394] = qk_wT.T @ xT      (bf16 matmuls, moving dim 394)
  v    [394, 768]  = xT.T @ v_wT       (bf16) with interleaved ones columns
  S.T  [197, 197]  = kT.T @ qT         (bf16 per head; both batches land in one
                                        [128,394] psum as two closed groups)
  E    = exp(S.T) * exp_rb             (one ACT exp per psum — ACT ops have
                                        ~530ns fixed overhead, so fewer+wider
                                        wins; exp(rb) multiply split DVE/Pool;
                                        no max-subtraction: |scores| < ~3)
  outT [128, 197]  = v.T @ E           (heads of a pair stacked at partitions
                                        0:64/64:128 via output col-groups; the
                                        softmax sums land in cols 197:394 of
                                        the same psum via ones-matmuls)
  attnoutT = outT * recip(colsums)     (one ACT reciprocal + one DVE multiply
                                        per pair — DVE recip is 3.2us/op on HW)
  out  = attnoutT.T @ proj_wT + bias   (bf16, projected once per 4-batch
                                        superblock: 7 M-tiles instead of 8;
                                        bias added via a pre-broadcast tensor)
"""
import sys
sys.path.insert(0, '/opt/trn_rl_repo')

import numpy as np
import ml_dtypes
from contextlib import ExitStack

import concourse.bass as bass
import concourse.tile as tile
from concourse.tile import add_dep_helper
from concourse import mybir
from concourse.bass_utils import run_bass_kernel_spmd
from concourse.vector_clock import ScopedClock, VectorClock

f32 = mybir.dt.float32
f32r = mybir.dt.float32r
bf16 = mybir.dt.bfloat16
f8 = mybir.dt.float8e4
DR = mybir.MatmulPerfMode.DoubleRow

N_CORES = 8
RB_MODE = "ident_pe"
B, N, C, H, HD = 128, 197, 768, 12, 64
BC = B // N_CORES          # batches per core
BLK = 2                    # batches per block
NB = BC // BLK             # blocks per core
NP = BLK * N               # block column width (394)
NPP = 400                  # xt8 tile pitch (DR ldweights needs step%16==0)
SCALE = HD ** -0.5
QS = 64.0                  # fp8 weight pre-scale for the qk gemm
EXP_SCALE = SCALE / (QS * QS)


class TileContextFixed(tile.TileContext):
    """The walrus in this container accepts at most ONE sync wait per
    instruction. Stock Tile attaches several (both on ordinary instructions
    during wait assignment and on the tail drain). Split the extras onto
    same-engine InstNoOps, and emit the tail drain one proc at a time."""

    def _lower_ordered_insts(self, ordered):
        for bb_name, insts in ordered.items():
            i = 0
            while i < len(insts):
                inst = insts[i]
                si = inst.sync_info
                if si is not None and si.on_wait and len(si.on_wait) > 1:
                    waits = list(si.on_wait)
                    inst.sync_info = mybir.SyncInfo(
                        on_wait=[waits[-1]], on_update=list(si.on_update)
                    )
                    nops = [
                        mybir.InstNoOp(
                            name=f"{inst.name}__wsplit{k}",
                            engine=inst.engine,
                            bass_nofuse=True,
                            sync_info=mybir.SyncInfo(on_wait=[w], on_update=[]),
                        )
                        for k, w in enumerate(waits[:-1])
                    ]
                    insts[i:i] = nops
                    i += len(nops)
                i += 1
        return super()._lower_ordered_insts(ordered)

    def _drain_and_barrier(self, tick_clock, wait_clock):
        gc = tick_clock.global_clock
        n = len(gc)
        for i in range(n):
            if gc[i] > 0:
                vc = VectorClock([0] * n)
                vc.require_at_least(i, gc[i])
                d = self.nc.sync.drain()
                wait_clock.add_sem_waits(d.ins, ScopedClock({None: vc}))
        self.nc.all_engine_barrier()
        assert self.sems is not None
        popped = self.nc._tile_sem_poison_stack.pop()
        assert popped is self._sem_poison
        self.nc.clear_and_free_semaphores(list(self.sems.allocated().values()))
        self.nc.all_engine_barrier()


def _act_recip(eng, out, in_):
    imm = lambda v: mybir.ImmediateValue(dtype=f32, value=v)
    return eng.add_instruction(mybir.InstActivation(
        name=eng.bass.get_next_instruction_name(),
        func=mybir.ActivationFunctionType.Reciprocal,
        ins=[eng.lower_ap(in_), imm(0.0), imm(1.0), imm(0.0)],
        outs=[eng.lower_ap(out)],
    ))


def build_nc(rb_mode=RB_MODE, patt_bufs=3, pmm_bufs=3, ppv_bufs=2, e_bufs=10):
    # rb_mode: how exp(S+rb) is formed:
    #   "mul_pool"  E = exp(S) * erb on gpsimd
    #   "mul_dve"   E = exp(S) * erb on DVE
    #   "mul_split" alternate gpsimd/DVE by head parity
    #   "ident_pe"  S += rb via identity matmul on PE, E = exp(S)
    nc = bass.Bass("TRN2", target_bir_lowering=False, debug=False)
    Exp = mybir.ActivationFunctionType.Exp

    xT8_d = nc.dram_tensor("xT8", [NB, 128, 2, 6, NP], f8, kind="ExternalInput").ap()
    qkw_d = nc.dram_tensor("qkw", [C, 2 * C], f8, kind="ExternalInput").ap()
    vw_d = nc.dram_tensor("vw", [2, C, C], f8, kind="ExternalInput").ap()
    pw_d = nc.dram_tensor("pw", [C, C], bf16, kind="ExternalInput").ap()
    pb_d = nc.dram_tensor("pb", [1, C], f32, kind="ExternalInput").ap()
    qb_d = nc.dram_tensor("qb", [128, 6], f32, kind="ExternalInput").ap()

    erb_d = nc.dram_tensor("erb", [H, N, NP], bf16, kind="ExternalInput").ap()
    out_d = nc.dram_tensor("out", [BC * N, C], f32, kind="ExternalOutput").ap()

    MT = ((0, 128), (128, 69))  # (row offset, rows) m-tiles of 197

    with TileContextFixed(nc) as tc, ExitStack() as ctx:
        consts = ctx.enter_context(tc.tile_pool(name="consts", bufs=1))
        xt8_p = ctx.enter_context(tc.tile_pool(name="xt8", bufs=2))
        qkt_p = ctx.enter_context(tc.tile_pool(name="qkt", bufs=3))
        v_p = ctx.enter_context(tc.tile_pool(name="v", bufs=2))
        at_p = ctx.enter_context(tc.tile_pool(name="at", bufs=3))
        e_p = ctx.enter_context(tc.tile_pool(name="e", bufs=e_bufs))
        rcp_p = ctx.enter_context(tc.tile_pool(name="rcp", bufs=4))
        stage_p = ctx.enter_context(tc.tile_pool(name="stage", bufs=3))
        pmm = ctx.enter_context(tc.tile_pool(name="pmm", bufs=pmm_bufs, space="PSUM"))
        patt = ctx.enter_context(tc.tile_pool(name="patt", bufs=patt_bufs, space="PSUM"))
        ppv = ctx.enter_context(tc.tile_pool(name="ppv", bufs=ppv_bufs, space="PSUM"))

        # One serial DMA stream (sync queue), ordered by first consumption:
        # qk weights (chunked by mi group), split-fp8 x for block 0, q bias,
        # split-fp8 v weights, exp(rel-bias), then later blocks / proj consts.
        qkw_s = consts.tile([128, 6, 2 * C], f8)
        qkw_r = qkw_d.rearrange("(k p) c -> p k c", p=128)
        nc.sync.dma_start(out=qkw_s[:, :, 0:512], in_=qkw_r[:, :, 0:512])
        xt8_pre = xt8_p.tile([128, 2, 6, NPP], f8)
        nc.sync.dma_start(out=xt8_pre[:, 0, :, 0:NP], in_=xT8_d[0][:, 0])
        qb_s = consts.tile([128, 6], f32)
        nc.sync.dma_start(out=qb_s[:], in_=qb_d[:])
        nc.sync.dma_start(out=xt8_pre[:, 1, :, 0:NP], in_=xT8_d[0][:, 1])
        nc.sync.dma_start(out=qkw_s[:, :, 512:1024], in_=qkw_r[:, :, 512:1024])
        nc.sync.dma_start(out=qkw_s[:, :, 1024:1536], in_=qkw_r[:, :, 1024:1536])
        vw_s = consts.tile([128, 2, 6, C], f8)
        for s in range(2):
            nc.sync.dma_start(out=vw_s[:, s],
                              in_=vw_d[s].rearrange("(k p) c -> p k c", p=128))
        erb0_s = consts.tile([128, H, NP], bf16)
        erb1_s = consts.tile([69, H, NP], bf16)
        nc.sync.dma_start(out=erb0_s[:], in_=erb_d[:, 0:128, :].rearrange("h p n -> p h n"))
        nc.sync.dma_start(out=erb1_s[:], in_=erb_d[:, 128:197, :].rearrange("h p n -> p h n"))
        pw_s = consts.tile([128, 6, C], bf16)
        pbb_s = consts.tile([128, C], f32)
        ones64 = consts.tile([128, 64], bf16)
        nc.gpsimd.memset(ones64[:], 1.0)

        SB = NB // 2                      # superblocks of 4 batches
        MT7 = [(g, min(128, 2 * NP - g)) for g in range(0, 2 * NP, 128)]

        # ---------- emission helpers (software pipelining) ----------
        # PE executes its instruction stream in order, so filler work
        # (next block's qk/v gemms, ready proj m-tiles) is interleaved into
        # the attention emission to keep PE busy while ACT produces E.

        def emit_dma(blk):
            xt8_s = xt8_p.tile([128, 2, 6, NPP], f8)
            nc.sync.dma_start(out=xt8_s[:, :, :, 0:NP], in_=xT8_d[blk])
            if blk == 1:
                nc.sync.dma_start(
                    out=pw_s[:], in_=pw_d.rearrange("(k p) c -> p k c", p=128))
                nc.sync.dma_start(
                    out=pbb_s[:], in_=bass.AP(tensor=pb_d.tensor, offset=0,
                                              ap=[[0, 128], [1, C]]))
            return xt8_s

        def emit_qk_mi(xt8_s, qkt_s, mi):
            # qkT [128, NP] for one mi-tile via split-fp8 DoubleRow gemm;
            # q/k scaled by QS=64, rescale folded into the exp
            ps = pmm.tile([128, NP], f32, tag="pmm")
            for s in range(2):
                for t in range(3):
                    nc.tensor.matmul(
                        ps[:],
                        lhsT=qkw_s[:, 2 * t:2 * t + 2, mi * 128:(mi + 1) * 128],
                        rhs=xt8_s[:, s, 2 * t:2 * t + 2, 0:NP],
                        start=(s == 0 and t == 0), stop=(s == 1 and t == 2),
                        perf_mode=DR,
                    )
            if mi < 6:
                nc.vector.tensor_scalar_add(qkt_s[:, mi, :], ps[:], qb_s[:, mi:mi + 1])
            else:
                nc.vector.tensor_copy(out=qkt_s[:, mi, :], in_=ps[:])

        def emit_v_tile(xt8_s, v_s, j, t, nt):
            # v natural [msz, 6 heads x 64] via split-fp8 gemm
            # (hi*hi + hi*lo + lo*hi; the lo*lo term is negligible)
            r0, msz = MT[t]
            ps = pmm.tile([128, 384], f32, tag="pmm")
            for pi, (sx, sv) in enumerate(((0, 0), (1, 0), (0, 1))):
                for kt in range(3):
                    nc.tensor.matmul(
                        ps[0:msz, :],
                        lhsT=xt8_s[:, sx, 2 * kt:2 * kt + 2,
                                   j * N + r0: j * N + r0 + msz],
                        rhs=vw_s[:, sv, 2 * kt:2 * kt + 2, nt * 384:(nt + 1) * 384],
                        start=(pi == 0 and kt == 0), stop=(pi == 2 and kt == 2),
                        perf_mode=DR,
                    )
            nc.vector.tensor_copy(
                out=v_s[0:msz, j, t, nt * 6:(nt + 1) * 6, :],
                in_=ps[0:msz, :].rearrange("p (h d) -> p h d", h=6),
            )

        def emit_proj_tile(at_s, sb, g0, msz):
            # one m-tile of the superblock projection, + bias, + 1/QS rescale
            stage = stage_p.tile([128, C], f32)
            for nt in range(2):
                ps = pmm.tile([128, 384], f32, tag="pmm")
                for k in range(6):
                    nc.tensor.matmul(
                        ps[0:msz, :],
                        lhsT=at_s[:, k, g0:g0 + msz],
                        rhs=pw_s[:, k, nt * 384:(nt + 1) * 384],
                        start=(k == 0), stop=(k == 5),
                    )
                nc.vector.scalar_tensor_tensor(
                    out=stage[0:msz, nt * 384:(nt + 1) * 384],
                    in0=ps[0:msz, :], scalar=1.0 / QS,
                    in1=pbb_s[0:msz, nt * 384:(nt + 1) * 384],
                    op0=mybir.AluOpType.mult, op1=mybir.AluOpType.add,
                )
            nc.sync.dma_start(
                out=out_d[sb * 2 * NP + g0: sb * 2 * NP + g0 + msz, :],
                in_=stage[0:msz, :],
            )

        def emit_attn(qkt_s, v_s, at_s, off, filler):
            # scores psum holds both batches of the half-block as two CLOSED
            # groups; one exp + one exp(rb)-multiply per psum. Filler closures
            # are drained between the scores and PV groups of each head-pair.
            nfill = max(1, (len(filler) + 5) // 6) if filler else 0
            for hp in range(6):
                es = {}
                for t, (r0, msz) in enumerate(MT):
                    erb_t = erb0_s if t == 0 else erb1_s
                    ps_a = patt.tile([128, NP], f32, tag="patt")
                    ps_b = patt.tile([128, NP], f32, tag="patt")
                    pss = {0: ps_a, 1: ps_b}
                    for j in range(BLK):
                        for hi in range(2):
                            nc.tensor.matmul(
                                pss[hi][0:msz, j * N:(j + 1) * N],
                                lhsT=qkt_s[64 * hi:64 * (hi + 1), 6 + hp,
                                           j * N + r0: j * N + r0 + msz],
                                rhs=qkt_s[64 * hi:64 * (hi + 1), hp, j * N:(j + 1) * N],
                                start=True, stop=True, skip_group_check=True,
                            )
                    for hi in range(2):
                        h = 2 * hp + hi
                        e = e_p.tile([128, NP], bf16, tag="e")
                        nc.scalar.activation(out=e[0:msz, :], in_=pss[hi][0:msz, :],
                                             func=Exp, scale=EXP_SCALE)
                        eng = nc.gpsimd if hi == 0 else nc.vector
                        eng.tensor_mul(e[0:msz, :], e[0:msz, :], erb_t[0:msz, h, :])
                        es[(t, hi)] = e
                for _ in range(nfill):
                    if filler:
                        filler.pop(0)()
                for j in range(BLK):
                    ps_o = ppv.tile([128, 2 * N], f32, tag="ppv")
                    for hi in range(2):
                        h = 2 * hp + hi
                        for t, (r0, msz) in enumerate(MT):
                            nc.tensor.matmul(
                                ps_o[hi * 64:(hi + 1) * 64, 0:N],
                                lhsT=v_s[0:msz, j, t, h, :],
                                rhs=es[(t, hi)][0:msz, j * N:(j + 1) * N],
                                start=(t == 0), stop=(t == 1),
                                skip_group_check=True,
                            )
                    for hi in range(2):
                        for t, (r0, msz) in enumerate(MT):
                            nc.tensor.matmul(
                                ps_o[hi * 64:(hi + 1) * 64, N:2 * N],
                                lhsT=ones64[0:msz, :],
                                rhs=es[(t, hi)][0:msz, j * N:(j + 1) * N],
                                start=(t == 0), stop=(t == 1),
                                skip_group_check=True,
                            )
                    rcp = rcp_p.tile([128, N], f32, tag="rcp")
                    _act_recip(nc.scalar, rcp[:], ps_o[:, N:2 * N])
                    nc.vector.tensor_mul(
                        at_s[:, hp, off + j * N:off + (j + 1) * N],
                        ps_o[:, 0:N], rcp[:],
                    )
            while filler:
                filler.pop(0)()

        # ---------- pipelined emission ----------
        blk_tiles = {}

        def make_blk_items(blk, xt8_s):
            qkt_s = qkt_p.tile([128, H, NP], bf16)
            v_s = v_p.tile([128, BLK, 2, H, 64], bf16)
            blk_tiles[blk] = (qkt_s, v_s)
            items = [
                (lambda mi=mi: emit_qk_mi(xt8_s, qkt_s, mi)) for mi in range(12)
            ] + [
                (lambda j=j, t=t, nt=nt: emit_v_tile(xt8_s, v_s, j, t, nt))
                for j in range(BLK) for t in range(2) for nt in range(2)
            ]
            return items

        # block 0 is emitted straight (nothing to interleave into)
        for it in make_blk_items(0, xt8_pre):
            it()

        pending = []
        at_tiles = {}
        for hb in range(NB):
            sb, bh = divmod(hb, 2)
            if bh == 0:
                at_tiles[sb] = at_p.tile([128, 6, 2 * NP], bf16, name="at_s", tag="at_s")
            filler = list(pending)
            pending = []
            if hb + 1 < NB:
                xt8_n = emit_dma(hb + 1)
                filler += make_blk_items(hb + 1, xt8_n)
            qkt_s, v_s = blk_tiles[hb]
            emit_attn(qkt_s, v_s, at_tiles[sb], bh * NP, filler)
            if bh == 0:
                # proj m-tiles fully inside this half-block's columns
                pending += [
                    (lambda sb=sb, g0=g0, msz=msz:
                     emit_proj_tile(at_tiles[sb], sb, g0, msz))
                    for g0, msz in MT7 if g0 + msz <= NP
                ]
            else:
                pending += [
                    (lambda sb=sb, g0=g0, msz=msz:
                     emit_proj_tile(at_tiles[sb], sb, g0, msz))
                    for g0, msz in MT7 if g0 + msz > NP
                ]
        for it in pending:
            it()
    return nc


_NC = None


def _get_nc():
    global _NC
    if _NC is None:
        _NC = build_nc()
    return _NC


_EXEC = None


def _get_exec():
    """Build the sharded PJRT executable once and reuse it across calls
    (run_bass_via_pjrt re-traces jax.jit on every invocation)."""
    global _EXEC
    if _EXEC is not None:
        return _EXEC
    import jax
    import numpy as _np
    from jax.sharding import Mesh, PartitionSpec
    from jax.experimental.shard_map import shard_map
    import concourse.mybir as mybir_
    from concourse import bass2jax

    nc = _get_nc()
    bass2jax.install_neuronx_cc_hook()
    partition_name = nc.partition_id_tensor.name if nc.partition_id_tensor else None
    in_names, out_names, out_avals = [], [], []
    for alloc in nc.m.functions[0].allocations:
        if not isinstance(alloc, mybir_.MemoryLocationSet):
            continue
        name = alloc.memorylocations[0].name
        if alloc.kind == "ExternalInput":
            if name != partition_name:
                in_names.append(name)
        elif alloc.kind == "ExternalOutput":
            out_names.append(name)
            out_avals.append(jax.core.ShapedArray(
                tuple(alloc.tensor_shape), mybir_.dt.np(alloc.dtype)))
    all_names = list(in_names)
    if partition_name is not None:
        all_names = all_names + [partition_name]

    def _body(*args):
        operands = list(args)
        if partition_name is not None:
            operands.append(bass2jax.partition_id_tensor())
        outs = bass2jax._bass_exec_p.bind(
            *operands,
            out_avals=tuple(out_avals),
            in_names=tuple(all_names),
            out_names=tuple(out_names),
            lowering_input_output_aliases=(),
            sim_require_finite=True,
            sim_require_nnan=True,
            nc=nc,
        )
        return tuple(outs)

    devices = jax.devices()[:N_CORES]
    mesh = Mesh(_np.asarray(devices), ("core",))
    # xT is data-parallel (split on axis 0); every other input is replicated,
    # so it uploads once instead of 8x.
    in_specs = tuple(
        PartitionSpec("core") if name == "xT8" else PartitionSpec()
        for name in in_names
    )
    out_specs = (PartitionSpec("core"),) * len(out_avals)
    sharded = jax.jit(
        shard_map(_body, mesh=mesh, in_specs=in_specs, out_specs=out_specs,
                  check_rep=False),
        keep_unused=True,
    )
    _EXEC = (sharded, in_names, out_names, out_avals)
    return _EXEC


def _prep_host(x, qkv_w, q_bias, v_bias, rel_pos_table, proj_w, proj_b, rel_index,
               rb_mode="mul_pool"):
    x = np.asarray(x, np.float32)
    qkv_w = np.asarray(qkv_w, np.float32)
    xT32 = np.ascontiguousarray(x.transpose(0, 2, 1))          # [B, C, N]
    # split-fp8 x packed to the SBUF tile layout [blk, p, s, k, j, n] so each
    # 2-batch block is ONE contiguous DMA
    x8h = xT32.astype(ml_dtypes.float8_e4m3)
    x8l = (xT32 - x8h.astype(np.float32)).astype(ml_dtypes.float8_e4m3)
    x8 = np.stack([x8h, x8l], axis=1)                          # [B, 2, C, N]
    x8p = x8.reshape(B // 2, 2, 2, 6, 128, N).transpose(0, 4, 2, 3, 1, 5)
    xT8 = np.ascontiguousarray(x8p.reshape(B // 2, 128, 2, 6, 2 * N))
    qk_wT = np.ascontiguousarray(qkv_w[:2 * C].T) * QS         # [C, 2C]
    qk_wT = qk_wT.astype(ml_dtypes.float8_e4m3)
    qb = (np.asarray(q_bias, np.float32) * QS).reshape(6, 128).T.copy()  # [128, 6]
    vw64 = np.ascontiguousarray(qkv_w[2 * C:].T) * QS          # [C, C]
    vwh = vw64.astype(ml_dtypes.float8_e4m3)
    vwl = (vw64 - vwh.astype(np.float32)).astype(ml_dtypes.float8_e4m3)
    v_wT = np.ascontiguousarray(np.stack([vwh, vwl], axis=0))  # [2, C, C]
    proj_wT = np.ascontiguousarray(np.asarray(proj_w, np.float32).T).astype(ml_dtypes.bfloat16)
    pb_eff = (np.asarray(proj_b, np.float32)
              + np.asarray(proj_w, np.float32) @ np.asarray(v_bias, np.float32))
    rb = np.asarray(rel_pos_table, np.float32)[
        np.asarray(rel_index).reshape(-1)].reshape(N, N, H)    # [n, m, h]
    rbT = np.exp(rb.transpose(2, 1, 0))
    rbT = np.concatenate([rbT] * BLK, axis=2)
    erbT = rbT.astype(ml_dtypes.bfloat16)
    return xT8, qk_wT, qb, v_wT, proj_wT, pb_eff.reshape(1, C), erbT


def kernel(x, qkv_w, q_bias, v_bias, rel_pos_table, proj_w, proj_b, rel_index):
    xT8, qk_wT, qb, v_wT, proj_wT, pb_eff, erbT = _prep_host(
        x, qkv_w, q_bias, v_bias, rel_pos_table, proj_w, proj_b, rel_index,
        rb_mode=RB_MODE)
    per_core = {
        "xT8": xT8,                                 # split on axis 0
        "qkw": qk_wT, "vw": v_wT, "pw": proj_wT,
        "pb": pb_eff, "qb": qb, "erb": erbT,
    }
    try:
        sharded, in_names, out_names, out_avals = _get_exec()
        concat_in = [np.ascontiguousarray(per_core[name]) for name in in_names]
        out_arrs = sharded(*concat_in)
        out = np.asarray(out_arrs[out_names.index("out")]).reshape(B, N, C)
    except Exception:
        # Robust fallback: the stock SPMD runner (slower per call, same NEFF).
        in_maps = []
        for c in range(N_CORES):
            m = {k: v for k, v in per_core.items() if k != "xT8"}
            m["xT8"] = np.ascontiguousarray(xT8[c * NB:(c + 1) * NB])
            in_maps.append(m)
        res = run_bass_kernel_spmd(_get_nc(), in_maps, core_ids=list(range(N_CORES)))
        out = np.concatenate(
            [res.results[c]["out"].reshape(BC, N, C) for c in range(N_CORES)], axis=0)
    return out.astype(np.float32)



# revision 37
# speedup vs baseline: 1.2723x; 1.0297x over previous
"""BEiT-style windowed attention (B=128, N=197, C=768, H=12) on 8 TRN2 NeuronCores.

Data-parallel over batch: 16 batches per core, 2-batch half-blocks inside
4-batch superblocks. Host pre-processing casts x and the qkv/v/proj weights to
bf16, folds the attention scale into the q weights/bias, folds v_bias into the
projection bias (softmax rows sum to 1), and pre-gathers exp(rel_pos_bias).

Device pipeline per core, per 2-batch half-block:
  qkT  [1536, 394] = qk_wT.T @ xT      (bf16 matmuls, moving dim 394)
  v    [394, 768]  = xT.T @ v_wT       (bf16) with interleaved ones columns
  S.T  [197, 197]  = kT.T @ qT         (bf16 per head; both batches land in one
                                        [128,394] psum as two closed groups)
  E    = exp(S.T) * exp_rb             (one ACT exp per psum — ACT ops have
                                        ~530ns fixed overhead, so fewer+wider
                                        wins; exp(rb) multiply split DVE/Pool;
                                        no max-subtraction: |scores| < ~3)
  outT [128, 197]  = v.T @ E           (heads of a pair stacked at partitions
                                        0:64/64:128 via output col-groups; the
                                        softmax sums land in cols 197:394 of
                                        the same psum via ones-matmuls)
  attnoutT = outT * recip(colsums)     (one ACT reciprocal + one DVE multiply
                                        per pair — DVE recip is 3.2us/op on HW)
  out  = attnoutT.T @ proj_wT + bias   (bf16, projected once per 4-batch
                                        superblock: 7 M-tiles instead of 8;
                                        bias added via a pre-broadcast tensor)
"""
import sys
sys.path.insert(0, '/opt/trn_rl_repo')

import numpy as np
import ml_dtypes
from contextlib import ExitStack

import concourse.bass as bass
import concourse.tile as tile
from concourse.tile import add_dep_helper
from concourse import mybir
from concourse.bass_utils import run_bass_kernel_spmd
from concourse.vector_clock import ScopedClock, VectorClock

f32 = mybir.dt.float32
f32r = mybir.dt.float32r
bf16 = mybir.dt.bfloat16
f8 = mybir.dt.float8e4
DR = mybir.MatmulPerfMode.DoubleRow

N_CORES = 8
RB_MODE = "ident_pe"
B, N, C, H, HD = 128, 197, 768, 12, 64
BC = B // N_CORES          # batches per core
BLK = 2                    # batches per block
NB = BC // BLK             # blocks per core
NP = BLK * N               # block column width (394)
NPP = 400                  # xt8 tile pitch (DR ldweights needs step%16==0)
SCALE = HD ** -0.5
QS = 64.0                  # fp8 weight pre-scale for the qk gemm
EXP_SCALE = SCALE / (QS * QS)


class TileContextFixed(tile.TileContext):
    """The walrus in this container accepts at most ONE sync wait per
    instruction. Stock Tile attaches several (both on ordinary instructions
    during wait assignment and on the tail drain). Split the extras onto
    same-engine InstNoOps, and emit the tail drain one proc at a time."""

    def _lower_ordered_insts(self, ordered):
        for bb_name, insts in ordered.items():
            i = 0
            while i < len(insts):
                inst = insts[i]
                si = inst.sync_info
                if si is not None and si.on_wait and len(si.on_wait) > 1:
                    waits = list(si.on_wait)
                    inst.sync_info = mybir.SyncInfo(
                        on_wait=[waits[-1]], on_update=list(si.on_update)
                    )
                    nops = [
                        mybir.InstNoOp(
                            name=f"{inst.name}__wsplit{k}",
                            engine=inst.engine,
                            bass_nofuse=True,
                            sync_info=mybir.SyncInfo(on_wait=[w], on_update=[]),
                        )
                        for k, w in enumerate(waits[:-1])
                    ]
                    insts[i:i] = nops
                    i += len(nops)
                i += 1
        return super()._lower_ordered_insts(ordered)

    def _drain_and_barrier(self, tick_clock, wait_clock):
        gc = tick_clock.global_clock
        n = len(gc)
        for i in range(n):
            if gc[i] > 0:
                vc = VectorClock([0] * n)
                vc.require_at_least(i, gc[i])
                d = self.nc.sync.drain()
                wait_clock.add_sem_waits(d.ins, ScopedClock({None: vc}))
        self.nc.all_engine_barrier()
        assert self.sems is not None
        popped = self.nc._tile_sem_poison_stack.pop()
        assert popped is self._sem_poison
        self.nc.clear_and_free_semaphores(list(self.sems.allocated().values()))
        self.nc.all_engine_barrier()


def _act_recip(eng, out, in_):
    imm = lambda v: mybir.ImmediateValue(dtype=f32, value=v)
    return eng.add_instruction(mybir.InstActivation(
        name=eng.bass.get_next_instruction_name(),
        func=mybir.ActivationFunctionType.Reciprocal,
        ins=[eng.lower_ap(in_), imm(0.0), imm(1.0), imm(0.0)],
        outs=[eng.lower_ap(out)],
    ))


def build_nc(rb_mode=RB_MODE, patt_bufs=3, pmm_bufs=3, ppv_bufs=2, e_bufs=10):
    # rb_mode: how exp(S+rb) is formed:
    #   "mul_pool"  E = exp(S) * erb on gpsimd
    #   "mul_dve"   E = exp(S) * erb on DVE
    #   "mul_split" alternate gpsimd/DVE by head parity
    #   "ident_pe"  S += rb via identity matmul on PE, E = exp(S)
    nc = bass.Bass("TRN2", target_bir_lowering=False, debug=False)
    Exp = mybir.ActivationFunctionType.Exp

    xT8_d = nc.dram_tensor("xT8", [NB, 128, 2, 6, NP], f8, kind="ExternalInput").ap()
    qkw_d = nc.dram_tensor("qkw", [C, 2 * C], f8, kind="ExternalInput").ap()
    vw_d = nc.dram_tensor("vw", [2, C, C], f8, kind="ExternalInput").ap()
    pw_d = nc.dram_tensor("pw", [C, C], bf16, kind="ExternalInput").ap()
    pb_d = nc.dram_tensor("pb", [1, C], f32, kind="ExternalInput").ap()
    qb_d = nc.dram_tensor("qb", [128, 6], f32, kind="ExternalInput").ap()

    erb_d = nc.dram_tensor("erb", [H, N, NP], bf16, kind="ExternalInput").ap()
    out_d = nc.dram_tensor("out", [BC * N, C], f32, kind="ExternalOutput").ap()

    MT = ((0, 128), (128, 69))  # (row offset, rows) m-tiles of 197

    with TileContextFixed(nc) as tc, ExitStack() as ctx:
        consts = ctx.enter_context(tc.tile_pool(name="consts", bufs=1))
        xt8_p = ctx.enter_context(tc.tile_pool(name="xt8", bufs=2))
        qkt_p = ctx.enter_context(tc.tile_pool(name="qkt", bufs=3))
        v_p = ctx.enter_context(tc.tile_pool(name="v", bufs=2))
        at_p = ctx.enter_context(tc.tile_pool(name="at", bufs=3))
        e_p = ctx.enter_context(tc.tile_pool(name="e", bufs=e_bufs))
        rcp_p = ctx.enter_context(tc.tile_pool(name="rcp", bufs=4))
        stage_p = ctx.enter_context(tc.tile_pool(name="stage", bufs=3))
        pmm = ctx.enter_context(tc.tile_pool(name="pmm", bufs=pmm_bufs, space="PSUM"))
        patt = ctx.enter_context(tc.tile_pool(name="patt", bufs=patt_bufs, space="PSUM"))
        ppv = ctx.enter_context(tc.tile_pool(name="ppv", bufs=ppv_bufs, space="PSUM"))

        # One serial DMA stream (sync queue), ordered by first consumption:
        # qk weights (chunked by mi group), split-fp8 x for block 0, q bias,
        # split-fp8 v weights, exp(rel-bias), then later blocks / proj consts.
        qkw_s = consts.tile([128, 6, 2 * C], f8)
        qkw_r = qkw_d.rearrange("(k p) c -> p k c", p=128)
        nc.sync.dma_start(out=qkw_s[:, :, 0:512], in_=qkw_r[:, :, 0:512])
        xt8_pre = xt8_p.tile([128, 2, 6, NPP], f8)
        nc.sync.dma_start(out=xt8_pre[:, 0, :, 0:NP], in_=xT8_d[0][:, 0])
        qb_s = consts.tile([128, 6], f32)
        nc.sync.dma_start(out=qb_s[:], in_=qb_d[:])
        nc.sync.dma_start(out=xt8_pre[:, 1, :, 0:NP], in_=xT8_d[0][:, 1])
        nc.sync.dma_start(out=qkw_s[:, :, 512:1024], in_=qkw_r[:, :, 512:1024])
        nc.sync.dma_start(out=qkw_s[:, :, 1024:1536], in_=qkw_r[:, :, 1024:1536])
        vw_s = consts.tile([128, 2, 6, C], f8)
        for s in range(2):
            nc.sync.dma_start(out=vw_s[:, s],
                              in_=vw_d[s].rearrange("(k p) c -> p k c", p=128))
        xt8_b1 = xt8_p.tile([128, 2, 6, NPP], f8)
        nc.sync.dma_start(out=xt8_b1[:, :, :, 0:NP], in_=xT8_d[1])
        erb0_s = consts.tile([128, H, NP], bf16)
        erb1_s = consts.tile([69, H, NP], bf16)
        nc.sync.dma_start(out=erb0_s[:], in_=erb_d[:, 0:128, :].rearrange("h p n -> p h n"))
        nc.sync.dma_start(out=erb1_s[:], in_=erb_d[:, 128:197, :].rearrange("h p n -> p h n"))
        pw_s = consts.tile([128, 6, C], bf16)
        pbb_s = consts.tile([128, C], f32)
        ones64 = consts.tile([128, 64], bf16)
        nc.gpsimd.memset(ones64[:], 1.0)

        SB = NB // 2                      # superblocks of 4 batches
        MT7 = [(g, min(128, 2 * NP - g)) for g in range(0, 2 * NP, 128)]

        # ---------- emission helpers (software pipelining) ----------
        # PE executes its instruction stream in order, so filler work
        # (next block's qk/v gemms, ready proj m-tiles) is interleaved into
        # the attention emission to keep PE busy while ACT produces E.

        def emit_dma(blk):
            if blk == 1:
                nc.sync.dma_start(
                    out=pw_s[:], in_=pw_d.rearrange("(k p) c -> p k c", p=128))
                nc.sync.dma_start(
                    out=pbb_s[:], in_=bass.AP(tensor=pb_d.tensor, offset=0,
                                              ap=[[0, 128], [1, C]]))
                return xt8_b1
            xt8_s = xt8_p.tile([128, 2, 6, NPP], f8)
            nc.sync.dma_start(out=xt8_s[:, :, :, 0:NP], in_=xT8_d[blk])
            return xt8_s

        def emit_qk_mi(xt8_s, qkt_s, mi):
            # qkT [128, NP] for one mi-tile via split-fp8 DoubleRow gemm;
            # q/k scaled by QS=64, rescale folded into the exp
            ps = pmm.tile([128, NP], f32, tag="pmm")
            for s in range(2):
                for t in range(3):
                    nc.tensor.matmul(
                        ps[:],
                        lhsT=qkw_s[:, 2 * t:2 * t + 2, mi * 128:(mi + 1) * 128],
                        rhs=xt8_s[:, s, 2 * t:2 * t + 2, 0:NP],
                        start=(s == 0 and t == 0), stop=(s == 1 and t == 2),
                        perf_mode=DR,
                    )
            if mi < 6:
                nc.vector.tensor_scalar_add(qkt_s[:, mi, :], ps[:], qb_s[:, mi:mi + 1])
            else:
                nc.vector.tensor_copy(out=qkt_s[:, mi, :], in_=ps[:])

        def emit_v_tile(xt8_s, v_s, j, t, nt):
            # v natural [msz, 6 heads x 64] via split-fp8 gemm
            # (hi*hi + hi*lo + lo*hi; the lo*lo term is negligible)
            r0, msz = MT[t]
            ps = pmm.tile([128, 384], f32, tag="pmm")
            for pi, (sx, sv) in enumerate(((0, 0), (1, 0), (0, 1))):
                for kt in range(3):
                    nc.tensor.matmul(
                        ps[0:msz, :],
                        lhsT=xt8_s[:, sx, 2 * kt:2 * kt + 2,
                                   j * N + r0: j * N + r0 + msz],
                        rhs=vw_s[:, sv, 2 * kt:2 * kt + 2, nt * 384:(nt + 1) * 384],
                        start=(pi == 0 and kt == 0), stop=(pi == 2 and kt == 2),
                        perf_mode=DR,
                    )
            nc.vector.tensor_copy(
                out=v_s[0:msz, j, t, nt * 6:(nt + 1) * 6, :],
                in_=ps[0:msz, :].rearrange("p (h d) -> p h d", h=6),
            )

        def emit_proj_tile(at_s, sb, g0, msz):
            # one m-tile of the superblock projection, + bias, + 1/QS rescale
            stage = stage_p.tile([128, C], f32)
            for nt in range(2):
                ps = pmm.tile([128, 384], f32, tag="pmm")
                for k in range(6):
                    nc.tensor.matmul(
                        ps[0:msz, :],
                        lhsT=at_s[:, k, g0:g0 + msz],
                        rhs=pw_s[:, k, nt * 384:(nt + 1) * 384],
                        start=(k == 0), stop=(k == 5),
                    )
                nc.vector.scalar_tensor_tensor(
                    out=stage[0:msz, nt * 384:(nt + 1) * 384],
                    in0=ps[0:msz, :], scalar=1.0 / QS,
                    in1=pbb_s[0:msz, nt * 384:(nt + 1) * 384],
                    op0=mybir.AluOpType.mult, op1=mybir.AluOpType.add,
                )
            nc.sync.dma_start(
                out=out_d[sb * 2 * NP + g0: sb * 2 * NP + g0 + msz, :],
                in_=stage[0:msz, :],
            )

        def emit_attn(qkt_s, v_s, at_s, off, filler):
            # scores psum holds both batches of the half-block as two CLOSED
            # groups; one exp + one exp(rb)-multiply per psum. Filler closures
            # are drained between the scores and PV groups of each head-pair.
            budget = (sum(c for c, _ in filler) / 6.0) if filler else 0.0
            for hp in range(6):
                es = {}
                for t, (r0, msz) in enumerate(MT):
                    erb_t = erb0_s if t == 0 else erb1_s
                    ps_a = patt.tile([128, NP], f32, tag="patt")
                    ps_b = patt.tile([128, NP], f32, tag="patt")
                    pss = {0: ps_a, 1: ps_b}
                    for j in range(BLK):
                        for hi in range(2):
                            nc.tensor.matmul(
                                pss[hi][0:msz, j * N:(j + 1) * N],
                                lhsT=qkt_s[64 * hi:64 * (hi + 1), 6 + hp,
                                           j * N + r0: j * N + r0 + msz],
                                rhs=qkt_s[64 * hi:64 * (hi + 1), hp, j * N:(j + 1) * N],
                                start=True, stop=True, skip_group_check=True,
                            )
                    for hi in range(2):
                        h = 2 * hp + hi
                        e = e_p.tile([128, NP], bf16, tag="e")
                        nc.scalar.activation(out=e[0:msz, :], in_=pss[hi][0:msz, :],
                                             func=Exp, scale=EXP_SCALE)
                        eng = nc.gpsimd if hi == 0 else nc.vector
                        eng.tensor_mul(e[0:msz, :], e[0:msz, :], erb_t[0:msz, h, :])
                        es[(t, hi)] = e
                acc = 0.0
                while filler and acc < budget:
                    c, fn = filler.pop(0)
                    fn()
                    acc += c
                for j in range(BLK):
                    ps_o = ppv.tile([128, 2 * N], f32, tag="ppv")
                    for hi in range(2):
                        h = 2 * hp + hi
                        for t, (r0, msz) in enumerate(MT):
                            nc.tensor.matmul(
                                ps_o[hi * 64:(hi + 1) * 64, 0:N],
                                lhsT=v_s[0:msz, j, t, h, :],
                                rhs=es[(t, hi)][0:msz, j * N:(j + 1) * N],
                                start=(t == 0), stop=(t == 1),
                                skip_group_check=True,
                            )
                    for hi in range(2):
                        for t, (r0, msz) in enumerate(MT):
                            nc.tensor.matmul(
                                ps_o[hi * 64:(hi + 1) * 64, N:2 * N],
                                lhsT=ones64[0:msz, :],
                                rhs=es[(t, hi)][0:msz, j * N:(j + 1) * N],
                                start=(t == 0), stop=(t == 1),
                                skip_group_check=True,
                            )
                    rcp = rcp_p.tile([128, N], f32, tag="rcp")
                    _act_recip(nc.scalar, rcp[:], ps_o[:, N:2 * N])
                    nc.vector.tensor_mul(
                        at_s[:, hp, off + j * N:off + (j + 1) * N],
                        ps_o[:, 0:N], rcp[:],
                    )
            while filler:
                filler.pop(0)[1]()

        # ---------- pipelined emission ----------
        blk_tiles = {}

        def make_blk_items(blk, xt8_s):
            qkt_s = qkt_p.tile([128, H, NP], bf16)
            v_s = v_p.tile([128, BLK, 2, H, 64], bf16)
            blk_tiles[blk] = (qkt_s, v_s)
            items = [
                (0.25, lambda mi=mi: emit_qk_mi(xt8_s, qkt_s, mi)) for mi in range(12)
            ] + [
                (0.72, lambda j=j, t=t, nt=nt: emit_v_tile(xt8_s, v_s, j, t, nt))
                for j in range(BLK) for t in range(2) for nt in range(2)
            ]
            return items

        # block 0 is emitted straight (nothing to interleave into)
        for _, it in make_blk_items(0, xt8_pre):
            it()

        pending = []
        at_tiles = {}
        for hb in range(NB):
            sb, bh = divmod(hb, 2)
            if bh == 0:
                at_tiles[sb] = at_p.tile([128, 6, 2 * NP], bf16, name="at_s", tag="at_s")
            filler = []
            if hb + 1 < NB:
                xt8_n = emit_dma(hb + 1)
                filler += make_blk_items(hb + 1, xt8_n)
            filler += pending
            pending = []
            qkt_s, v_s = blk_tiles[hb]
            emit_attn(qkt_s, v_s, at_tiles[sb], bh * NP, filler)
            if bh == 0:
                # proj m-tiles fully inside this half-block's columns
                pending += [
                    (0.96, lambda sb=sb, g0=g0, msz=msz:
                     emit_proj_tile(at_tiles[sb], sb, g0, msz))
                    for g0, msz in MT7 if g0 + msz <= NP
                ]
            else:
                pending += [
                    (0.96, lambda sb=sb, g0=g0, msz=msz:
                     emit_proj_tile(at_tiles[sb], sb, g0, msz))
                    for g0, msz in MT7 if g0 + msz > NP
                ]
        for _, it in pending:
            it()
    return nc


_NC = None


def _get_nc():
    global _NC
    if _NC is None:
        _NC = build_nc()
    return _NC


_EXEC = None


def _get_exec():
    """Build the sharded PJRT executable once and reuse it across calls
    (run_bass_via_pjrt re-traces jax.jit on every invocation)."""
    global _EXEC
    if _EXEC is not None:
        return _EXEC
    import jax
    import numpy as _np
    from jax.sharding import Mesh, PartitionSpec
    from jax.experimental.shard_map import shard_map
    import concourse.mybir as mybir_
    from concourse import bass2jax

    nc = _get_nc()
    bass2jax.install_neuronx_cc_hook()
    partition_name = nc.partition_id_tensor.name if nc.partition_id_tensor else None
    in_names, out_names, out_avals = [], [], []
    for alloc in nc.m.functions[0].allocations:
        if not isinstance(alloc, mybir_.MemoryLocationSet):
            continue
        name = alloc.memorylocations[0].name
        if alloc.kind == "ExternalInput":
            if name != partition_name:
                in_names.append(name)
        elif alloc.kind == "ExternalOutput":
            out_names.append(name)
            out_avals.append(jax.core.ShapedArray(
                tuple(alloc.tensor_shape), mybir_.dt.np(alloc.dtype)))
    all_names = list(in_names)
    if partition_name is not None:
        all_names = all_names + [partition_name]

    def _body(*args):
        operands = list(args)
        if partition_name is not None:
            operands.append(bass2jax.partition_id_tensor())
        outs = bass2jax._bass_exec_p.bind(
            *operands,
            out_avals=tuple(out_avals),
            in_names=tuple(all_names),
            out_names=tuple(out_names),
            lowering_input_output_aliases=(),
            sim_require_finite=True,
            sim_require_nnan=True,
            nc=nc,
        )
        return tuple(outs)

    devices = jax.devices()[:N_CORES]
    mesh = Mesh(_np.asarray(devices), ("core",))
    # xT is data-parallel (split on axis 0); every other input is replicated,
    # so it uploads once instead of 8x.
    in_specs = tuple(
        PartitionSpec("core") if name == "xT8" else PartitionSpec()
        for name in in_names
    )
    out_specs = (PartitionSpec("core"),) * len(out_avals)
    sharded = jax.jit(
        shard_map(_body, mesh=mesh, in_specs=in_specs, out_specs=out_specs,
                  check_rep=False),
        keep_unused=True,
    )
    _EXEC = (sharded, in_names, out_names, out_avals)
    return _EXEC


def _prep_host(x, qkv_w, q_bias, v_bias, rel_pos_table, proj_w, proj_b, rel_index,
               rb_mode="mul_pool"):
    x = np.asarray(x, np.float32)
    qkv_w = np.asarray(qkv_w, np.float32)
    xT32 = np.ascontiguousarray(x.transpose(0, 2, 1))          # [B, C, N]
    # split-fp8 x packed to the SBUF tile layout [blk, p, s, k, j, n] so each
    # 2-batch block is ONE contiguous DMA
    x8h = xT32.astype(ml_dtypes.float8_e4m3)
    x8l = (xT32 - x8h.astype(np.float32)).astype(ml_dtypes.float8_e4m3)
    x8 = np.stack([x8h, x8l], axis=1)                          # [B, 2, C, N]
    x8p = x8.reshape(B // 2, 2, 2, 6, 128, N).transpose(0, 4, 2, 3, 1, 5)
    xT8 = np.ascontiguousarray(x8p.reshape(B // 2, 128, 2, 6, 2 * N))
    qk_wT = np.ascontiguousarray(qkv_w[:2 * C].T) * QS         # [C, 2C]
    qk_wT = qk_wT.astype(ml_dtypes.float8_e4m3)
    qb = (np.asarray(q_bias, np.float32) * QS).reshape(6, 128).T.copy()  # [128, 6]
    vw64 = np.ascontiguousarray(qkv_w[2 * C:].T) * QS          # [C, C]
    vwh = vw64.astype(ml_dtypes.float8_e4m3)
    vwl = (vw64 - vwh.astype(np.float32)).astype(ml_dtypes.float8_e4m3)
    v_wT = np.ascontiguousarray(np.stack([vwh, vwl], axis=0))  # [2, C, C]
    proj_wT = np.ascontiguousarray(np.asarray(proj_w, np.float32).T).astype(ml_dtypes.bfloat16)
    pb_eff = (np.asarray(proj_b, np.float32)
              + np.asarray(proj_w, np.float32) @ np.asarray(v_bias, np.float32))
    rb = np.asarray(rel_pos_table, np.float32)[
        np.asarray(rel_index).reshape(-1)].reshape(N, N, H)    # [n, m, h]
    rbT = np.exp(rb.transpose(2, 1, 0))
    rbT = np.concatenate([rbT] * BLK, axis=2)
    erbT = rbT.astype(ml_dtypes.bfloat16)
    return xT8, qk_wT, qb, v_wT, proj_wT, pb_eff.reshape(1, C), erbT


def kernel(x, qkv_w, q_bias, v_bias, rel_pos_table, proj_w, proj_b, rel_index):
    xT8, qk_wT, qb, v_wT, proj_wT, pb_eff, erbT = _prep_host(
        x, qkv_w, q_bias, v_bias, rel_pos_table, proj_w, proj_b, rel_index,
        rb_mode=RB_MODE)
    per_core = {
        "xT8": xT8,                                 # split on axis 0
        "qkw": qk_wT, "vw": v_wT, "pw": proj_wT,
        "pb": pb_eff, "qb": qb, "erb": erbT,
    }
    try:
        sharded, in_names, out_names, out_avals = _get_exec()
        concat_in = [np.ascontiguousarray(per_core[name]) for name in in_names]
        out_arrs = sharded(*concat_in)
        out = np.asarray(out_arrs[out_names.index("out")]).reshape(B, N, C)
    except Exception:
        # Robust fallback: the stock SPMD runner (slower per call, same NEFF).
        in_maps = []
        for c in range(N_CORES):
            m = {k: v for k, v in per_core.items() if k != "xT8"}
            m["xT8"] = np.ascontiguousarray(xT8[c * NB:(c + 1) * NB])
            in_maps.append(m)
        res = run_bass_kernel_spmd(_get_nc(), in_maps, core_ids=list(range(N_CORES)))
        out = np.concatenate(
            [res.results[c]["out"].reshape(BC, N, C) for c in range(N_CORES)], axis=0)
    return out.astype(np.float32)



# revision 57
# speedup vs baseline: 1.2840x; 1.0092x over previous
"""BEiT-style windowed attention (B=128, N=197, C=768, H=12) on 8 TRN2 NeuronCores.

Data-parallel over batch: 16 batches per core, 2-batch half-blocks inside
4-batch superblocks. Host pre-processing casts x and the qkv/v/proj weights to
bf16, folds the attention scale into the q weights/bias, folds v_bias into the
projection bias (softmax rows sum to 1), and pre-gathers exp(rel_pos_bias).

Device pipeline per core, per 2-batch half-block:
  qkT  [1536, 394] = qk_wT.T @ xT      (bf16 matmuls, moving dim 394)
  v    [394, 768]  = xT.T @ v_wT       (bf16) with interleaved ones columns
  S.T  [197, 197]  = kT.T @ qT         (bf16 per head; both batches land in one
                                        [128,394] psum as two closed groups)
  E    = exp(S.T) * exp_rb             (one ACT exp per psum — ACT ops have
                                        ~530ns fixed overhead, so fewer+wider
                                        wins; exp(rb) multiply split DVE/Pool;
                                        no max-subtraction: |scores| < ~3)
  outT [128, 197]  = v.T @ E           (heads of a pair stacked at partitions
                                        0:64/64:128 via output col-groups; the
                                        softmax sums land in cols 197:394 of
                                        the same psum via ones-matmuls)
  attnoutT = outT * recip(colsums)     (one ACT reciprocal + one DVE multiply
                                        per pair — DVE recip is 3.2us/op on HW)
  out  = attnoutT.T @ proj_wT + bias   (bf16, projected once per 4-batch
                                        superblock: 7 M-tiles instead of 8;
                                        bias added via a pre-broadcast tensor)
"""
import sys
sys.path.insert(0, '/opt/trn_rl_repo')

import numpy as np
import ml_dtypes
from contextlib import ExitStack

import concourse.bass as bass
import concourse.tile as tile
from concourse.tile import add_dep_helper
from concourse import mybir
from concourse.bass_utils import run_bass_kernel_spmd
from concourse.vector_clock import ScopedClock, VectorClock

f32 = mybir.dt.float32
f32r = mybir.dt.float32r
bf16 = mybir.dt.bfloat16
f8 = mybir.dt.float8e4
DR = mybir.MatmulPerfMode.DoubleRow

N_CORES = 8
RB_MODE = "ident_pe"
B, N, C, H, HD = 128, 197, 768, 12, 64
BC = B // N_CORES          # batches per core
BLK = 2                    # batches per block
NB = BC // BLK             # blocks per core
NP = BLK * N               # block column width (394)
NPP = 400                  # xt8 tile pitch (DR ldweights needs step%16==0)
SCALE = HD ** -0.5
QS = 64.0                  # fp8 weight pre-scale for the qk gemm
EXP_SCALE = SCALE / (QS * QS)


class TileContextFixed(tile.TileContext):
    """The walrus in this container accepts at most ONE sync wait per
    instruction. Stock Tile attaches several (both on ordinary instructions
    during wait assignment and on the tail drain). Split the extras onto
    same-engine InstNoOps, and emit the tail drain one proc at a time."""

    def _lower_ordered_insts(self, ordered):
        for bb_name, insts in ordered.items():
            i = 0
            while i < len(insts):
                inst = insts[i]
                si = inst.sync_info
                if si is not None and si.on_wait and len(si.on_wait) > 1:
                    waits = list(si.on_wait)
                    inst.sync_info = mybir.SyncInfo(
                        on_wait=[waits[-1]], on_update=list(si.on_update)
                    )
                    nops = [
                        mybir.InstNoOp(
                            name=f"{inst.name}__wsplit{k}",
                            engine=inst.engine,
                            bass_nofuse=True,
                            sync_info=mybir.SyncInfo(on_wait=[w], on_update=[]),
                        )
                        for k, w in enumerate(waits[:-1])
                    ]
                    insts[i:i] = nops
                    i += len(nops)
                i += 1
        return super()._lower_ordered_insts(ordered)

    def _drain_and_barrier(self, tick_clock, wait_clock):
        gc = tick_clock.global_clock
        n = len(gc)
        for i in range(n):
            if gc[i] > 0:
                vc = VectorClock([0] * n)
                vc.require_at_least(i, gc[i])
                d = self.nc.sync.drain()
                wait_clock.add_sem_waits(d.ins, ScopedClock({None: vc}))
        self.nc.all_engine_barrier()
        assert self.sems is not None
        popped = self.nc._tile_sem_poison_stack.pop()
        assert popped is self._sem_poison
        self.nc.clear_and_free_semaphores(list(self.sems.allocated().values()))
        self.nc.all_engine_barrier()


def _act_recip(eng, out, in_):
    imm = lambda v: mybir.ImmediateValue(dtype=f32, value=v)
    return eng.add_instruction(mybir.InstActivation(
        name=eng.bass.get_next_instruction_name(),
        func=mybir.ActivationFunctionType.Reciprocal,
        ins=[eng.lower_ap(in_), imm(0.0), imm(1.0), imm(0.0)],
        outs=[eng.lower_ap(out)],
    ))


def build_nc(rb_mode=RB_MODE, patt_bufs=2, pmm_bufs=2, ppv_bufs=2, e_bufs=8):
    # rb_mode: how exp(S+rb) is formed:
    #   "mul_pool"  E = exp(S) * erb on gpsimd
    #   "mul_dve"   E = exp(S) * erb on DVE
    #   "mul_split" alternate gpsimd/DVE by head parity
    #   "ident_pe"  S += rb via identity matmul on PE, E = exp(S)
    nc = bass.Bass("TRN2", target_bir_lowering=False, debug=False)
    Exp = mybir.ActivationFunctionType.Exp

    xT8_d = nc.dram_tensor("xT8", [NB, 128, 2, 6, NP], f8, kind="ExternalInput").ap()
    qkw_d = nc.dram_tensor("qkw", [C, 2 * C], f8, kind="ExternalInput").ap()
    vw_d = nc.dram_tensor("vw", [2, C, C], f8, kind="ExternalInput").ap()
    pw_d = nc.dram_tensor("pw", [C, C], bf16, kind="ExternalInput").ap()
    pb_d = nc.dram_tensor("pb", [1, C], f32, kind="ExternalInput").ap()
    qb_d = nc.dram_tensor("qb", [128, 6], f32, kind="ExternalInput").ap()

    erb_d = nc.dram_tensor("erb", [H, N, NP], bf16, kind="ExternalInput").ap()
    out_d = nc.dram_tensor("out", [BC * N, C], f32, kind="ExternalOutput").ap()

    MT = ((0, 128), (128, 69))  # (row offset, rows) m-tiles of 197

    with TileContextFixed(nc) as tc, ExitStack() as ctx:
        consts = ctx.enter_context(tc.tile_pool(name="consts", bufs=1))
        xt8_p = ctx.enter_context(tc.tile_pool(name="xt8", bufs=3))
        qkt_p = ctx.enter_context(tc.tile_pool(name="qkt", bufs=3))
        v_p = ctx.enter_context(tc.tile_pool(name="v", bufs=2))
        at_p = ctx.enter_context(tc.tile_pool(name="at", bufs=2))
        e_p = ctx.enter_context(tc.tile_pool(name="e", bufs=e_bufs))
        rcp_p = ctx.enter_context(tc.tile_pool(name="rcp", bufs=4))
        stage_p = ctx.enter_context(tc.tile_pool(name="stage", bufs=3))
        pmm = ctx.enter_context(tc.tile_pool(name="pmm", bufs=pmm_bufs, space="PSUM"))
        patt = ctx.enter_context(tc.tile_pool(name="patt", bufs=patt_bufs, space="PSUM"))
        ppv = ctx.enter_context(tc.tile_pool(name="ppv", bufs=ppv_bufs, space="PSUM"))

        # One serial DMA stream (sync queue), ordered by first consumption:
        # qk weights (chunked by mi group), split-fp8 x for block 0, q bias,
        # split-fp8 v weights, exp(rel-bias), then later blocks / proj consts.
        qkw_s = consts.tile([128, 6, 2 * C], f8)
        qkw_r = qkw_d.rearrange("(k p) c -> p k c", p=128)
        nc.sync.dma_start(out=qkw_s[:, :, 0:512], in_=qkw_r[:, :, 0:512])
        xt8_pre = xt8_p.tile([128, 2, 6, NPP], f8)
        nc.sync.dma_start(out=xt8_pre[:, 0, :, 0:NP], in_=xT8_d[0][:, 0])
        qb_s = consts.tile([128, 6], f32)
        nc.sync.dma_start(out=qb_s[:], in_=qb_d[:])
        nc.sync.dma_start(out=xt8_pre[:, 1, :, 0:NP], in_=xT8_d[0][:, 1])
        nc.sync.dma_start(out=qkw_s[:, :, 512:1024], in_=qkw_r[:, :, 512:1024])
        nc.sync.dma_start(out=qkw_s[:, :, 1024:1536], in_=qkw_r[:, :, 1024:1536])
        vw_s = consts.tile([128, 2, 6, C], f8)
        for s in range(2):
            nc.sync.dma_start(out=vw_s[:, s],
                              in_=vw_d[s].rearrange("(k p) c -> p k c", p=128))
        xt8_b1 = xt8_p.tile([128, 2, 6, NPP], f8)
        nc.sync.dma_start(out=xt8_b1[:, :, :, 0:NP], in_=xT8_d[1])
        erb0_s = consts.tile([128, H, NP], bf16)
        erb1_s = consts.tile([69, H, NP], bf16)
        nc.sync.dma_start(out=erb0_s[:], in_=erb_d[:, 0:128, :].rearrange("h p n -> p h n"))
        nc.sync.dma_start(out=erb1_s[:], in_=erb_d[:, 128:197, :].rearrange("h p n -> p h n"))
        pw_s = consts.tile([128, 6, C], bf16)
        pbb_s = consts.tile([128, C], f32)
        ones64 = consts.tile([128, 64], bf16)
        nc.gpsimd.memset(ones64[:], 1.0)

        SB = NB // 2                      # superblocks of 4 batches
        MT7 = [(g, min(128, 2 * NP - g)) for g in range(0, 2 * NP, 128)]

        # ---------- emission helpers (software pipelining) ----------
        # PE executes its instruction stream in order, so filler work
        # (next block's qk/v gemms, ready proj m-tiles) is interleaved into
        # the attention emission to keep PE busy while ACT produces E.

        def emit_dma(blk):
            xt8_s = xt8_p.tile([128, 2, 6, NPP], f8)
            nc.sync.dma_start(out=xt8_s[:, :, :, 0:NP], in_=xT8_d[blk])
            return xt8_s

        def emit_qk_mi(xt8_s, qkt8, mi):
            # qkT [128, NP] for one mi-tile via split-fp8 DoubleRow gemm;
            # q/k scaled by QS=64, rescale folded into the exp
            ps = pmm.tile([128, NP], f32, tag="pmm")
            for s in range(2):
                for t in range(3):
                    nc.tensor.matmul(
                        ps[:],
                        lhsT=qkw_s[:, 2 * t:2 * t + 2, mi * 128:(mi + 1) * 128],
                        rhs=xt8_s[:, s, 2 * t:2 * t + 2, 0:NP],
                        start=(s == 0 and t == 0), stop=(s == 1 and t == 2),
                        perf_mode=DR,
                    )
            if mi < 6:
                nc.vector.tensor_scalar_add(qkt8[:, mi, 0:NP], ps[:], qb_s[:, mi:mi + 1])
            else:
                nc.vector.tensor_copy(out=qkt8[:, mi, 0:NP], in_=ps[:])

        def emit_v_tile(xt8_s, v_s, j, t, nt):
            # v natural [msz, 6 heads x 64] via split-fp8 gemm
            # (hi*hi + hi*lo + lo*hi; the lo*lo term is negligible)
            r0, msz = MT[t]
            ps = pmm.tile([128, 384], f32, tag="pmm")
            for pi, (sx, sv) in enumerate(((0, 0), (1, 0), (0, 1))):
                for kt in range(3):
                    nc.tensor.matmul(
                        ps[0:msz, :],
                        lhsT=xt8_s[:, sx, 2 * kt:2 * kt + 2,
                                   j * N + r0: j * N + r0 + msz],
                        rhs=vw_s[:, sv, 2 * kt:2 * kt + 2, nt * 384:(nt + 1) * 384],
                        start=(pi == 0 and kt == 0), stop=(pi == 2 and kt == 2),
                        perf_mode=DR,
                    )
            nc.vector.tensor_copy(
                out=v_s[0:msz, j, t, nt * 6:(nt + 1) * 6, :],
                in_=ps[0:msz, :].rearrange("p (h d) -> p h d", h=6),
            )

        def emit_repack(qkt8, qkt8r, b):
            # reshape hd=64 (on partitions) into the DoubleRow [32 x 2] packing.
            # Partition base b = hp%3 (only bases 0/32/64 are addressable by
            # the PE); free slots (mi//3, head parity, d-high) hold the rest.
            # One base-group per call, on the software-DGE queue, emitted as
            # soon as its last prerequisite qk tile (mi=9+b) is in.
            for h2 in range(2):
                for dhi in range(2):
                    p0 = 64 * h2 + 32 * dhi
                    nc.sync.dma_start(
                        out=qkt8r[32 * b:32 * b + 32, :, h2, dhi, 0:NP],
                        in_=qkt8[p0:p0 + 32, b::3, 0:NP],
                    )

        def emit_proj_tile(at_s, sb, g0, msz):
            # one m-tile of the superblock projection, + bias, + 1/QS rescale
            stage = stage_p.tile([128, C], f32)
            for nt in range(2):
                ps = pmm.tile([128, 384], f32, tag="pmm")
                for k in range(6):
                    nc.tensor.matmul(
                        ps[0:msz, :],
                        lhsT=at_s[:, k, g0:g0 + msz],
                        rhs=pw_s[:, k, nt * 384:(nt + 1) * 384],
                        start=(k == 0), stop=(k == 5),
                    )
                nc.vector.scalar_tensor_tensor(
                    out=stage[0:msz, nt * 384:(nt + 1) * 384],
                    in0=ps[0:msz, :], scalar=1.0 / QS,
                    in1=pbb_s[0:msz, nt * 384:(nt + 1) * 384],
                    op0=mybir.AluOpType.mult, op1=mybir.AluOpType.add,
                )
            nc.sync.dma_start(
                out=out_d[sb * 2 * NP + g0: sb * 2 * NP + g0 + msz, :],
                in_=stage[0:msz, :],
            )

        def emit_attn(qkt8r, v_s, at_s, off, filler):
            # scores psum holds both batches of the half-block as two CLOSED
            # groups; one exp + one exp(rb)-multiply per psum. Filler closures
            # are drained between the scores and PV groups of each head-pair.
            budget = (sum(c for c, _ in filler) / 6.0) if filler else 0.0

            def emit_scores(hp):
                es = {}
                for t, (r0, msz) in enumerate(MT):
                    erb_t = erb0_s if t == 0 else erb1_s
                    # both heads of the pair in one 2-bank psum tile: one wide
                    # exp + one wide exp(rb)-multiply instead of two of each
                    pt = patt.tile([128, 2, 512], f32, tag="patt")
                    for j in range(BLK):
                        for hi in range(2):
                            b = 32 * (hp % 3)
                            nc.tensor.matmul(
                                pt[0:msz, hi, j * N:(j + 1) * N],
                                lhsT=qkt8r[b:b + 32, 2 + hp // 3, hi, :,
                                           j * N + r0: j * N + r0 + msz],
                                rhs=qkt8r[b:b + 32, hp // 3, hi, :,
                                          j * N:(j + 1) * N],
                                start=True, stop=True, skip_group_check=True,
                                perf_mode=DR,
                            )
                    e = e_p.tile([128, 2, NP], bf16, tag="e")
                    nc.scalar.activation(out=e[0:msz, :, :], in_=pt[0:msz, :, 0:NP],
                                         func=Exp, scale=EXP_SCALE)
                    nc.vector.tensor_mul(e[0:msz, :, :], e[0:msz, :, :],
                                         erb_t[0:msz, 2 * hp:2 * hp + 2, :])
                    es[t] = e
                return es

            def emit_pv(hp, es):
                for j in range(BLK):
                    ps_o = ppv.tile([128, 2 * N], f32, tag="ppv")
                    for hi in range(2):
                        h = 2 * hp + hi
                        for t, (r0, msz) in enumerate(MT):
                            nc.tensor.matmul(
                                ps_o[hi * 64:(hi + 1) * 64, 0:N],
                                lhsT=v_s[0:msz, j, t, h, :],
                                rhs=es[t][0:msz, hi, j * N:(j + 1) * N],
                                start=(t == 0), stop=(t == 1),
                                skip_group_check=True,
                            )
                    for hi in range(2):
                        for t, (r0, msz) in enumerate(MT):
                            nc.tensor.matmul(
                                ps_o[hi * 64:(hi + 1) * 64, N:2 * N],
                                lhsT=ones64[0:msz, :],
                                rhs=es[t][0:msz, hi, j * N:(j + 1) * N],
                                start=(t == 0), stop=(t == 1),
                                skip_group_check=True,
                            )
                    rcp = rcp_p.tile([128, N], f32, tag="rcp")
                    _act_recip(nc.scalar, rcp[:], ps_o[:, N:2 * N])
                    nc.vector.tensor_mul(
                        at_s[:, hp, off + j * N:off + (j + 1) * N],
                        ps_o[:, 0:N], rcp[:],
                    )

            # PV lags scores/exp by one head-pair so E production has a full
            # slot of slack before its consumer, and the recips never delay
            # the next exp on the in-order ACT queue.
            prev = None
            for hp in range(6):
                es = emit_scores(hp)
                if prev is not None:
                    emit_pv(*prev)
                acc = 0.0
                while filler and acc < budget:
                    c, fn = filler.pop(0)
                    fn()
                    acc += c
                prev = (hp, es)
            emit_pv(*prev)
            while filler:
                filler.pop(0)[1]()

        # ---------- pipelined emission ----------
        blk_tiles = {}

        def make_blk_items(blk, xt8_s):
            qkt8 = qkt_p.tile([128, H, 400], f8)
            qkt8r = qkt_p.tile([96, 4, 2, 2, 400], f8, name="qkt8r", tag="qkt")
            v_s = v_p.tile([128, BLK, 2, H, 64], bf16)
            blk_tiles[blk] = (qkt8r, v_s)
            def qk_item(mi):
                emit_qk_mi(xt8_s, qkt8, mi)
                if mi >= 9:
                    emit_repack(qkt8, qkt8r, mi - 9)
            items = [
                (0.25, lambda mi=mi: qk_item(mi)) for mi in range(12)
            ] + [
                (0.72, lambda j=j, t=t, nt=nt: emit_v_tile(xt8_s, v_s, j, t, nt))
                for nt in range(2) for j in range(BLK) for t in range(2)
            ]
            return items, []  # defer disabled

        # block 0 is emitted straight (nothing to interleave into)
        for _, it in make_blk_items(0, xt8_pre)[0]:
            it()

        pending = []
        at_tiles = {}
        xt8_tiles = {0: xt8_pre, 1: xt8_b1}
        nc.sync.dma_start(
            out=pw_s[:], in_=pw_d.rearrange("(k p) c -> p k c", p=128))
        nc.sync.dma_start(
            out=pbb_s[:], in_=bass.AP(tensor=pb_d.tensor, offset=0,
                                      ap=[[0, 128], [1, C]]))
        for hb in range(NB):
            sb, bh = divmod(hb, 2)
            if bh == 0:
                at_tiles[sb] = at_p.tile([128, 6, 2 * NP], bf16, name="at_s", tag="at_s")
            filler = []
            defer = []
            if hb + 1 < NB:
                if hb + 1 not in xt8_tiles:
                    xt8_tiles[hb + 1] = emit_dma(hb + 1)
                fi, defer = make_blk_items(hb + 1, xt8_tiles[hb + 1])
                filler += fi
            filler += pending
            pending = defer
            qkt8r, v_s = blk_tiles[hb]
            emit_attn(qkt8r, v_s, at_tiles[sb], bh * NP, filler)
            if bh == 0:
                # proj m-tiles fully inside this half-block's columns
                pending += [
                    (0.96, lambda sb=sb, g0=g0, msz=msz:
                     emit_proj_tile(at_tiles[sb], sb, g0, msz))
                    for g0, msz in MT7 if g0 + msz <= NP
                ]
            else:
                pending += [
                    (0.96, lambda sb=sb, g0=g0, msz=msz:
                     emit_proj_tile(at_tiles[sb], sb, g0, msz))
                    for g0, msz in MT7 if g0 + msz > NP
                ]
        for _, it in pending:
            it()
    return nc


_NC = None


def _get_nc():
    global _NC
    if _NC is None:
        _NC = build_nc()
    return _NC


_EXEC = None


def _get_exec():
    """Build the sharded PJRT executable once and reuse it across calls
    (run_bass_via_pjrt re-traces jax.jit on every invocation)."""
    global _EXEC
    if _EXEC is not None:
        return _EXEC
    import jax
    import numpy as _np
    from jax.sharding import Mesh, PartitionSpec
    from jax.experimental.shard_map import shard_map
    import concourse.mybir as mybir_
    from concourse import bass2jax

    nc = _get_nc()
    bass2jax.install_neuronx_cc_hook()
    partition_name = nc.partition_id_tensor.name if nc.partition_id_tensor else None
    in_names, out_names, out_avals = [], [], []
    for alloc in nc.m.functions[0].allocations:
        if not isinstance(alloc, mybir_.MemoryLocationSet):
            continue
        name = alloc.memorylocations[0].name
        if alloc.kind == "ExternalInput":
            if name != partition_name:
                in_names.append(name)
        elif alloc.kind == "ExternalOutput":
            out_names.append(name)
            out_avals.append(jax.core.ShapedArray(
                tuple(alloc.tensor_shape), mybir_.dt.np(alloc.dtype)))
    all_names = list(in_names)
    if partition_name is not None:
        all_names = all_names + [partition_name]

    def _body(*args):
        operands = list(args)
        if partition_name is not None:
            operands.append(bass2jax.partition_id_tensor())
        outs = bass2jax._bass_exec_p.bind(
            *operands,
            out_avals=tuple(out_avals),
            in_names=tuple(all_names),
            out_names=tuple(out_names),
            lowering_input_output_aliases=(),
            sim_require_finite=True,
            sim_require_nnan=True,
            nc=nc,
        )
        return tuple(outs)

    devices = jax.devices()[:N_CORES]
    mesh = Mesh(_np.asarray(devices), ("core",))
    # xT is data-parallel (split on axis 0); every other input is replicated,
    # so it uploads once instead of 8x.
    in_specs = tuple(
        PartitionSpec("core") if name == "xT8" else PartitionSpec()
        for name in in_names
    )
    out_specs = (PartitionSpec("core"),) * len(out_avals)
    sharded = jax.jit(
        shard_map(_body, mesh=mesh, in_specs=in_specs, out_specs=out_specs,
                  check_rep=False),
        keep_unused=True,
    )
    _EXEC = (sharded, in_names, out_names, out_avals)
    return _EXEC


def _prep_host(x, qkv_w, q_bias, v_bias, rel_pos_table, proj_w, proj_b, rel_index,
               rb_mode="mul_pool"):
    x = np.asarray(x, np.float32)
    qkv_w = np.asarray(qkv_w, np.float32)
    xT32 = np.ascontiguousarray(x.transpose(0, 2, 1))          # [B, C, N]
    # split-fp8 x packed to the SBUF tile layout [blk, p, s, k, j, n] so each
    # 2-batch block is ONE contiguous DMA
    x8h = xT32.astype(ml_dtypes.float8_e4m3)
    x8l = (xT32 - x8h.astype(np.float32)).astype(ml_dtypes.float8_e4m3)
    x8 = np.stack([x8h, x8l], axis=1)                          # [B, 2, C, N]
    x8p = x8.reshape(B // 2, 2, 2, 6, 128, N).transpose(0, 4, 2, 3, 1, 5)
    xT8 = np.ascontiguousarray(x8p.reshape(B // 2, 128, 2, 6, 2 * N))
    qk_wT = np.ascontiguousarray(qkv_w[:2 * C].T) * QS         # [C, 2C]
    qk_wT = qk_wT.astype(ml_dtypes.float8_e4m3)
    qb = (np.asarray(q_bias, np.float32) * QS).reshape(6, 128).T.copy()  # [128, 6]
    vw64 = np.ascontiguousarray(qkv_w[2 * C:].T) * QS          # [C, C]
    vwh = vw64.astype(ml_dtypes.float8_e4m3)
    vwl = (vw64 - vwh.astype(np.float32)).astype(ml_dtypes.float8_e4m3)
    v_wT = np.ascontiguousarray(np.stack([vwh, vwl], axis=0))  # [2, C, C]
    proj_wT = np.ascontiguousarray(np.asarray(proj_w, np.float32).T).astype(ml_dtypes.bfloat16)
    pb_eff = (np.asarray(proj_b, np.float32)
              + np.asarray(proj_w, np.float32) @ np.asarray(v_bias, np.float32))
    rb = np.asarray(rel_pos_table, np.float32)[
        np.asarray(rel_index).reshape(-1)].reshape(N, N, H)    # [n, m, h]
    rbT = np.exp(rb.transpose(2, 1, 0))
    rbT = np.concatenate([rbT] * BLK, axis=2)
    erbT = rbT.astype(ml_dtypes.bfloat16)
    return xT8, qk_wT, qb, v_wT, proj_wT, pb_eff.reshape(1, C), erbT


def kernel(x, qkv_w, q_bias, v_bias, rel_pos_table, proj_w, proj_b, rel_index):
    xT8, qk_wT, qb, v_wT, proj_wT, pb_eff, erbT = _prep_host(
        x, qkv_w, q_bias, v_bias, rel_pos_table, proj_w, proj_b, rel_index,
        rb_mode=RB_MODE)
    per_core = {
        "xT8": xT8,                                 # split on axis 0
        "qkw": qk_wT, "vw": v_wT, "pw": proj_wT,
        "pb": pb_eff, "qb": qb, "erb": erbT,
    }
    try:
        sharded, in_names, out_names, out_avals = _get_exec()
        concat_in = [np.ascontiguousarray(per_core[name]) for name in in_names]
        out_arrs = sharded(*concat_in)
        out = np.asarray(out_arrs[out_names.index("out")]).reshape(B, N, C)
    except Exception:
        # Robust fallback: the stock SPMD runner (slower per call, same NEFF).
        in_maps = []
        for c in range(N_CORES):
            m = {k: v for k, v in per_core.items() if k != "xT8"}
            m["xT8"] = np.ascontiguousarray(xT8[c * NB:(c + 1) * NB])
            in_maps.append(m)
        res = run_bass_kernel_spmd(_get_nc(), in_maps, core_ids=list(range(N_CORES)))
        out = np.concatenate(
            [res.results[c]["out"].reshape(BC, N, C) for c in range(N_CORES)], axis=0)
    return out.astype(np.float32)



# revision 89
# speedup vs baseline: 1.3187x; 1.0270x over previous
"""BEiT-style windowed attention (B=128, N=197, C=768, H=12) on 8 TRN2 NeuronCores.

Data-parallel over batch: 16 batches per core, 2-batch blocks. Host packs x as
split-fp8 (hi + lo e4m3, error-feedback residual) in the exact SBUF tile
layout (one DMA per block), quantizes qk/v weights to fp8e4m3 (x64 pre-scale),
keeps proj in bf16, folds v_bias into the projection bias, pre-gathers
exp(rel_pos_bias), and folds the attention scale into the exp's scale
immediate (SCALE/QS^2).

Device pipeline per core, per 2-batch block:
  qkT  [1536, 394] f32->fp8 = qkw8.T @ (x8h + x8l)   (DoubleRow fp8 gemm,
       K=256/inst, 0.5 cyc/col; psum copies write fp8e4m3 directly)
  repack: 12 SBUF->SBUF DMAs reshape hd=64 into the DoubleRow [32, 2] packing
       (partition base = hp%3 since only bases 0/32/64 are PE-addressable)
  v    [394, 768]  = (x8h+x8l) @ (vw8h+vw8l)  (3 of 4 cross terms, DoubleRow;
       more accurate than bf16 and 25% cheaper)
  S.T  [197, 197]  per head = one DoubleRow fp8 matmul (q/k fp8e4m3)
  E    = exp(S.T * SCALE/QS^2) * exp_rb   (both heads of a pair share one
       2-bank psum: one wide ACT exp + one wide DVE multiply per k-chunk)
  outT [128, 197]  = v.T @ E with softmax col-sums via ones-matmuls into the
       same psum; ACT reciprocal + DVE normalize-multiply
  out  = attnoutT.T @ proj_wT * (1/QS) + bias  (bf16, per 4-batch superblock)

Software pipelining: PE executes in order, so next-block qk/v gemms and ready
projection m-tiles are interleaved as time-paced "filler" between the scores
and PV groups of each attention window, and PV lags scores/exp by LAG
head-pairs so ACT's E production never starves the PE.
"""
import sys
sys.path.insert(0, '/opt/trn_rl_repo')

import numpy as np
import ml_dtypes
from contextlib import ExitStack

import concourse.bass as bass
import concourse.tile as tile
from concourse import mybir
from concourse.bass_utils import run_bass_kernel_spmd
from concourse.vector_clock import ScopedClock, VectorClock

f32 = mybir.dt.float32
f32r = mybir.dt.float32r
bf16 = mybir.dt.bfloat16
f8 = mybir.dt.float8e4
DR = mybir.MatmulPerfMode.DoubleRow

N_CORES = 8
RB_MODE = "ident_pe"
B, N, C, H, HD = 128, 197, 768, 12, 64
BC = B // N_CORES          # batches per core
BLK = 2                    # batches per block
NB = BC // BLK             # blocks per core
NP = BLK * N               # block column width (394)
NPP = 400                  # xt8 tile pitch (DR ldweights needs step%16==0)
LAG = 3                    # head-pairs of scores/exp lookahead before PV
SCALE = HD ** -0.5
QS = 64.0                  # fp8 weight pre-scale for the qk gemm
EXP_SCALE = SCALE / (QS * QS)


class TileContextFixed(tile.TileContext):
    """The walrus in this container accepts at most ONE sync wait per
    instruction. Stock Tile attaches several (both on ordinary instructions
    during wait assignment and on the tail drain). Split the extras onto
    same-engine InstNoOps, and emit the tail drain one proc at a time."""

    def _lower_ordered_insts(self, ordered):
        for bb_name, insts in ordered.items():
            i = 0
            while i < len(insts):
                inst = insts[i]
                si = inst.sync_info
                if si is not None and si.on_wait and len(si.on_wait) > 1:
                    waits = list(si.on_wait)
                    inst.sync_info = mybir.SyncInfo(
                        on_wait=[waits[-1]], on_update=list(si.on_update)
                    )
                    nops = [
                        mybir.InstNoOp(
                            name=f"{inst.name}__wsplit{k}",
                            engine=inst.engine,
                            bass_nofuse=True,
                            sync_info=mybir.SyncInfo(on_wait=[w], on_update=[]),
                        )
                        for k, w in enumerate(waits[:-1])
                    ]
                    insts[i:i] = nops
                    i += len(nops)
                i += 1
        return super()._lower_ordered_insts(ordered)

    def _drain_and_barrier(self, tick_clock, wait_clock):
        gc = tick_clock.global_clock
        n = len(gc)
        for i in range(n):
            if gc[i] > 0:
                vc = VectorClock([0] * n)
                vc.require_at_least(i, gc[i])
                d = self.nc.sync.drain()
                wait_clock.add_sem_waits(d.ins, ScopedClock({None: vc}))
        self.nc.all_engine_barrier()
        assert self.sems is not None
        popped = self.nc._tile_sem_poison_stack.pop()
        assert popped is self._sem_poison
        self.nc.clear_and_free_semaphores(list(self.sems.allocated().values()))
        self.nc.all_engine_barrier()


def _act_recip(eng, out, in_):
    imm = lambda v: mybir.ImmediateValue(dtype=f32, value=v)
    return eng.add_instruction(mybir.InstActivation(
        name=eng.bass.get_next_instruction_name(),
        func=mybir.ActivationFunctionType.Reciprocal,
        ins=[eng.lower_ap(in_), imm(0.0), imm(1.0), imm(0.0)],
        outs=[eng.lower_ap(out)],
    ))


def build_nc(rb_mode=RB_MODE, patt_bufs=4, pmm_bufs=2, ppv_bufs=2, e_bufs=10):
    # rb_mode: how exp(S+rb) is formed:
    #   "mul_pool"  E = exp(S) * erb on gpsimd
    #   "mul_dve"   E = exp(S) * erb on DVE
    #   "mul_split" alternate gpsimd/DVE by head parity
    #   "ident_pe"  S += rb via identity matmul on PE, E = exp(S)
    nc = bass.Bass("TRN2", target_bir_lowering=False, debug=False)
    Exp = mybir.ActivationFunctionType.Exp

    xT8_d = nc.dram_tensor("xT8", [NB, 128, 2, 6, NP], f8, kind="ExternalInput").ap()
    qkw_d = nc.dram_tensor("qkw", [C, 2 * C], f8, kind="ExternalInput").ap()
    vw_d = nc.dram_tensor("vw", [2, C, C], f8, kind="ExternalInput").ap()
    pw_d = nc.dram_tensor("pw", [C, C], bf16, kind="ExternalInput").ap()
    pb_d = nc.dram_tensor("pb", [1, C], f32, kind="ExternalInput").ap()
    qb_d = nc.dram_tensor("qb", [128, 6], f32, kind="ExternalInput").ap()

    erb_d = nc.dram_tensor("erb", [H, N, NP], bf16, kind="ExternalInput").ap()
    out_d = nc.dram_tensor("out", [BC * N, C], f32, kind="ExternalOutput").ap()

    MT = ((0, 128), (128, 69))  # (row offset, rows) m-tiles of 197

    with TileContextFixed(nc) as tc, ExitStack() as ctx:
        consts = ctx.enter_context(tc.tile_pool(name="consts", bufs=1))
        xt8_p = ctx.enter_context(tc.tile_pool(name="xt8", bufs=3))
        qkt_p = ctx.enter_context(tc.tile_pool(name="qkt", bufs=3))
        v_p = ctx.enter_context(tc.tile_pool(name="v", bufs=2))
        at_p = ctx.enter_context(tc.tile_pool(name="at", bufs=2))
        e_p = ctx.enter_context(tc.tile_pool(name="e", bufs=e_bufs))
        rcp_p = ctx.enter_context(tc.tile_pool(name="rcp", bufs=4))
        stage_p = ctx.enter_context(tc.tile_pool(name="stage", bufs=3))
        pmm = ctx.enter_context(tc.tile_pool(name="pmm", bufs=pmm_bufs, space="PSUM"))
        patt = ctx.enter_context(tc.tile_pool(name="patt", bufs=patt_bufs, space="PSUM"))
        ppv = ctx.enter_context(tc.tile_pool(name="ppv", bufs=ppv_bufs, space="PSUM"))

        # One serial DMA stream (sync queue), ordered by first consumption:
        # qk weights (chunked by mi group), split-fp8 x for block 0, q bias,
        # split-fp8 v weights, exp(rel-bias), then later blocks / proj consts.
        qkw_s = consts.tile([128, 6, 2 * C], f8)
        qkw_r = qkw_d.rearrange("(k p) c -> p k c", p=128)
        nc.sync.dma_start(out=qkw_s[:, :, 0:512], in_=qkw_r[:, :, 0:512])
        xt8_pre = xt8_p.tile([128, 2, 6, NPP], f8)
        nc.sync.dma_start(out=xt8_pre[:, 0, :, 0:NP], in_=xT8_d[0][:, 0])
        qb_s = consts.tile([128, 6], f32)
        nc.sync.dma_start(out=qb_s[:], in_=qb_d[:])
        nc.sync.dma_start(out=xt8_pre[:, 1, :, 0:NP], in_=xT8_d[0][:, 1])
        nc.sync.dma_start(out=qkw_s[:, :, 512:1024], in_=qkw_r[:, :, 512:1024])
        nc.sync.dma_start(out=qkw_s[:, :, 1024:1536], in_=qkw_r[:, :, 1024:1536])
        vw_s = consts.tile([128, 2, 6, C], f8)
        for s in range(2):
            nc.sync.dma_start(out=vw_s[:, s],
                              in_=vw_d[s].rearrange("(k p) c -> p k c", p=128))
        xt8_b1 = xt8_p.tile([128, 2, 6, NPP], f8)
        nc.sync.dma_start(out=xt8_b1[:, :, :, 0:NP], in_=xT8_d[1])
        erb0_s = consts.tile([128, H, NP], bf16)
        erb1_s = consts.tile([69, H, NP], bf16)
        nc.sync.dma_start(out=erb0_s[:], in_=erb_d[:, 0:128, :].rearrange("h p n -> p h n"))
        nc.sync.dma_start(out=erb1_s[:], in_=erb_d[:, 128:197, :].rearrange("h p n -> p h n"))
        pw_s = consts.tile([128, 6, C], bf16)
        pbb_s = consts.tile([128, C], f32)
        ones64 = consts.tile([128, 64], bf16)
        nc.gpsimd.memset(ones64[:], 1.0)

        SB = NB // 2                      # superblocks of 4 batches
        MT7 = [(g, min(128, 2 * NP - g)) for g in range(0, 2 * NP, 128)]

        # ---------- emission helpers (software pipelining) ----------
        # PE executes its instruction stream in order, so filler work
        # (next block's qk/v gemms, ready proj m-tiles) is interleaved into
        # the attention emission to keep PE busy while ACT produces E.

        def emit_dma(blk):
            xt8_s = xt8_p.tile([128, 2, 6, NPP], f8)
            nc.sync.dma_start(out=xt8_s[:, :, :, 0:NP], in_=xT8_d[blk])
            return xt8_s

        def emit_qk_mi(xt8_s, qkt8, mi):
            # qkT [128, NP] for one mi-tile via split-fp8 DoubleRow gemm;
            # q/k scaled by QS=64, rescale folded into the exp
            ps = pmm.tile([128, NP], f32, tag="pmm")
            for s in range(2):
                for t in range(3):
                    nc.tensor.matmul(
                        ps[:],
                        lhsT=qkw_s[:, 2 * t:2 * t + 2, mi * 128:(mi + 1) * 128],
                        rhs=xt8_s[:, s, 2 * t:2 * t + 2, 0:NP],
                        start=(s == 0 and t == 0), stop=(s == 1 and t == 2),
                        perf_mode=DR,
                    )
            if mi < 6:
                if mi % 2:
                    nc.scalar.add(qkt8[:, mi, 0:NP], ps[:], qb_s[:, mi:mi + 1])
                else:
                    nc.vector.tensor_scalar_add(qkt8[:, mi, 0:NP], ps[:],
                                                qb_s[:, mi:mi + 1])
            else:
                if mi % 2:
                    nc.scalar.copy(out=qkt8[:, mi, 0:NP], in_=ps[:])
                else:
                    nc.vector.tensor_copy(out=qkt8[:, mi, 0:NP], in_=ps[:])

        def emit_v_tile(xt8_s, v_s, j, t, nt):
            # v natural [msz, 6 heads x 64] via split-fp8 gemm
            # (hi*hi + hi*lo + lo*hi; the lo*lo term is negligible)
            r0, msz = MT[t]
            ps = pmm.tile([128, 384], f32, tag="pmm")
            for pi, (sx, sv) in enumerate(((0, 0), (1, 0), (0, 1))):
                for kt in range(3):
                    nc.tensor.matmul(
                        ps[0:msz, :],
                        lhsT=xt8_s[:, sx, 2 * kt:2 * kt + 2,
                                   j * N + r0: j * N + r0 + msz],
                        rhs=vw_s[:, sv, 2 * kt:2 * kt + 2, nt * 384:(nt + 1) * 384],
                        start=(pi == 0 and kt == 0), stop=(pi == 2 and kt == 2),
                        perf_mode=DR,
                    )
            nc.vector.tensor_copy(
                out=v_s[0:msz, j, t, nt * 6:(nt + 1) * 6, :],
                in_=ps[0:msz, :].rearrange("p (h d) -> p h d", h=6),
            )

        def emit_repack(qkt8, qkt8r, b):
            # reshape hd=64 (on partitions) into the DoubleRow [32 x 2] packing.
            # Partition base b = hp%3 (only bases 0/32/64 are addressable by
            # the PE); free slots (mi//3, head parity, d-high) hold the rest.
            # One base-group per call, on the software-DGE queue, emitted as
            # soon as its last prerequisite qk tile (mi=9+b) is in.
            for h2 in range(2):
                for dhi in range(2):
                    p0 = 64 * h2 + 32 * dhi
                    nc.sync.dma_start(
                        out=qkt8r[32 * b:32 * b + 32, :, h2, dhi, 0:NP],
                        in_=qkt8[p0:p0 + 32, b::3, 0:NP],
                    )

        def emit_proj_tile(at_s, sb, g0, msz):
            # one m-tile of the superblock projection, + bias, + 1/QS rescale
            stage = stage_p.tile([128, C], f32)
            for nt in range(2):
                ps = pmm.tile([128, 384], f32, tag="pmm")
                for k in range(6):
                    nc.tensor.matmul(
                        ps[0:msz, :],
                        lhsT=at_s[:, k, g0:g0 + msz],
                        rhs=pw_s[:, k, nt * 384:(nt + 1) * 384],
                        start=(k == 0), stop=(k == 5),
                    )
                nc.vector.scalar_tensor_tensor(
                    out=stage[0:msz, nt * 384:(nt + 1) * 384],
                    in0=ps[0:msz, :], scalar=1.0 / QS,
                    in1=pbb_s[0:msz, nt * 384:(nt + 1) * 384],
                    op0=mybir.AluOpType.mult, op1=mybir.AluOpType.add,
                )
            nc.sync.dma_start(
                out=out_d[sb * 2 * NP + g0: sb * 2 * NP + g0 + msz, :],
                in_=stage[0:msz, :],
            )

        def emit_pv(hp, es, v_s, at_s, off_):
            for j in range(BLK):
                ps_o = ppv.tile([128, 2 * N], f32, tag="ppv")
                for hi in range(2):
                    h = 2 * hp + hi
                    for t, (r0, msz) in enumerate(MT):
                        nc.tensor.matmul(
                            ps_o[hi * 64:(hi + 1) * 64, 0:N],
                            lhsT=v_s[0:msz, j, t, h, :],
                            rhs=es[t][0:msz, hi, j * N:(j + 1) * N],
                            start=(t == 0), stop=(t == 1),
                            skip_group_check=True,
                        )
                for hi in range(2):
                    for t, (r0, msz) in enumerate(MT):
                        nc.tensor.matmul(
                            ps_o[hi * 64:(hi + 1) * 64, N:2 * N],
                            lhsT=ones64[0:msz, :],
                            rhs=es[t][0:msz, hi, j * N:(j + 1) * N],
                            start=(t == 0), stop=(t == 1),
                            skip_group_check=True,
                        )
                rcp = rcp_p.tile([128, N], f32, tag="rcp")
                nc.vector.reciprocal(out=rcp[:], in_=ps_o[:, N:2 * N])
                nc.vector.tensor_mul(
                    at_s[:, hp, off_ + j * N:off_ + (j + 1) * N],
                    ps_o[:, 0:N], rcp[:],
                )

        def emit_attn(qkt8r, v_s, at_s, off, filler, pvq):
            # scores psum holds both batches of the half-block as two CLOSED
            # groups; one exp + one exp(rb)-multiply per psum. Filler closures
            # are drained between the scores and PV groups of each head-pair.
            wts = (1.0, 1.0, 1.0, 1.0, 1.0, 1.0)
            tot = sum(c for c, _ in filler) if filler else 0.0

            def emit_scores(hp):
                es = {}
                for t, (r0, msz) in enumerate(MT):
                    erb_t = erb0_s if t == 0 else erb1_s
                    # per-head single-bank psum tiles: finer ring granularity
                    # at window boundaries; one exp per head, one wide
                    # exp(rb)-multiply over both halves
                    e = e_p.tile([128, 2, NP], bf16, tag="e")
                    for hi in range(2):
                        pt = patt.tile([128, 512], f32, tag="patt")
                        for j in range(BLK):
                            b = 32 * (hp % 3)
                            nc.tensor.matmul(
                                pt[0:msz, j * N:(j + 1) * N],
                                lhsT=qkt8r[b:b + 32, 2 + hp // 3, hi, :,
                                           j * N + r0: j * N + r0 + msz],
                                rhs=qkt8r[b:b + 32, hp // 3, hi, :,
                                          j * N:(j + 1) * N],
                                start=True, stop=True, skip_group_check=True,
                                perf_mode=DR,
                            )
                        nc.scalar.activation(out=e[0:msz, hi, :], in_=pt[0:msz, 0:NP],
                                             func=Exp, scale=EXP_SCALE)
                    nc.vector.tensor_mul(e[0:msz, :, :], e[0:msz, :, :],
                                         erb_t[0:msz, 2 * hp:2 * hp + 2, :])
                    es[t] = e
                return es

            # PV lags scores/exp by one head-pair so E production has a full
            # slot of slack before its consumer, and the recips never delay
            # the next exp on the in-order ACT queue.
            for hp in range(6):
                es = emit_scores(hp)
                if len(pvq) >= LAG:
                    emit_pv(*pvq.pop(0))
                acc = 0.0
                budget = tot * wts[hp] / 6.0
                while filler and acc < budget:
                    c, fn = filler.pop(0)
                    fn()
                    acc += c
                pvq.append((hp, es, v_s, at_s, off))
            while filler:
                filler.pop(0)[1]()

        # ---------- pipelined emission ----------
        blk_tiles = {}

        def make_blk_items(blk, xt8_s):
            qkt8 = qkt_p.tile([128, H, 400], f8)
            qkt8r = qkt_p.tile([96, 4, 2, 2, 400], f8, name="qkt8r", tag="qkt")
            v_s = v_p.tile([128, BLK, 2, H, 64], bf16)
            blk_tiles[blk] = (qkt8r, v_s)
            def qk_item(mi):
                emit_qk_mi(xt8_s, qkt8, mi)
                if mi >= 9:
                    emit_repack(qkt8, qkt8r, mi - 9)
            items = [
                (0.25, lambda mi=mi: qk_item(mi)) for mi in range(12)
            ] + [
                (0.72, lambda j=j, t=t, nt=nt: emit_v_tile(xt8_s, v_s, j, t, nt))
                for nt in range(2) for j in range(BLK) for t in range(2)
            ]
            return items, []  # defer disabled

        # block 0 is mostly emitted straight (nothing to interleave into)
        items0, defer0 = make_blk_items(0, xt8_pre)
        for _, it in items0:
            it()

        pending = list(defer0)
        pvq = []
        at_tiles = {}
        xt8_tiles = {0: xt8_pre, 1: xt8_b1}
        nc.sync.dma_start(
            out=pw_s[:], in_=pw_d.rearrange("(k p) c -> p k c", p=128))
        nc.sync.dma_start(
            out=pbb_s[:], in_=bass.AP(tensor=pb_d.tensor, offset=0,
                                      ap=[[0, 128], [1, C]]))
        for hb in range(NB):
            sb, bh = divmod(hb, 2)
            if bh == 0:
                at_tiles[sb] = at_p.tile([128, 6, 2 * NP], bf16, name="at_s", tag="at_s")
            filler = list(pending) if hb == 0 else []
            if hb + 1 < NB:
                if hb + 1 not in xt8_tiles:
                    xt8_tiles[hb + 1] = emit_dma(hb + 1)
                fi, _ = make_blk_items(hb + 1, xt8_tiles[hb + 1])
                filler += fi
            if hb != 0:
                filler += pending
            pending = []
            qkt8r, v_s = blk_tiles[hb]
            emit_attn(qkt8r, v_s, at_tiles[sb], bh * NP, filler, pvq)
            while pvq:
                emit_pv(*pvq.pop(0))
            if bh == 0:
                # proj m-tiles fully inside this half-block's columns
                pending += [
                    (0.96, lambda sb=sb, g0=g0, msz=msz:
                     emit_proj_tile(at_tiles[sb], sb, g0, msz))
                    for g0, msz in MT7 if g0 + msz <= NP
                ]
            else:
                pending += [
                    (0.96, lambda sb=sb, g0=g0, msz=msz:
                     emit_proj_tile(at_tiles[sb], sb, g0, msz))
                    for g0, msz in MT7 if g0 + msz > NP
                ]
        while pvq:
            emit_pv(*pvq.pop(0))
        for _, it in pending:
            it()
    return nc


_NC = None


def _get_nc():
    global _NC
    if _NC is None:
        _NC = build_nc()
    return _NC


_EXEC = None


def _get_exec():
    """Build the sharded PJRT executable once and reuse it across calls
    (run_bass_via_pjrt re-traces jax.jit on every invocation)."""
    global _EXEC
    if _EXEC is not None:
        return _EXEC
    import jax
    import numpy as _np
    from jax.sharding import Mesh, PartitionSpec
    from jax.experimental.shard_map import shard_map
    import concourse.mybir as mybir_
    from concourse import bass2jax

    nc = _get_nc()
    bass2jax.install_neuronx_cc_hook()
    partition_name = nc.partition_id_tensor.name if nc.partition_id_tensor else None
    in_names, out_names, out_avals = [], [], []
    for alloc in nc.m.functions[0].allocations:
        if not isinstance(alloc, mybir_.MemoryLocationSet):
            continue
        name = alloc.memorylocations[0].name
        if alloc.kind == "ExternalInput":
            if name != partition_name:
                in_names.append(name)
        elif alloc.kind == "ExternalOutput":
            out_names.append(name)
            out_avals.append(jax.core.ShapedArray(
                tuple(alloc.tensor_shape), mybir_.dt.np(alloc.dtype)))
    all_names = list(in_names)
    if partition_name is not None:
        all_names = all_names + [partition_name]

    def _body(*args):
        operands = list(args)
        if partition_name is not None:
            operands.append(bass2jax.partition_id_tensor())
        outs = bass2jax._bass_exec_p.bind(
            *operands,
            out_avals=tuple(out_avals),
            in_names=tuple(all_names),
            out_names=tuple(out_names),
            lowering_input_output_aliases=(),
            sim_require_finite=True,
            sim_require_nnan=True,
            nc=nc,
        )
        return tuple(outs)

    devices = jax.devices()[:N_CORES]
    mesh = Mesh(_np.asarray(devices), ("core",))
    # xT is data-parallel (split on axis 0); every other input is replicated,
    # so it uploads once instead of 8x.
    in_specs = tuple(
        PartitionSpec("core") if name == "xT8" else PartitionSpec()
        for name in in_names
    )
    out_specs = (PartitionSpec("core"),) * len(out_avals)
    sharded = jax.jit(
        shard_map(_body, mesh=mesh, in_specs=in_specs, out_specs=out_specs,
                  check_rep=False),
        keep_unused=True,
    )
    _EXEC = (sharded, in_names, out_names, out_avals)
    return _EXEC


def _prep_host(x, qkv_w, q_bias, v_bias, rel_pos_table, proj_w, proj_b, rel_index,
               rb_mode="mul_pool"):
    x = np.asarray(x, np.float32)
    qkv_w = np.asarray(qkv_w, np.float32)
    xT32 = np.ascontiguousarray(x.transpose(0, 2, 1))          # [B, C, N]
    # split-fp8 x packed to the SBUF tile layout [blk, p, s, k, j, n] so each
    # 2-batch block is ONE contiguous DMA
    x8h = xT32.astype(ml_dtypes.float8_e4m3)
    x8l = (xT32 - x8h.astype(np.float32)).astype(ml_dtypes.float8_e4m3)
    x8 = np.stack([x8h, x8l], axis=1)                          # [B, 2, C, N]
    x8p = x8.reshape(B // 2, 2, 2, 6, 128, N).transpose(0, 4, 2, 3, 1, 5)
    xT8 = np.ascontiguousarray(x8p.reshape(B // 2, 128, 2, 6, 2 * N))
    qk_wT = np.ascontiguousarray(qkv_w[:2 * C].T) * QS         # [C, 2C]
    qk_wT = qk_wT.astype(ml_dtypes.float8_e4m3)
    qb = (np.asarray(q_bias, np.float32) * QS).reshape(6, 128).T.copy()  # [128, 6]
    vw64 = np.ascontiguousarray(qkv_w[2 * C:].T) * QS          # [C, C]
    vwh = vw64.astype(ml_dtypes.float8_e4m3)
    vwl = (vw64 - vwh.astype(np.float32)).astype(ml_dtypes.float8_e4m3)
    v_wT = np.ascontiguousarray(np.stack([vwh, vwl], axis=0))  # [2, C, C]
    proj_wT = np.ascontiguousarray(np.asarray(proj_w, np.float32).T).astype(ml_dtypes.bfloat16)
    pb_eff = (np.asarray(proj_b, np.float32)
              + np.asarray(proj_w, np.float32) @ np.asarray(v_bias, np.float32))
    rb = np.asarray(rel_pos_table, np.float32)[
        np.asarray(rel_index).reshape(-1)].reshape(N, N, H)    # [n, m, h]
    rbT = np.exp(rb.transpose(2, 1, 0))
    rbT = np.concatenate([rbT] * BLK, axis=2)
    erbT = rbT.astype(ml_dtypes.bfloat16)
    return xT8, qk_wT, qb, v_wT, proj_wT, pb_eff.reshape(1, C), erbT


def kernel(x, qkv_w, q_bias, v_bias, rel_pos_table, proj_w, proj_b, rel_index):
    xT8, qk_wT, qb, v_wT, proj_wT, pb_eff, erbT = _prep_host(
        x, qkv_w, q_bias, v_bias, rel_pos_table, proj_w, proj_b, rel_index,
        rb_mode=RB_MODE)
    per_core = {
        "xT8": xT8,                                 # split on axis 0
        "qkw": qk_wT, "vw": v_wT, "pw": proj_wT,
        "pb": pb_eff, "qb": qb, "erb": erbT,
    }
    try:
        sharded, in_names, out_names, out_avals = _get_exec()
        concat_in = [np.ascontiguousarray(per_core[name]) for name in in_names]
        out_arrs = sharded(*concat_in)
        out = np.asarray(out_arrs[out_names.index("out")]).reshape(B, N, C)
    except Exception:
        # Robust fallback: the stock SPMD runner (slower per call, same NEFF).
        in_maps = []
        for c in range(N_CORES):
            m = {k: v for k, v in per_core.items() if k != "xT8"}
            m["xT8"] = np.ascontiguousarray(xT8[c * NB:(c + 1) * NB])
            in_maps.append(m)
        res = run_bass_kernel_spmd(_get_nc(), in_maps, core_ids=list(range(N_CORES)))
        out = np.concatenate(
            [res.results[c]["out"].reshape(BC, N, C) for c in range(N_CORES)], axis=0)
    return out.astype(np.float32)



# revision 92
# speedup vs baseline: 1.3195x; 1.0006x over previous
"""BEiT-style windowed attention (B=128, N=197, C=768, H=12) on 8 TRN2 NeuronCores.

Data-parallel over batch: 16 batches per core, 2-batch blocks. Host packs x as
split-fp8 (hi + lo e4m3, error-feedback residual) in the exact SBUF tile
layout (one DMA per block), quantizes qk/v weights to fp8e4m3 (x64 pre-scale),
keeps proj in bf16, folds v_bias into the projection bias, pre-gathers
exp(rel_pos_bias), and folds the attention scale into the exp's scale
immediate (SCALE/QS^2).

Device pipeline per core, per 2-batch block:
  qkT  [1536, 394] f32->fp8 = qkw8.T @ (x8h + x8l)   (DoubleRow fp8 gemm,
       K=256/inst, 0.5 cyc/col; psum copies write fp8e4m3 directly)
  repack: 12 SBUF->SBUF DMAs reshape hd=64 into the DoubleRow [32, 2] packing
       (partition base = hp%3 since only bases 0/32/64 are PE-addressable)
  v    [394, 768]  = (x8h+x8l) @ (vw8h+vw8l)  (3 of 4 cross terms, DoubleRow;
       more accurate than bf16 and 25% cheaper)
  S.T  [197, 197]  per head = one DoubleRow fp8 matmul (q/k fp8e4m3)
  E    = exp(S.T * SCALE/QS^2) * exp_rb   (both heads of a pair share one
       2-bank psum: one wide ACT exp + one wide DVE multiply per k-chunk)
  outT [128, 197]  = v.T @ E with softmax col-sums via ones-matmuls into the
       same psum; ACT reciprocal + DVE normalize-multiply
  out  = attnoutT.T @ proj_wT * (1/QS) + bias  (bf16, per 4-batch superblock)

Software pipelining: PE executes in order, so next-block qk/v gemms and ready
projection m-tiles are interleaved as time-paced "filler" between the scores
and PV groups of each attention window, and PV lags scores/exp by LAG
head-pairs so ACT's E production never starves the PE.
"""
import sys
sys.path.insert(0, '/opt/trn_rl_repo')

import numpy as np
import ml_dtypes
from contextlib import ExitStack

import concourse.bass as bass
import concourse.tile as tile
from concourse import mybir
from concourse.bass_utils import run_bass_kernel_spmd
from concourse.vector_clock import ScopedClock, VectorClock

f32 = mybir.dt.float32
f32r = mybir.dt.float32r
bf16 = mybir.dt.bfloat16
f8 = mybir.dt.float8e4
DR = mybir.MatmulPerfMode.DoubleRow

N_CORES = 8
RB_MODE = "ident_pe"
B, N, C, H, HD = 128, 197, 768, 12, 64
BC = B // N_CORES          # batches per core
BLK = 2                    # batches per block
NB = BC // BLK             # blocks per core
NP = BLK * N               # block column width (394)
NPP = 400                  # xt8 tile pitch (DR ldweights needs step%16==0)
LAG = 3                    # head-pairs of scores/exp lookahead before PV
SCALE = HD ** -0.5
QS = 64.0                  # fp8 weight pre-scale for the qk gemm
EXP_SCALE = SCALE / (QS * QS)


class TileContextFixed(tile.TileContext):
    """The walrus in this container accepts at most ONE sync wait per
    instruction. Stock Tile attaches several (both on ordinary instructions
    during wait assignment and on the tail drain). Split the extras onto
    same-engine InstNoOps, and emit the tail drain one proc at a time."""

    def _lower_ordered_insts(self, ordered):
        for bb_name, insts in ordered.items():
            i = 0
            while i < len(insts):
                inst = insts[i]
                si = inst.sync_info
                if si is not None and si.on_wait and len(si.on_wait) > 1:
                    waits = list(si.on_wait)
                    inst.sync_info = mybir.SyncInfo(
                        on_wait=[waits[-1]], on_update=list(si.on_update)
                    )
                    nops = [
                        mybir.InstNoOp(
                            name=f"{inst.name}__wsplit{k}",
                            engine=inst.engine,
                            bass_nofuse=True,
                            sync_info=mybir.SyncInfo(on_wait=[w], on_update=[]),
                        )
                        for k, w in enumerate(waits[:-1])
                    ]
                    insts[i:i] = nops
                    i += len(nops)
                i += 1
        return super()._lower_ordered_insts(ordered)

    def _drain_and_barrier(self, tick_clock, wait_clock):
        gc = tick_clock.global_clock
        n = len(gc)
        for i in range(n):
            if gc[i] > 0:
                vc = VectorClock([0] * n)
                vc.require_at_least(i, gc[i])
                d = self.nc.sync.drain()
                wait_clock.add_sem_waits(d.ins, ScopedClock({None: vc}))
        self.nc.all_engine_barrier()
        assert self.sems is not None
        popped = self.nc._tile_sem_poison_stack.pop()
        assert popped is self._sem_poison
        self.nc.clear_and_free_semaphores(list(self.sems.allocated().values()))
        self.nc.all_engine_barrier()


def _act_recip(eng, out, in_):
    imm = lambda v: mybir.ImmediateValue(dtype=f32, value=v)
    return eng.add_instruction(mybir.InstActivation(
        name=eng.bass.get_next_instruction_name(),
        func=mybir.ActivationFunctionType.Reciprocal,
        ins=[eng.lower_ap(in_), imm(0.0), imm(1.0), imm(0.0)],
        outs=[eng.lower_ap(out)],
    ))


def build_nc(rb_mode=RB_MODE, patt_bufs=4, pmm_bufs=2, ppv_bufs=2, e_bufs=10):
    # rb_mode: how exp(S+rb) is formed:
    #   "mul_pool"  E = exp(S) * erb on gpsimd
    #   "mul_dve"   E = exp(S) * erb on DVE
    #   "mul_split" alternate gpsimd/DVE by head parity
    #   "ident_pe"  S += rb via identity matmul on PE, E = exp(S)
    nc = bass.Bass("TRN2", target_bir_lowering=False, debug=False)
    Exp = mybir.ActivationFunctionType.Exp

    xT8_d = nc.dram_tensor("xT8", [NB, 128, 2, 6, NP], f8, kind="ExternalInput").ap()
    qkw_d = nc.dram_tensor("qkw", [C, 2 * C], f8, kind="ExternalInput").ap()
    vw_d = nc.dram_tensor("vw", [2, C, C], f8, kind="ExternalInput").ap()
    pw_d = nc.dram_tensor("pw", [C, C], bf16, kind="ExternalInput").ap()
    pb_d = nc.dram_tensor("pb", [1, C], f32, kind="ExternalInput").ap()
    qb_d = nc.dram_tensor("qb", [128, 6], f32, kind="ExternalInput").ap()

    erb_d = nc.dram_tensor("erb", [H, N, NP], bf16, kind="ExternalInput").ap()
    out_d = nc.dram_tensor("out", [BC * N, C], f32, kind="ExternalOutput").ap()

    MT = ((0, 128), (128, 69))  # (row offset, rows) m-tiles of 197

    with TileContextFixed(nc) as tc, ExitStack() as ctx:
        consts = ctx.enter_context(tc.tile_pool(name="consts", bufs=1))
        xt8_p = ctx.enter_context(tc.tile_pool(name="xt8", bufs=3))
        qkt_p = ctx.enter_context(tc.tile_pool(name="qkt", bufs=4))
        v_p = ctx.enter_context(tc.tile_pool(name="v", bufs=2))
        at_p = ctx.enter_context(tc.tile_pool(name="at", bufs=2))
        e_p = ctx.enter_context(tc.tile_pool(name="e", bufs=e_bufs))
        rcp_p = ctx.enter_context(tc.tile_pool(name="rcp", bufs=4))
        stage_p = ctx.enter_context(tc.tile_pool(name="stage", bufs=3))
        pmm = ctx.enter_context(tc.tile_pool(name="pmm", bufs=pmm_bufs, space="PSUM"))
        patt = ctx.enter_context(tc.tile_pool(name="patt", bufs=patt_bufs, space="PSUM"))
        ppv = ctx.enter_context(tc.tile_pool(name="ppv", bufs=ppv_bufs, space="PSUM"))

        # One serial DMA stream (sync queue), ordered by first consumption:
        # qk weights (chunked by mi group), split-fp8 x for block 0, q bias,
        # split-fp8 v weights, exp(rel-bias), then later blocks / proj consts.
        qkw_s = consts.tile([128, 6, 2 * C], f8)
        qkw_r = qkw_d.rearrange("(k p) c -> p k c", p=128)
        nc.sync.dma_start(out=qkw_s[:, :, 0:512], in_=qkw_r[:, :, 0:512])
        xt8_pre = xt8_p.tile([128, 2, 6, NPP], f8)
        nc.sync.dma_start(out=xt8_pre[:, 0, :, 0:NP], in_=xT8_d[0][:, 0])
        qb_s = consts.tile([128, 6], f32)
        nc.sync.dma_start(out=qb_s[:], in_=qb_d[:])
        nc.sync.dma_start(out=xt8_pre[:, 1, :, 0:NP], in_=xT8_d[0][:, 1])
        nc.sync.dma_start(out=qkw_s[:, :, 512:1024], in_=qkw_r[:, :, 512:1024])
        nc.sync.dma_start(out=qkw_s[:, :, 1024:1536], in_=qkw_r[:, :, 1024:1536])
        vw_s = consts.tile([128, 2, 6, C], f8)
        for s in range(2):
            nc.sync.dma_start(out=vw_s[:, s],
                              in_=vw_d[s].rearrange("(k p) c -> p k c", p=128))
        xt8_b1 = xt8_p.tile([128, 2, 6, NPP], f8)
        nc.sync.dma_start(out=xt8_b1[:, :, :, 0:NP], in_=xT8_d[1])
        erb0_s = consts.tile([128, H, NP], bf16)
        erb1_s = consts.tile([69, H, NP], bf16)
        nc.sync.dma_start(out=erb0_s[:], in_=erb_d[:, 0:128, :].rearrange("h p n -> p h n"))
        nc.sync.dma_start(out=erb1_s[:], in_=erb_d[:, 128:197, :].rearrange("h p n -> p h n"))
        pw_s = consts.tile([128, 6, C], bf16)
        pbb_s = consts.tile([128, C], f32)
        ones64 = consts.tile([128, 64], bf16)
        nc.gpsimd.memset(ones64[:], 1.0)

        SB = NB // 2                      # superblocks of 4 batches
        MT7 = [(g, min(128, 2 * NP - g)) for g in range(0, 2 * NP, 128)]

        # ---------- emission helpers (software pipelining) ----------
        # PE executes its instruction stream in order, so filler work
        # (next block's qk/v gemms, ready proj m-tiles) is interleaved into
        # the attention emission to keep PE busy while ACT produces E.

        def emit_dma(blk):
            xt8_s = xt8_p.tile([128, 2, 6, NPP], f8)
            nc.sync.dma_start(out=xt8_s[:, :, :, 0:NP], in_=xT8_d[blk])
            return xt8_s

        def emit_qk_mi(xt8_s, qkt8, mi):
            # qkT [128, NP] for one mi-tile via split-fp8 DoubleRow gemm;
            # q/k scaled by QS=64, rescale folded into the exp
            ps = pmm.tile([128, NP], f32, tag="pmm")
            for s in range(2):
                for t in range(3):
                    nc.tensor.matmul(
                        ps[:],
                        lhsT=qkw_s[:, 2 * t:2 * t + 2, mi * 128:(mi + 1) * 128],
                        rhs=xt8_s[:, s, 2 * t:2 * t + 2, 0:NP],
                        start=(s == 0 and t == 0), stop=(s == 1 and t == 2),
                        perf_mode=DR,
                    )
            if mi < 6:
                if mi % 2:
                    nc.scalar.add(qkt8[:, mi, 0:NP], ps[:], qb_s[:, mi:mi + 1])
                else:
                    nc.vector.tensor_scalar_add(qkt8[:, mi, 0:NP], ps[:],
                                                qb_s[:, mi:mi + 1])
            else:
                if mi % 2:
                    nc.scalar.copy(out=qkt8[:, mi, 0:NP], in_=ps[:])
                else:
                    nc.vector.tensor_copy(out=qkt8[:, mi, 0:NP], in_=ps[:])

        def emit_v_tile(xt8_s, v_s, j, t, nt):
            # v natural [msz, 6 heads x 64] via split-fp8 gemm
            # (hi*hi + hi*lo + lo*hi; the lo*lo term is negligible)
            r0, msz = MT[t]
            ps = pmm.tile([128, 384], f32, tag="pmm")
            for pi, (sx, sv) in enumerate(((0, 0), (1, 0), (0, 1))):
                for kt in range(3):
                    nc.tensor.matmul(
                        ps[0:msz, :],
                        lhsT=xt8_s[:, sx, 2 * kt:2 * kt + 2,
                                   j * N + r0: j * N + r0 + msz],
                        rhs=vw_s[:, sv, 2 * kt:2 * kt + 2, nt * 384:(nt + 1) * 384],
                        start=(pi == 0 and kt == 0), stop=(pi == 2 and kt == 2),
                        perf_mode=DR,
                    )
            nc.vector.tensor_copy(
                out=v_s[0:msz, j, t, nt * 6:(nt + 1) * 6, :],
                in_=ps[0:msz, :].rearrange("p (h d) -> p h d", h=6),
            )

        def emit_repack(qkt8, qkt8r, b):
            # reshape hd=64 (on partitions) into the DoubleRow [32 x 2] packing.
            # Partition base b = hp%3 (only bases 0/32/64 are addressable by
            # the PE); free slots (mi//3, head parity, d-high) hold the rest.
            # One base-group per call, on the software-DGE queue, emitted as
            # soon as its last prerequisite qk tile (mi=9+b) is in.
            for h2 in range(2):
                for dhi in range(2):
                    p0 = 64 * h2 + 32 * dhi
                    nc.sync.dma_start(
                        out=qkt8r[32 * b:32 * b + 32, :, h2, dhi, 0:NP],
                        in_=qkt8[p0:p0 + 32, b::3, 0:NP],
                    )

        def emit_proj_tile(at_s, sb, g0, msz):
            # one m-tile of the superblock projection, + bias, + 1/QS rescale
            stage = stage_p.tile([128, C], f32)
            for nt in range(2):
                ps = pmm.tile([128, 384], f32, tag="pmm")
                for k in range(6):
                    nc.tensor.matmul(
                        ps[0:msz, :],
                        lhsT=at_s[:, k, g0:g0 + msz],
                        rhs=pw_s[:, k, nt * 384:(nt + 1) * 384],
                        start=(k == 0), stop=(k == 5),
                    )
                nc.vector.scalar_tensor_tensor(
                    out=stage[0:msz, nt * 384:(nt + 1) * 384],
                    in0=ps[0:msz, :], scalar=1.0 / QS,
                    in1=pbb_s[0:msz, nt * 384:(nt + 1) * 384],
                    op0=mybir.AluOpType.mult, op1=mybir.AluOpType.add,
                )
            nc.sync.dma_start(
                out=out_d[sb * 2 * NP + g0: sb * 2 * NP + g0 + msz, :],
                in_=stage[0:msz, :],
            )

        def emit_pv(hp, es, v_s, at_s, off_):
            for j in range(BLK):
                ps_o = ppv.tile([128, 2 * N], f32, tag="ppv")
                for hi in range(2):
                    h = 2 * hp + hi
                    for t, (r0, msz) in enumerate(MT):
                        nc.tensor.matmul(
                            ps_o[hi * 64:(hi + 1) * 64, 0:N],
                            lhsT=v_s[0:msz, j, t, h, :],
                            rhs=es[t][0:msz, hi, j * N:(j + 1) * N],
                            start=(t == 0), stop=(t == 1),
                            skip_group_check=True,
                        )
                for hi in range(2):
                    for t, (r0, msz) in enumerate(MT):
                        nc.tensor.matmul(
                            ps_o[hi * 64:(hi + 1) * 64, N:2 * N],
                            lhsT=ones64[0:msz, :],
                            rhs=es[t][0:msz, hi, j * N:(j + 1) * N],
                            start=(t == 0), stop=(t == 1),
                            skip_group_check=True,
                        )
                rcp = rcp_p.tile([128, N], f32, tag="rcp")
                nc.vector.reciprocal(out=rcp[:], in_=ps_o[:, N:2 * N])
                nc.vector.tensor_mul(
                    at_s[:, hp, off_ + j * N:off_ + (j + 1) * N],
                    ps_o[:, 0:N], rcp[:],
                )

        def emit_attn(qkt8r, v_s, at_s, off, filler, pvq):
            # scores psum holds both batches of the half-block as two CLOSED
            # groups; one exp + one exp(rb)-multiply per psum. Filler closures
            # are drained between the scores and PV groups of each head-pair.
            wts = (1.0, 1.0, 1.0, 1.0, 1.0, 1.0)
            tot = sum(c for c, _ in filler) if filler else 0.0

            def emit_scores(hp):
                es = {}
                for t, (r0, msz) in enumerate(MT):
                    erb_t = erb0_s if t == 0 else erb1_s
                    # per-head single-bank psum tiles: finer ring granularity
                    # at window boundaries; one exp per head, one wide
                    # exp(rb)-multiply over both halves
                    e = e_p.tile([128, 2, NP], bf16, tag="e")
                    for hi in range(2):
                        pt = patt.tile([128, 512], f32, tag="patt")
                        for j in range(BLK):
                            b = 32 * (hp % 3)
                            nc.tensor.matmul(
                                pt[0:msz, j * N:(j + 1) * N],
                                lhsT=qkt8r[b:b + 32, 2 + hp // 3, hi, :,
                                           j * N + r0: j * N + r0 + msz],
                                rhs=qkt8r[b:b + 32, hp // 3, hi, :,
                                          j * N:(j + 1) * N],
                                start=True, stop=True, skip_group_check=True,
                                perf_mode=DR,
                            )
                        nc.scalar.activation(out=e[0:msz, hi, :], in_=pt[0:msz, 0:NP],
                                             func=Exp, scale=EXP_SCALE)
                    nc.vector.tensor_mul(e[0:msz, :, :], e[0:msz, :, :],
                                         erb_t[0:msz, 2 * hp:2 * hp + 2, :])
                    es[t] = e
                return es

            # PV lags scores/exp by one head-pair so E production has a full
            # slot of slack before its consumer, and the recips never delay
            # the next exp on the in-order ACT queue.
            for hp in range(6):
                es = emit_scores(hp)
                if len(pvq) >= LAG:
                    emit_pv(*pvq.pop(0))
                acc = 0.0
                budget = tot * wts[hp] / 6.0
                while filler and acc < budget:
                    c, fn = filler.pop(0)
                    fn()
                    acc += c
                pvq.append((hp, es, v_s, at_s, off))
            while filler:
                filler.pop(0)[1]()

        # ---------- pipelined emission ----------
        blk_tiles = {}

        def make_blk_items(blk, xt8_s):
            qkt8 = qkt_p.tile([128, H, 400], f8)
            qkt8r = qkt_p.tile([96, 4, 2, 2, 400], f8, name="qkt8r", tag="qkt")
            v_s = v_p.tile([128, BLK, 2, H, 64], bf16)
            blk_tiles[blk] = (qkt8r, v_s)
            def qk_item(mi):
                emit_qk_mi(xt8_s, qkt8, mi)
                if mi >= 9:
                    emit_repack(qkt8, qkt8r, mi - 9)
            items = [
                (0.25, lambda mi=mi: qk_item(mi)) for mi in range(12)
            ] + [
                (0.72, lambda j=j, t=t, nt=nt: emit_v_tile(xt8_s, v_s, j, t, nt))
                for nt in range(2) for j in range(BLK) for t in range(2)
            ]
            return items, []  # defer disabled

        # block 0 is mostly emitted straight (nothing to interleave into)
        items0, defer0 = make_blk_items(0, xt8_pre)
        for _, it in items0:
            it()

        pending = list(defer0)
        pvq = []
        at_tiles = {}
        xt8_tiles = {0: xt8_pre, 1: xt8_b1}
        nc.sync.dma_start(
            out=pw_s[:], in_=pw_d.rearrange("(k p) c -> p k c", p=128))
        nc.sync.dma_start(
            out=pbb_s[:], in_=bass.AP(tensor=pb_d.tensor, offset=0,
                                      ap=[[0, 128], [1, C]]))
        for hb in range(NB):
            sb, bh = divmod(hb, 2)
            if bh == 0:
                at_tiles[sb] = at_p.tile([128, 6, 2 * NP], bf16, name="at_s", tag="at_s")
            filler = list(pending) if hb == 0 else []
            if hb + 1 < NB:
                if hb + 1 not in xt8_tiles:
                    xt8_tiles[hb + 1] = emit_dma(hb + 1)
                fi, _ = make_blk_items(hb + 1, xt8_tiles[hb + 1])
                filler += fi
            if hb != 0:
                filler += pending
            pending = []
            qkt8r, v_s = blk_tiles[hb]
            emit_attn(qkt8r, v_s, at_tiles[sb], bh * NP, filler, pvq)
            while pvq:
                emit_pv(*pvq.pop(0))
            if bh == 0:
                # proj m-tiles fully inside this half-block's columns
                pending += [
                    (0.96, lambda sb=sb, g0=g0, msz=msz:
                     emit_proj_tile(at_tiles[sb], sb, g0, msz))
                    for g0, msz in MT7 if g0 + msz <= NP
                ]
            else:
                pending += [
                    (0.96, lambda sb=sb, g0=g0, msz=msz:
                     emit_proj_tile(at_tiles[sb], sb, g0, msz))
                    for g0, msz in MT7 if g0 + msz > NP
                ]
        while pvq:
            emit_pv(*pvq.pop(0))
        for _, it in pending:
            it()
    return nc


_NC = None


def _get_nc():
    global _NC
    if _NC is None:
        _NC = build_nc()
    return _NC


_EXEC = None


def _get_exec():
    """Build the sharded PJRT executable once and reuse it across calls
    (run_bass_via_pjrt re-traces jax.jit on every invocation)."""
    global _EXEC
    if _EXEC is not None:
        return _EXEC
    import jax
    import numpy as _np
    from jax.sharding import Mesh, PartitionSpec
    from jax.experimental.shard_map import shard_map
    import concourse.mybir as mybir_
    from concourse import bass2jax

    nc = _get_nc()
    bass2jax.install_neuronx_cc_hook()
    partition_name = nc.partition_id_tensor.name if nc.partition_id_tensor else None
    in_names, out_names, out_avals = [], [], []
    for alloc in nc.m.functions[0].allocations:
        if not isinstance(alloc, mybir_.MemoryLocationSet):
            continue
        name = alloc.memorylocations[0].name
        if alloc.kind == "ExternalInput":
            if name != partition_name:
                in_names.append(name)
        elif alloc.kind == "ExternalOutput":
            out_names.append(name)
            out_avals.append(jax.core.ShapedArray(
                tuple(alloc.tensor_shape), mybir_.dt.np(alloc.dtype)))
    all_names = list(in_names)
    if partition_name is not None:
        all_names = all_names + [partition_name]

    def _body(*args):
        operands = list(args)
        if partition_name is not None:
            operands.append(bass2jax.partition_id_tensor())
        outs = bass2jax._bass_exec_p.bind(
            *operands,
            out_avals=tuple(out_avals),
            in_names=tuple(all_names),
            out_names=tuple(out_names),
            lowering_input_output_aliases=(),
            sim_require_finite=True,
            sim_require_nnan=True,
            nc=nc,
        )
        return tuple(outs)

    devices = jax.devices()[:N_CORES]
    mesh = Mesh(_np.asarray(devices), ("core",))
    # xT is data-parallel (split on axis 0); every other input is replicated,
    # so it uploads once instead of 8x.
    in_specs = tuple(
        PartitionSpec("core") if name == "xT8" else PartitionSpec()
        for name in in_names
    )
    out_specs = (PartitionSpec("core"),) * len(out_avals)
    sharded = jax.jit(
        shard_map(_body, mesh=mesh, in_specs=in_specs, out_specs=out_specs,
                  check_rep=False),
        keep_unused=True,
    )
    _EXEC = (sharded, in_names, out_names, out_avals)
    return _EXEC


def _prep_host(x, qkv_w, q_bias, v_bias, rel_pos_table, proj_w, proj_b, rel_index,
               rb_mode="mul_pool"):
    x = np.asarray(x, np.float32)
    qkv_w = np.asarray(qkv_w, np.float32)
    xT32 = np.ascontiguousarray(x.transpose(0, 2, 1))          # [B, C, N]
    # split-fp8 x packed to the SBUF tile layout [blk, p, s, k, j, n] so each
    # 2-batch block is ONE contiguous DMA
    x8h = xT32.astype(ml_dtypes.float8_e4m3)
    x8l = (xT32 - x8h.astype(np.float32)).astype(ml_dtypes.float8_e4m3)
    x8 = np.stack([x8h, x8l], axis=1)                          # [B, 2, C, N]
    x8p = x8.reshape(B // 2, 2, 2, 6, 128, N).transpose(0, 4, 2, 3, 1, 5)
    xT8 = np.ascontiguousarray(x8p.reshape(B // 2, 128, 2, 6, 2 * N))
    qk_wT = np.ascontiguousarray(qkv_w[:2 * C].T) * QS         # [C, 2C]
    qk_wT = qk_wT.astype(ml_dtypes.float8_e4m3)
    qb = (np.asarray(q_bias, np.float32) * QS).reshape(6, 128).T.copy()  # [128, 6]
    vw64 = np.ascontiguousarray(qkv_w[2 * C:].T) * QS          # [C, C]
    vwh = vw64.astype(ml_dtypes.float8_e4m3)
    vwl = (vw64 - vwh.astype(np.float32)).astype(ml_dtypes.float8_e4m3)
    v_wT = np.ascontiguousarray(np.stack([vwh, vwl], axis=0))  # [2, C, C]
    proj_wT = np.ascontiguousarray(np.asarray(proj_w, np.float32).T).astype(ml_dtypes.bfloat16)
    pb_eff = (np.asarray(proj_b, np.float32)
              + np.asarray(proj_w, np.float32) @ np.asarray(v_bias, np.float32))
    rb = np.asarray(rel_pos_table, np.float32)[
        np.asarray(rel_index).reshape(-1)].reshape(N, N, H)    # [n, m, h]
    rbT = np.exp(rb.transpose(2, 1, 0))
    rbT = np.concatenate([rbT] * BLK, axis=2)
    erbT = rbT.astype(ml_dtypes.bfloat16)
    return xT8, qk_wT, qb, v_wT, proj_wT, pb_eff.reshape(1, C), erbT


def kernel(x, qkv_w, q_bias, v_bias, rel_pos_table, proj_w, proj_b, rel_index):
    xT8, qk_wT, qb, v_wT, proj_wT, pb_eff, erbT = _prep_host(
        x, qkv_w, q_bias, v_bias, rel_pos_table, proj_w, proj_b, rel_index,
        rb_mode=RB_MODE)
    per_core = {
        "xT8": xT8,                                 # split on axis 0
        "qkw": qk_wT, "vw": v_wT, "pw": proj_wT,
        "pb": pb_eff, "qb": qb, "erb": erbT,
    }
    try:
        sharded, in_names, out_names, out_avals = _get_exec()
        concat_in = [np.ascontiguousarray(per_core[name]) for name in in_names]
        out_arrs = sharded(*concat_in)
        out = np.asarray(out_arrs[out_names.index("out")]).reshape(B, N, C)
    except Exception:
        # Robust fallback: the stock SPMD runner (slower per call, same NEFF).
        in_maps = []
        for c in range(N_CORES):
            m = {k: v for k, v in per_core.items() if k != "xT8"}
            m["xT8"] = np.ascontiguousarray(xT8[c * NB:(c + 1) * NB])
            in_maps.append(m)
        res = run_bass_kernel_spmd(_get_nc(), in_maps, core_ids=list(range(N_CORES)))
        out = np.concatenate(
            [res.results[c]["out"].reshape(BC, N, C) for c in range(N_CORES)], axis=0)
    return out.astype(np.float32)



# revision 93
# speedup vs baseline: 1.3208x; 1.0010x over previous
"""BEiT-style windowed attention (B=128, N=197, C=768, H=12) on 8 TRN2 NeuronCores.

Data-parallel over batch: 16 batches per core, 2-batch blocks. Host packs x as
split-fp8 (hi + lo e4m3, error-feedback residual) in the exact SBUF tile
layout (one DMA per block), quantizes qk/v weights to fp8e4m3 (x64 pre-scale),
keeps proj in bf16, folds v_bias into the projection bias, pre-gathers
exp(rel_pos_bias), and folds the attention scale into the exp's scale
immediate (SCALE/QS^2).

Device pipeline per core, per 2-batch block:
  qkT  [1536, 394] f32->fp8 = qkw8.T @ (x8h + x8l)   (DoubleRow fp8 gemm,
       K=256/inst, 0.5 cyc/col; psum copies write fp8e4m3 directly)
  repack: 12 SBUF->SBUF DMAs reshape hd=64 into the DoubleRow [32, 2] packing
       (partition base = hp%3 since only bases 0/32/64 are PE-addressable)
  v    [394, 768]  = (x8h+x8l) @ (vw8h+vw8l)  (3 of 4 cross terms, DoubleRow;
       more accurate than bf16 and 25% cheaper)
  S.T  [197, 197]  per head = one DoubleRow fp8 matmul (q/k fp8e4m3)
  E    = exp(S.T * SCALE/QS^2) * exp_rb   (both heads of a pair share one
       2-bank psum: one wide ACT exp + one wide DVE multiply per k-chunk)
  outT [128, 197]  = v.T @ E with softmax col-sums via ones-matmuls into the
       same psum; ACT reciprocal + DVE normalize-multiply
  out  = attnoutT.T @ proj_wT * (1/QS) + bias  (bf16, per 4-batch superblock)

Software pipelining: PE executes in order, so next-block qk/v gemms and ready
projection m-tiles are interleaved as time-paced "filler" between the scores
and PV groups of each attention window, and PV lags scores/exp by LAG
head-pairs so ACT's E production never starves the PE.
"""
import sys
sys.path.insert(0, '/opt/trn_rl_repo')

import numpy as np
import ml_dtypes
from contextlib import ExitStack

import concourse.bass as bass
import concourse.tile as tile
from concourse import mybir
from concourse.bass_utils import run_bass_kernel_spmd
from concourse.vector_clock import ScopedClock, VectorClock

f32 = mybir.dt.float32
f32r = mybir.dt.float32r
bf16 = mybir.dt.bfloat16
f8 = mybir.dt.float8e4
DR = mybir.MatmulPerfMode.DoubleRow

N_CORES = 8
RB_MODE = "ident_pe"
B, N, C, H, HD = 128, 197, 768, 12, 64
BC = B // N_CORES          # batches per core
BLK = 2                    # batches per block
NB = BC // BLK             # blocks per core
NP = BLK * N               # block column width (394)
NPP = 400                  # xt8 tile pitch (DR ldweights needs step%16==0)
LAG = 4                    # head-pairs of scores/exp lookahead before PV
SCALE = HD ** -0.5
QS = 64.0                  # fp8 weight pre-scale for the qk gemm
EXP_SCALE = SCALE / (QS * QS)


class TileContextFixed(tile.TileContext):
    """The walrus in this container accepts at most ONE sync wait per
    instruction. Stock Tile attaches several (both on ordinary instructions
    during wait assignment and on the tail drain). Split the extras onto
    same-engine InstNoOps, and emit the tail drain one proc at a time."""

    def _lower_ordered_insts(self, ordered):
        for bb_name, insts in ordered.items():
            i = 0
            while i < len(insts):
                inst = insts[i]
                si = inst.sync_info
                if si is not None and si.on_wait and len(si.on_wait) > 1:
                    waits = list(si.on_wait)
                    inst.sync_info = mybir.SyncInfo(
                        on_wait=[waits[-1]], on_update=list(si.on_update)
                    )
                    nops = [
                        mybir.InstNoOp(
                            name=f"{inst.name}__wsplit{k}",
                            engine=inst.engine,
                            bass_nofuse=True,
                            sync_info=mybir.SyncInfo(on_wait=[w], on_update=[]),
                        )
                        for k, w in enumerate(waits[:-1])
                    ]
                    insts[i:i] = nops
                    i += len(nops)
                i += 1
        return super()._lower_ordered_insts(ordered)

    def _drain_and_barrier(self, tick_clock, wait_clock):
        gc = tick_clock.global_clock
        n = len(gc)
        for i in range(n):
            if gc[i] > 0:
                vc = VectorClock([0] * n)
                vc.require_at_least(i, gc[i])
                d = self.nc.sync.drain()
                wait_clock.add_sem_waits(d.ins, ScopedClock({None: vc}))
        self.nc.all_engine_barrier()
        assert self.sems is not None
        popped = self.nc._tile_sem_poison_stack.pop()
        assert popped is self._sem_poison
        self.nc.clear_and_free_semaphores(list(self.sems.allocated().values()))
        self.nc.all_engine_barrier()


def _act_recip(eng, out, in_):
    imm = lambda v: mybir.ImmediateValue(dtype=f32, value=v)
    return eng.add_instruction(mybir.InstActivation(
        name=eng.bass.get_next_instruction_name(),
        func=mybir.ActivationFunctionType.Reciprocal,
        ins=[eng.lower_ap(in_), imm(0.0), imm(1.0), imm(0.0)],
        outs=[eng.lower_ap(out)],
    ))


def build_nc(rb_mode=RB_MODE, patt_bufs=4, pmm_bufs=2, ppv_bufs=2, e_bufs=10):
    # rb_mode: how exp(S+rb) is formed:
    #   "mul_pool"  E = exp(S) * erb on gpsimd
    #   "mul_dve"   E = exp(S) * erb on DVE
    #   "mul_split" alternate gpsimd/DVE by head parity
    #   "ident_pe"  S += rb via identity matmul on PE, E = exp(S)
    nc = bass.Bass("TRN2", target_bir_lowering=False, debug=False)
    Exp = mybir.ActivationFunctionType.Exp

    xT8_d = nc.dram_tensor("xT8", [NB, 128, 2, 6, NP], f8, kind="ExternalInput").ap()
    qkw_d = nc.dram_tensor("qkw", [C, 2 * C], f8, kind="ExternalInput").ap()
    vw_d = nc.dram_tensor("vw", [2, C, C], f8, kind="ExternalInput").ap()
    pw_d = nc.dram_tensor("pw", [C, C], bf16, kind="ExternalInput").ap()
    pb_d = nc.dram_tensor("pb", [1, C], f32, kind="ExternalInput").ap()
    qb_d = nc.dram_tensor("qb", [128, 6], f32, kind="ExternalInput").ap()

    erb_d = nc.dram_tensor("erb", [H, N, NP], bf16, kind="ExternalInput").ap()
    out_d = nc.dram_tensor("out", [BC * N, C], f32, kind="ExternalOutput").ap()

    MT = ((0, 128), (128, 69))  # (row offset, rows) m-tiles of 197

    with TileContextFixed(nc) as tc, ExitStack() as ctx:
        consts = ctx.enter_context(tc.tile_pool(name="consts", bufs=1))
        xt8_p = ctx.enter_context(tc.tile_pool(name="xt8", bufs=3))
        qkt_p = ctx.enter_context(tc.tile_pool(name="qkt", bufs=4))
        v_p = ctx.enter_context(tc.tile_pool(name="v", bufs=2))
        at_p = ctx.enter_context(tc.tile_pool(name="at", bufs=2))
        e_p = ctx.enter_context(tc.tile_pool(name="e", bufs=e_bufs))
        rcp_p = ctx.enter_context(tc.tile_pool(name="rcp", bufs=4))
        stage_p = ctx.enter_context(tc.tile_pool(name="stage", bufs=3))
        pmm = ctx.enter_context(tc.tile_pool(name="pmm", bufs=pmm_bufs, space="PSUM"))
        patt = ctx.enter_context(tc.tile_pool(name="patt", bufs=patt_bufs, space="PSUM"))
        ppv = ctx.enter_context(tc.tile_pool(name="ppv", bufs=ppv_bufs, space="PSUM"))

        # One serial DMA stream (sync queue), ordered by first consumption:
        # qk weights (chunked by mi group), split-fp8 x for block 0, q bias,
        # split-fp8 v weights, exp(rel-bias), then later blocks / proj consts.
        qkw_s = consts.tile([128, 6, 2 * C], f8)
        qkw_r = qkw_d.rearrange("(k p) c -> p k c", p=128)
        nc.sync.dma_start(out=qkw_s[:, :, 0:512], in_=qkw_r[:, :, 0:512])
        xt8_pre = xt8_p.tile([128, 2, 6, NPP], f8)
        nc.sync.dma_start(out=xt8_pre[:, 0, :, 0:NP], in_=xT8_d[0][:, 0])
        qb_s = consts.tile([128, 6], f32)
        nc.sync.dma_start(out=qb_s[:], in_=qb_d[:])
        nc.sync.dma_start(out=xt8_pre[:, 1, :, 0:NP], in_=xT8_d[0][:, 1])
        nc.sync.dma_start(out=qkw_s[:, :, 512:1024], in_=qkw_r[:, :, 512:1024])
        nc.sync.dma_start(out=qkw_s[:, :, 1024:1536], in_=qkw_r[:, :, 1024:1536])
        vw_s = consts.tile([128, 2, 6, C], f8)
        for s in range(2):
            nc.sync.dma_start(out=vw_s[:, s],
                              in_=vw_d[s].rearrange("(k p) c -> p k c", p=128))
        xt8_b1 = xt8_p.tile([128, 2, 6, NPP], f8)
        nc.sync.dma_start(out=xt8_b1[:, :, :, 0:NP], in_=xT8_d[1])
        erb0_s = consts.tile([128, H, NP], bf16)
        erb1_s = consts.tile([69, H, NP], bf16)
        nc.sync.dma_start(out=erb0_s[:], in_=erb_d[:, 0:128, :].rearrange("h p n -> p h n"))
        nc.sync.dma_start(out=erb1_s[:], in_=erb_d[:, 128:197, :].rearrange("h p n -> p h n"))
        pw_s = consts.tile([128, 6, C], bf16)
        pbb_s = consts.tile([128, C], f32)
        ones64 = consts.tile([128, 64], bf16)
        nc.gpsimd.memset(ones64[:], 1.0)

        SB = NB // 2                      # superblocks of 4 batches
        MT7 = [(g, min(128, 2 * NP - g)) for g in range(0, 2 * NP, 128)]

        # ---------- emission helpers (software pipelining) ----------
        # PE executes its instruction stream in order, so filler work
        # (next block's qk/v gemms, ready proj m-tiles) is interleaved into
        # the attention emission to keep PE busy while ACT produces E.

        def emit_dma(blk):
            xt8_s = xt8_p.tile([128, 2, 6, NPP], f8)
            nc.sync.dma_start(out=xt8_s[:, :, :, 0:NP], in_=xT8_d[blk])
            return xt8_s

        def emit_qk_mi(xt8_s, qkt8, mi):
            # qkT [128, NP] for one mi-tile via split-fp8 DoubleRow gemm;
            # q/k scaled by QS=64, rescale folded into the exp
            ps = pmm.tile([128, NP], f32, tag="pmm")
            for s in range(2):
                for t in range(3):
                    nc.tensor.matmul(
                        ps[:],
                        lhsT=qkw_s[:, 2 * t:2 * t + 2, mi * 128:(mi + 1) * 128],
                        rhs=xt8_s[:, s, 2 * t:2 * t + 2, 0:NP],
                        start=(s == 0 and t == 0), stop=(s == 1 and t == 2),
                        perf_mode=DR,
                    )
            if mi < 6:
                if mi % 2:
                    nc.scalar.add(qkt8[:, mi, 0:NP], ps[:], qb_s[:, mi:mi + 1])
                else:
                    nc.vector.tensor_scalar_add(qkt8[:, mi, 0:NP], ps[:],
                                                qb_s[:, mi:mi + 1])
            else:
                if mi % 2:
                    nc.scalar.copy(out=qkt8[:, mi, 0:NP], in_=ps[:])
                else:
                    nc.vector.tensor_copy(out=qkt8[:, mi, 0:NP], in_=ps[:])

        def emit_v_tile(xt8_s, v_s, j, t, nt):
            # v natural [msz, 6 heads x 64] via split-fp8 gemm
            # (hi*hi + hi*lo + lo*hi; the lo*lo term is negligible)
            r0, msz = MT[t]
            ps = pmm.tile([128, 384], f32, tag="pmm")
            for pi, (sx, sv) in enumerate(((0, 0), (1, 0), (0, 1))):
                for kt in range(3):
                    nc.tensor.matmul(
                        ps[0:msz, :],
                        lhsT=xt8_s[:, sx, 2 * kt:2 * kt + 2,
                                   j * N + r0: j * N + r0 + msz],
                        rhs=vw_s[:, sv, 2 * kt:2 * kt + 2, nt * 384:(nt + 1) * 384],
                        start=(pi == 0 and kt == 0), stop=(pi == 2 and kt == 2),
                        perf_mode=DR,
                    )
            nc.vector.tensor_copy(
                out=v_s[0:msz, j, t, nt * 6:(nt + 1) * 6, :],
                in_=ps[0:msz, :].rearrange("p (h d) -> p h d", h=6),
            )

        def emit_repack(qkt8, qkt8r, b):
            # reshape hd=64 (on partitions) into the DoubleRow [32 x 2] packing.
            # Partition base b = hp%3 (only bases 0/32/64 are addressable by
            # the PE); free slots (mi//3, head parity, d-high) hold the rest.
            # One base-group per call, on the software-DGE queue, emitted as
            # soon as its last prerequisite qk tile (mi=9+b) is in.
            for h2 in range(2):
                for dhi in range(2):
                    p0 = 64 * h2 + 32 * dhi
                    nc.sync.dma_start(
                        out=qkt8r[32 * b:32 * b + 32, :, h2, dhi, 0:NP],
                        in_=qkt8[p0:p0 + 32, b::3, 0:NP],
                    )

        def emit_proj_tile(at_s, sb, g0, msz):
            # one m-tile of the superblock projection, + bias, + 1/QS rescale
            stage = stage_p.tile([128, C], f32)
            for nt in range(2):
                ps = pmm.tile([128, 384], f32, tag="pmm")
                for k in range(6):
                    nc.tensor.matmul(
                        ps[0:msz, :],
                        lhsT=at_s[:, k, g0:g0 + msz],
                        rhs=pw_s[:, k, nt * 384:(nt + 1) * 384],
                        start=(k == 0), stop=(k == 5),
                    )
                nc.vector.scalar_tensor_tensor(
                    out=stage[0:msz, nt * 384:(nt + 1) * 384],
                    in0=ps[0:msz, :], scalar=1.0 / QS,
                    in1=pbb_s[0:msz, nt * 384:(nt + 1) * 384],
                    op0=mybir.AluOpType.mult, op1=mybir.AluOpType.add,
                )
            nc.sync.dma_start(
                out=out_d[sb * 2 * NP + g0: sb * 2 * NP + g0 + msz, :],
                in_=stage[0:msz, :],
            )

        def emit_pv(hp, es, v_s, at_s, off_):
            for j in range(BLK):
                ps_o = ppv.tile([128, 2 * N], f32, tag="ppv")
                for hi in range(2):
                    h = 2 * hp + hi
                    for t, (r0, msz) in enumerate(MT):
                        nc.tensor.matmul(
                            ps_o[hi * 64:(hi + 1) * 64, 0:N],
                            lhsT=v_s[0:msz, j, t, h, :],
                            rhs=es[t][0:msz, hi, j * N:(j + 1) * N],
                            start=(t == 0), stop=(t == 1),
                            skip_group_check=True,
                        )
                for hi in range(2):
                    for t, (r0, msz) in enumerate(MT):
                        nc.tensor.matmul(
                            ps_o[hi * 64:(hi + 1) * 64, N:2 * N],
                            lhsT=ones64[0:msz, :],
                            rhs=es[t][0:msz, hi, j * N:(j + 1) * N],
                            start=(t == 0), stop=(t == 1),
                            skip_group_check=True,
                        )
                rcp = rcp_p.tile([128, N], f32, tag="rcp")
                nc.vector.reciprocal(out=rcp[:], in_=ps_o[:, N:2 * N])
                nc.vector.tensor_mul(
                    at_s[:, hp, off_ + j * N:off_ + (j + 1) * N],
                    ps_o[:, 0:N], rcp[:],
                )

        def emit_attn(qkt8r, v_s, at_s, off, filler, pvq):
            # scores psum holds both batches of the half-block as two CLOSED
            # groups; one exp + one exp(rb)-multiply per psum. Filler closures
            # are drained between the scores and PV groups of each head-pair.
            wts = (1.0, 1.0, 1.0, 1.0, 1.0, 1.0)
            tot = sum(c for c, _ in filler) if filler else 0.0

            def emit_scores(hp):
                es = {}
                for t, (r0, msz) in enumerate(MT):
                    erb_t = erb0_s if t == 0 else erb1_s
                    # per-head single-bank psum tiles: finer ring granularity
                    # at window boundaries; one exp per head, one wide
                    # exp(rb)-multiply over both halves
                    e = e_p.tile([128, 2, NP], bf16, tag="e")
                    for hi in range(2):
                        pt = patt.tile([128, 512], f32, tag="patt")
                        for j in range(BLK):
                            b = 32 * (hp % 3)
                            nc.tensor.matmul(
                                pt[0:msz, j * N:(j + 1) * N],
                                lhsT=qkt8r[b:b + 32, 2 + hp // 3, hi, :,
                                           j * N + r0: j * N + r0 + msz],
                                rhs=qkt8r[b:b + 32, hp // 3, hi, :,
                                          j * N:(j + 1) * N],
                                start=True, stop=True, skip_group_check=True,
                                perf_mode=DR,
                            )
                        nc.scalar.activation(out=e[0:msz, hi, :], in_=pt[0:msz, 0:NP],
                                             func=Exp, scale=EXP_SCALE)
                    nc.vector.tensor_mul(e[0:msz, :, :], e[0:msz, :, :],
                                         erb_t[0:msz, 2 * hp:2 * hp + 2, :])
                    es[t] = e
                return es

            # PV lags scores/exp by one head-pair so E production has a full
            # slot of slack before its consumer, and the recips never delay
            # the next exp on the in-order ACT queue.
            for hp in range(6):
                es = emit_scores(hp)
                if len(pvq) >= LAG:
                    emit_pv(*pvq.pop(0))
                acc = 0.0
                budget = tot * wts[hp] / 6.0
                while filler and acc < budget:
                    c, fn = filler.pop(0)
                    fn()
                    acc += c
                pvq.append((hp, es, v_s, at_s, off))
            while filler:
                filler.pop(0)[1]()

        # ---------- pipelined emission ----------
        blk_tiles = {}

        def make_blk_items(blk, xt8_s):
            qkt8 = qkt_p.tile([128, H, 400], f8)
            qkt8r = qkt_p.tile([96, 4, 2, 2, 400], f8, name="qkt8r", tag="qkt")
            v_s = v_p.tile([128, BLK, 2, H, 64], bf16)
            blk_tiles[blk] = (qkt8r, v_s)
            def qk_item(mi):
                emit_qk_mi(xt8_s, qkt8, mi)
                if mi >= 9:
                    emit_repack(qkt8, qkt8r, mi - 9)
            items = [
                (0.25, lambda mi=mi: qk_item(mi)) for mi in range(12)
            ] + [
                (0.72, lambda j=j, t=t, nt=nt: emit_v_tile(xt8_s, v_s, j, t, nt))
                for nt in range(2) for j in range(BLK) for t in range(2)
            ]
            return items, []  # defer disabled

        # block 0 is mostly emitted straight (nothing to interleave into)
        items0, defer0 = make_blk_items(0, xt8_pre)
        for _, it in items0:
            it()

        pending = list(defer0)
        pvq = []
        at_tiles = {}
        xt8_tiles = {0: xt8_pre, 1: xt8_b1}
        nc.sync.dma_start(
            out=pw_s[:], in_=pw_d.rearrange("(k p) c -> p k c", p=128))
        nc.sync.dma_start(
            out=pbb_s[:], in_=bass.AP(tensor=pb_d.tensor, offset=0,
                                      ap=[[0, 128], [1, C]]))
        for hb in range(NB):
            sb, bh = divmod(hb, 2)
            if bh == 0:
                at_tiles[sb] = at_p.tile([128, 6, 2 * NP], bf16, name="at_s", tag="at_s")
            filler = list(pending) if hb == 0 else []
            if hb + 1 < NB:
                if hb + 1 not in xt8_tiles:
                    xt8_tiles[hb + 1] = emit_dma(hb + 1)
                fi, _ = make_blk_items(hb + 1, xt8_tiles[hb + 1])
                filler += fi
            if hb != 0:
                filler += pending
            pending = []
            qkt8r, v_s = blk_tiles[hb]
            emit_attn(qkt8r, v_s, at_tiles[sb], bh * NP, filler, pvq)
            while pvq:
                emit_pv(*pvq.pop(0))
            if bh == 0:
                # proj m-tiles fully inside this half-block's columns
                pending += [
                    (0.96, lambda sb=sb, g0=g0, msz=msz:
                     emit_proj_tile(at_tiles[sb], sb, g0, msz))
                    for g0, msz in MT7 if g0 + msz <= NP
                ]
            else:
                pending += [
                    (0.96, lambda sb=sb, g0=g0, msz=msz:
                     emit_proj_tile(at_tiles[sb], sb, g0, msz))
                    for g0, msz in MT7 if g0 + msz > NP
                ]
        while pvq:
            emit_pv(*pvq.pop(0))
        for _, it in pending:
            it()
    return nc


_NC = None


def _get_nc():
    global _NC
    if _NC is None:
        _NC = build_nc()
    return _NC


_EXEC = None


def _get_exec():
    """Build the sharded PJRT executable once and reuse it across calls
    (run_bass_via_pjrt re-traces jax.jit on every invocation)."""
    global _EXEC
    if _EXEC is not None:
        return _EXEC
    import jax
    import numpy as _np
    from jax.sharding import Mesh, PartitionSpec
    from jax.experimental.shard_map import shard_map
    import concourse.mybir as mybir_
    from concourse import bass2jax

    nc = _get_nc()
    bass2jax.install_neuronx_cc_hook()
    partition_name = nc.partition_id_tensor.name if nc.partition_id_tensor else None
    in_names, out_names, out_avals = [], [], []
    for alloc in nc.m.functions[0].allocations:
        if not isinstance(alloc, mybir_.MemoryLocationSet):
            continue
        name = alloc.memorylocations[0].name
        if alloc.kind == "ExternalInput":
            if name != partition_name:
                in_names.append(name)
        elif alloc.kind == "ExternalOutput":
            out_names.append(name)
            out_avals.append(jax.core.ShapedArray(
                tuple(alloc.tensor_shape), mybir_.dt.np(alloc.dtype)))
    all_names = list(in_names)
    if partition_name is not None:
        all_names = all_names + [partition_name]

    def _body(*args):
        operands = list(args)
        if partition_name is not None:
            operands.append(bass2jax.partition_id_tensor())
        outs = bass2jax._bass_exec_p.bind(
            *operands,
            out_avals=tuple(out_avals),
            in_names=tuple(all_names),
            out_names=tuple(out_names),
            lowering_input_output_aliases=(),
            sim_require_finite=True,
            sim_require_nnan=True,
            nc=nc,
        )
        return tuple(outs)

    devices = jax.devices()[:N_CORES]
    mesh = Mesh(_np.asarray(devices), ("core",))
    # xT is data-parallel (split on axis 0); every other input is replicated,
    # so it uploads once instead of 8x.
    in_specs = tuple(
        PartitionSpec("core") if name == "xT8" else PartitionSpec()
        for name in in_names
    )
    out_specs = (PartitionSpec("core"),) * len(out_avals)
    sharded = jax.jit(
        shard_map(_body, mesh=mesh, in_specs=in_specs, out_specs=out_specs,
                  check_rep=False),
        keep_unused=True,
    )
    _EXEC = (sharded, in_names, out_names, out_avals)
    return _EXEC


def _prep_host(x, qkv_w, q_bias, v_bias, rel_pos_table, proj_w, proj_b, rel_index,
               rb_mode="mul_pool"):
    x = np.asarray(x, np.float32)
    qkv_w = np.asarray(qkv_w, np.float32)
    xT32 = np.ascontiguousarray(x.transpose(0, 2, 1))          # [B, C, N]
    # split-fp8 x packed to the SBUF tile layout [blk, p, s, k, j, n] so each
    # 2-batch block is ONE contiguous DMA
    x8h = xT32.astype(ml_dtypes.float8_e4m3)
    x8l = (xT32 - x8h.astype(np.float32)).astype(ml_dtypes.float8_e4m3)
    x8 = np.stack([x8h, x8l], axis=1)                          # [B, 2, C, N]
    x8p = x8.reshape(B // 2, 2, 2, 6, 128, N).transpose(0, 4, 2, 3, 1, 5)
    xT8 = np.ascontiguousarray(x8p.reshape(B // 2, 128, 2, 6, 2 * N))
    qk_wT = np.ascontiguousarray(qkv_w[:2 * C].T) * QS         # [C, 2C]
    qk_wT = qk_wT.astype(ml_dtypes.float8_e4m3)
    qb = (np.asarray(q_bias, np.float32) * QS).reshape(6, 128).T.copy()  # [128, 6]
    vw64 = np.ascontiguousarray(qkv_w[2 * C:].T) * QS          # [C, C]
    vwh = vw64.astype(ml_dtypes.float8_e4m3)
    vwl = (vw64 - vwh.astype(np.float32)).astype(ml_dtypes.float8_e4m3)
    v_wT = np.ascontiguousarray(np.stack([vwh, vwl], axis=0))  # [2, C, C]
    proj_wT = np.ascontiguousarray(np.asarray(proj_w, np.float32).T).astype(ml_dtypes.bfloat16)
    pb_eff = (np.asarray(proj_b, np.float32)
              + np.asarray(proj_w, np.float32) @ np.asarray(v_bias, np.float32))
    rb = np.asarray(rel_pos_table, np.float32)[
        np.asarray(rel_index).reshape(-1)].reshape(N, N, H)    # [n, m, h]
    rbT = np.exp(rb.transpose(2, 1, 0))
    rbT = np.concatenate([rbT] * BLK, axis=2)
    erbT = rbT.astype(ml_dtypes.bfloat16)
    return xT8, qk_wT, qb, v_wT, proj_wT, pb_eff.reshape(1, C), erbT


def kernel(x, qkv_w, q_bias, v_bias, rel_pos_table, proj_w, proj_b, rel_index):
    xT8, qk_wT, qb, v_wT, proj_wT, pb_eff, erbT = _prep_host(
        x, qkv_w, q_bias, v_bias, rel_pos_table, proj_w, proj_b, rel_index,
        rb_mode=RB_MODE)
    per_core = {
        "xT8": xT8,                                 # split on axis 0
        "qkw": qk_wT, "vw": v_wT, "pw": proj_wT,
        "pb": pb_eff, "qb": qb, "erb": erbT,
    }
    try:
        sharded, in_names, out_names, out_avals = _get_exec()
        concat_in = [np.ascontiguousarray(per_core[name]) for name in in_names]
        out_arrs = sharded(*concat_in)
        out = np.asarray(out_arrs[out_names.index("out")]).reshape(B, N, C)
    except Exception:
        # Robust fallback: the stock SPMD runner (slower per call, same NEFF).
        in_maps = []
        for c in range(N_CORES):
            m = {k: v for k, v in per_core.items() if k != "xT8"}
            m["xT8"] = np.ascontiguousarray(xT8[c * NB:(c + 1) * NB])
            in_maps.append(m)
        res = run_bass_kernel_spmd(_get_nc(), in_maps, core_ids=list(range(N_CORES)))
        out = np.concatenate(
            [res.results[c]["out"].reshape(BC, N, C) for c in range(N_CORES)], axis=0)
    return out.astype(np.float32)

